# revision 1
# baseline (speedup 1.0000x reference)
"""Trainium2 Bass kernel v2 for nn_DeformableUpsampleBlock (fixed instance).

Same algorithm family as the baseline (quad-parity gather tables + on-PE
einsums), restructured around the TimelineSim cost model:
  - x loaded once ([64, HW]); conv3x3 uses K=64 contraction (no shifted copy)
  - pad-strip memsets instead of full-tile memsets
  - bilinear corner coefficients stored pair-duplicated so the big combine
    multiplies hit the DVE 2x fast mode; all combine work on DVE
  - second deform's table split into x/mid halves so the x half is built
    during phase C and only the mid half sits on the post-einsum tail
  - einsum1 / yd evacuation / output DMA pipelined per chunk
"""

import numpy as np

import concourse.bass as bass
import concourse.mybir as mybir
from concourse import bacc
import concourse.tile as tile
from concourse.bass_utils import run_bass_kernel_spmd
from concourse.masks import make_identity

F32 = mybir.dt.float32
F16 = mybir.dt.float16
I16 = mybir.dt.int16
AF = mybir.ActivationFunctionType
ALU = mybir.AluOpType

H = W = 64
HW = H * W              # 4096
NCH = 32                # pixel chunks of 128; pixel p -> [p % 128, p // 128]
PAD3 = 3
PP3 = H + 2 * PAD3      # 70
PAD1 = 2
PP1 = H + 2 * PAD1      # 68
NB = 34                 # quad blocks per side (both tables)
NROW = 4 * NB * NB      # 4624
HT3_COLS = 4992         # 39*128 >= 70*70 (+ quad-build overread)
HT1_COLS = 4864         # 38*128; quad build reads to 4761
EPS = 1e-5


# --------------------------------------------------------------------------
# host-side constants
# --------------------------------------------------------------------------

def _f16(a):
    return np.ascontiguousarray(a).astype(np.float16)


def host_constants(p):
    c = {}
    inv3 = (1.0 / np.sqrt(p['bn3_var'].astype(np.float64) + EPS)).astype(np.float32)
    s3 = (p['bn3_gamma'] * inv3).astype(np.float32)
    t3 = (p['bn3_beta'] - p['bn3_mean'] * s3).astype(np.float32)
    c['s3'] = s3.reshape(64, 1).copy()
    c['t3'] = t3.reshape(64, 1).copy()

    inv1 = (1.0 / np.sqrt(p['bn1_var'].astype(np.float64) + EPS)).astype(np.float32)
    s1 = (p['bn1_gamma'] * inv1).astype(np.float32)
    t1 = (p['bn1_beta'] - p['bn1_mean'] * s1).astype(np.float32)
    c['s1x'] = s1[:64].reshape(64, 1).copy()
    c['t1x'] = t1[:64].reshape(64, 1).copy()
    c['s1m'] = s1[64:].reshape(64, 1).copy()
    c['t1m'] = (t1[64:] + s1[64:] * p['b_d3']).reshape(64, 1).astype(np.float32)

    w3 = p['w_off3'].astype(np.float32)          # [18, 64, 3, 3]
    wC = np.zeros((128, 162), np.float32)
    for ky in range(3):
        for kx in range(3):
            g = 3 * ky + kx
            wC[:64, 18 * g:18 * g + 18] = w3[:, :, ky, kx].T
    c['wC'] = _f16(wC)
    c['boff3'] = p['b_off3'].astype(np.float32).reshape(18, 1)
    c['boff1'] = p['b_off1'].astype(np.float32).reshape(2, 1)

    wd3 = p['w_d3'].astype(np.float32).reshape(64, 64, 9)    # [o, c, k]
    wt = np.zeros((128, 320), np.float32)
    for g in range(5):
        for part in range(128):
            kap = 128 * g + part
            if kap < 576:
                wt[part, 64 * g:64 * g + 64] = wd3[:, kap % 64, kap // 64]
    c['wd3T'] = _f16(wt)

    c['woff1T'] = _f16(p['w_off1'].reshape(2, 128).T)
    c['wd1T'] = _f16(p['w_d1'].reshape(32, 128).T)
    c['bd1'] = p['b_d1'].astype(np.float32).reshape(32, 1)

    part = np.arange(128)[:, None]
    chunk = np.arange(NCH)[None, :]
    pix = chunk * 128 + part
    ymap = (pix // W).astype(np.float32)
    xmap = (pix % W).astype(np.float32)
    yb3 = np.zeros((128, NCH, 9), np.float32)
    xb3 = np.zeros((128, NCH, 9), np.float32)
    for k in range(9):
        yb3[:, :, k] = ymap + (k // 3 + PAD3 - 2)
        xb3[:, :, k] = xmap + (k % 3 + PAD3 - 2)
    c['yb3'] = yb3.reshape(128, NCH * 9)
    c['xb3'] = xb3.reshape(128, NCH * 9)
    c['yb1'] = ymap + (PAD1 - 1)
    c['xb1'] = xmap + (PAD1 - 1)
    c['pb3y'] = np.mod(c['yb3'], 2.0)
    c['pb3x'] = np.mod(c['xb3'], 2.0)
    c['pb1y'] = np.mod(c['yb1'], 2.0)
    c['pb1x'] = np.mod(c['xb1'], 2.0)
    repl = np.zeros((128, 128), np.float32)
    for q in range(16):
        repl[q, q::16] = 1.0
    c['repl'] = repl
    return c


_VEC_SPECS = [   # [P<=128, 1] f32 per-partition vectors -> blob 'cvec'
    ('s3', 64), ('t3', 64), ('s1x', 64), ('t1x', 64), ('s1m', 64),
    ('t1m', 64), ('boff3', 18), ('boff1', 2), ('bd1', 32),
]
_MAP_SPECS = [   # [128, N] f32 coordinate maps -> blob 'cmap'
    ('yb3', 288), ('xb3', 288), ('pb3y', 288), ('pb3x', 288),
    ('yb1', 32), ('xb1', 32), ('pb1y', 32), ('pb1x', 32),
    ('repl', 128),
]
_W_SPECS = [     # [128, N] f16 weights -> blob 'cw'
    ('wC', 162), ('wd3T', 320), ('woff1T', 2), ('wd1T', 32),
]
CONST_SPECS = [
    ('cvec', (128, len(_VEC_SPECS)), F32),
    ('cmap', (128, sum(n for _, n in _MAP_SPECS)), F32),
    ('cw', (128, sum(n for _, n in _W_SPECS)), F16),
]


def pack_constants(c):
    cvec = np.zeros((128, len(_VEC_SPECS)), np.float32)
    for i, (n, p) in enumerate(_VEC_SPECS):
        cvec[:p, i] = c[n].reshape(-1)
    cmap = np.concatenate([c[n].reshape(128, sz) for n, sz in _MAP_SPECS], axis=1)
    cw = np.concatenate([c[n].reshape(128, sz) for n, sz in _W_SPECS],
                        axis=1).astype(np.float16)
    return {'cvec': cvec.astype(np.float32), 'cmap': cmap.astype(np.float32),
            'cw': cw}


# --------------------------------------------------------------------------
# AP helpers
# --------------------------------------------------------------------------

def _rows(ap2d, off, rstride, nr, ncols):
    """[P, nr, ncols] view of a [P, N] AP: rows of length ncols, stride rstride."""
    v = ap2d[:, off:off + nr * rstride].rearrange('p (r q) -> p r q', q=rstride)
    return v[:, :, 0:ncols]


# --------------------------------------------------------------------------
# device program
# --------------------------------------------------------------------------

def build_nc():
    nc = bacc.Bacc()
    x_in = nc.declare_dram_parameter('x', [64, HW], F32, isOutput=False)
    consts = {}
    for name, shape, dt in CONST_SPECS:
        consts[name] = nc.declare_dram_parameter('c_' + name, list(shape), dt,
                                                 isOutput=False)
    out_ext = nc.declare_dram_parameter('out', [32, 2 * H, 2 * W], F32,
                                        isOutput=True)

    hT_dram = nc.dram_tensor('hT_dram', [HT3_COLS, 64], F16)
    quad3 = nc.dram_tensor('quad3', [NROW, 256], F16)
    h1Tx = nc.dram_tensor('h1Tx', [HT1_COLS, 64], F16)
    h1Tm = nc.dram_tensor('h1Tm', [HT1_COLS, 64], F16)
    quad1x = nc.dram_tensor('quad1x', [NROW, 256], F16)
    quad1m = nc.dram_tensor('quad1m', [NROW, 256], F16)
    gate = nc.dram_tensor('gate', [1, 16], F16)

    with tile.TileContext(nc) as tc:
        _body(nc, tc, x_in, consts, out_ext, hT_dram, quad3,
              h1Tx, h1Tm, quad1x, quad1m, gate)
    nc.finalize()
    return nc


def _coords(nc, scratch, dyv, dxv, ybv, xbv, pbyv, pbxv, coef_out, row_out):
    """dyv/dxv/ybv/xbv/scratch: f32 views, identical free shape.
    coef_out: f16 view [.., 4, 2] (corner, dup last). row_out: i16 like dyv."""
    sy, sx, fy, fx, y0, x0, ta, tb = scratch
    V = nc.vector
    V.tensor_scalar(out=sy, in0=dyv, scalar1=0.0, scalar2=None, op0=ALU.is_ge)
    V.tensor_scalar(out=sx, in0=dxv, scalar1=0.0, scalar2=None, op0=ALU.is_ge)
    V.scalar_tensor_tensor(out=fy, in0=dyv, scalar=1.0, in1=sy,
                           op0=ALU.add, op1=ALU.subtract)
    V.scalar_tensor_tensor(out=fx, in0=dxv, scalar=1.0, in1=sx,
                           op0=ALU.add, op1=ALU.subtract)
    V.tensor_tensor(out=y0, in0=sy, in1=ybv, op=ALU.add)
    V.tensor_tensor(out=x0, in0=sx, in1=xbv, op=ALU.add)
    # parity ay = pby XOR sy = pby + sy - 2*pby*sy (pbyv = parity of base)
    V.tensor_tensor(out=ta, in0=pbyv, in1=sy, op=ALU.mult)
    V.tensor_tensor(out=tb, in0=pbyv, in1=sy, op=ALU.add)
    V.scalar_tensor_tensor(out=sy, in0=ta, scalar=-2.0, in1=tb,
                           op0=ALU.mult, op1=ALU.add)
    V.tensor_tensor(out=ta, in0=pbxv, in1=sx, op=ALU.mult)
    V.tensor_tensor(out=tb, in0=pbxv, in1=sx, op=ALU.add)
    V.scalar_tensor_tensor(out=sx, in0=ta, scalar=-2.0, in1=tb,
                           op0=ALU.mult, op1=ALU.add)
    # row = 17*Y0 + 0.5*X0 + 2295*ay + 1155.5*ax  (exact in f32)
    V.tensor_scalar(out=ta, in0=y0, scalar1=17.0, scalar2=None, op0=ALU.mult)
    V.scalar_tensor_tensor(out=ta, in0=sy, scalar=2295.0, in1=ta,
                           op0=ALU.mult, op1=ALU.add)
    V.tensor_scalar(out=tb, in0=x0, scalar1=0.5, scalar2=None, op0=ALU.mult)
    V.scalar_tensor_tensor(out=tb, in0=sx, scalar=1155.5, in1=tb,
                           op0=ALU.mult, op1=ALU.add)
    V.tensor_tensor(out=tb, in0=ta, in1=tb, op=ALU.add)
    V.tensor_copy(out=row_out, in_=tb)
    # corner coefficients; reuse sy/sx for (1-fy), (1-fx)
    V.tensor_scalar(out=sy, in0=fy, scalar1=-1.0, scalar2=1.0,
                    op0=ALU.mult, op1=ALU.add)
    V.tensor_scalar(out=sx, in0=fx, scalar1=-1.0, scalar2=1.0,
                    op0=ALU.mult, op1=ALU.add)
    nd = coef_out.ndim - 2
    for i, (a, b) in enumerate([(sy, sx), (sy, fx), (fy, sx), (fy, fx)]):
        for j in range(2):
            V.tensor_tensor(out=coef_out[(slice(None),) * nd + (i, j)],
                            in0=a, in1=b, op=ALU.mult)


def _wrap_idx(nc, rowi16_v, wrapped, eng=None):
    """rowi16_v: [128, nk, nch] i16 (contiguous) -> wrapped [128, nk, 256]:
    wrapped[q, k, chunk*8 + r] = row[16r+q, k, chunk], replicated to the 8
    16-partition groups."""
    eng = eng or nc.sync
    for r in range(8):
        eng.dma_start(out=wrapped[0:16, :, r::8],
                      in_=rowi16_v[16 * r:16 * r + 16, :, :])
    for gsz in (16, 32, 64):
        eng.dma_start(out=wrapped[gsz:2 * gsz, :, :],
                      in_=wrapped[0:gsz, :, :])


def _quad_build(nc, src_dram, dst_dram, pp, chans, b0=0, b1=NB,
                parities=None, eng=None):
    """DRAM->DRAM DMAs (3-dim APs) building the quad-parity block table
    for block rows by in [b0, b1); optionally only some (ay, ax) parities."""
    q = 2 * pp
    nb = b1 - b0
    for ay in range(2):
        for ax in range(2):
            if parities is not None and (ay, ax) not in parities:
                continue
            s = ay * 2 + ax
            for yy in range(2):
                r0 = (ay + yy) * pp + ax + b0 * q
                sv = src_dram[:][r0:r0 + nb * q, :] \
                    .rearrange('(by q) c -> by q c', q=q)[:, 0:2 * NB, :] \
                    .rearrange('by (bx xx) c -> by bx (xx c)', xx=2)
                dv = dst_dram[NB * NB * s + NB * b0:
                              NB * NB * s + NB * b1,
                              2 * chans * yy:2 * chans * (yy + 1)] \
                    .rearrange('(by bx) e -> by bx e', bx=NB)
                (eng or nc.sync).dma_start(out=dv, in_=sv)


def _pad_memset(nc, t, npart, pp, pad, w, ncols):
    """Zero only the padding cells of a padded image tile t [npart, ncols]."""
    head = pad * pp + pad
    nc.gpsimd.memset(t[0:npart, 0:head], 0.0)
    gapw = pp - w
    r0, r1 = pad, pad + w  # gap r covers trail of row r / lead of row r+1
    ngap = r1 - r0 - 1
    gv = _rows(t[0:npart, :], r0 * pp + pad + w, pp, ngap, gapw)
    nc.gpsimd.memset(gv, 0.0)
    tail0 = (r1 - 1) * pp + pad + w
    nc.gpsimd.memset(t[0:npart, tail0:ncols], 0.0)



def _body(nc, tc, x_in, consts, out_ext, hT_dram, quad3,
          h1Tx, h1Tm, quad1x, quad1m, gate):
    V, S, G, PE, SY = nc.vector, nc.scalar, nc.gpsimd, nc.tensor, nc.sync

    with (
        tc.tile_pool(name='persist', bufs=1) as pp,
        tc.tile_pool(name='psum', bufs=2, space='PSUM') as psp,
        tc.tile_pool(name='psumv', bufs=2, space='PSUM') as psv,
    ):
        # ---------------- constants / persistent tiles -------------------
        blobs = {}
        for name, shape, dt in CONST_SPECS:
            t = pp.tile(list(shape), dt, tag='c_' + name, name='c_' + name)
            blobs[name] = t
        C = {}
        for i, (n, p_) in enumerate(_VEC_SPECS):
            C[n] = blobs['cvec'][0:p_, i:i + 1]
        col = 0
        for n, sz in _MAP_SPECS:
            C[n] = blobs['cmap'][:, col:col + sz]
            col += sz
        col = 0
        for n, sz in _W_SPECS:
            C[n] = blobs['cw'][:, col:col + sz]
            col += sz
        idt32 = pp.tile([32, 32], F32, tag='idt32', name='idt32')
        idt128h = pp.tile([128, 128], F16, tag='idt128h', name='idt128h')
        hpad2 = pp.tile([64, HT3_COLS], F16, tag='hpad2', name='hpad2')
        h1pad = pp.tile([128, HT1_COLS], F16, tag='h1pad', name='h1pad')
        coefD = pp.tile([128, 9, NCH, 4, 2], F16, tag='coefD', name='coefD')
        coef1D = pp.tile([128, NCH, 4, 2], F16, tag='coef1D', name='coef1D')
        wrapped3 = pp.tile([128, 9, 256], I16, tag='wrapped3', name='wrapped3')
        wrapped1 = pp.tile([128, 1, 256], I16, tag='wrapped1', name='wrapped1')
        h1Tst = pp.tile([128, 37, 128], F16, tag='h1Tst', name='h1Tst')
        off3T = pp.tile([128, NCH, 18], F32, tag='off3T', name='off3T')
        off1T = pp.tile([128, NCH, 2], F32, tag='off1T', name='off1T')

        # ---------------- phase A: bn3, transposes, conv, coords ---------
        with tc.tile_pool(name='ph1', bufs=1) as ph1, \
             tc.tile_pool(name='oev', bufs=2) as oev:
            x2 = ph1.tile([64, HW], F32, tag='x2', name='x2')
            SY.dma_start(out=x2[:, 0:2048], in_=x_in[:][:, 0:2048])
            SY.dma_start(out=x2[:, 2048:HW], in_=x_in[:][:, 2048:HW])
            for name, shape, dt in CONST_SPECS:
                SY.dma_start(out=blobs[name][:, :], in_=consts[name][:])
            make_identity(nc, idt32[:, :])
            make_identity(nc, idt128h[:, :])
            _pad_memset(nc, hpad2, 64, PP3, PAD3, W, HT3_COLS)
            _pad_memset(nc, h1pad, 128, PP1, PAD1, W, HT1_COLS)
            xv = x2[:, :].rearrange('p (r c) -> p r c', r=H)

            # bn3 in two row-chunks (pipelines with the x DMA halves)
            for half in range(2):
                S.activation(_rows(hpad2[0:64, :],
                                   (PAD3 + 32 * half) * PP3 + PAD3,
                                   PP3, 32, W),
                             xv[0:64, 32 * half:32 * half + 32, :], AF.Relu,
                             bias=C['t3'], scale=C['s3'])

            # hT transposes: [64, 128] -> [128, 64] per 128-col chunk
            hTst = ph1.tile([128, 39, 64], F16, tag='hTst', name='hTst')
            for t0 in range(0, 39, 4):
                nt = min(4, 39 - t0)
                pv = psv.tile([128, 512], F16, tag='pv', name='pv')
                for j in range(nt):
                    PE.transpose(pv[:, 64 * j:64 * j + 64],
                                 hpad2[0:64,
                                       128 * (t0 + j):128 * (t0 + j) + 128],
                                 idt128h[0:64, 0:64])
                if (t0 // 4) % 2 == 1:
                    V.tensor_copy(out=hTst[:, t0:t0 + nt, :],
                                  in_=pv[:, 0:64 * nt])
                else:
                    S.activation(hTst[:, t0:t0 + nt, :],
                                 pv[:, 0:64 * nt]
                                 .rearrange('p (a b) -> p a b', b=64),
                                 AF.Identity)
            SY.dma_start(out=hT_dram[:].rearrange('(a p) c -> p a c', p=128),
                         in_=hTst[:, :, :])
            _quad_build(nc, hT_dram, quad3, PP3, 64)

            # conv3x3 (K=64): 2 blocks x 4 psum chunks x 9 taps
            for blk in range(2):
                pcs = [psp.tile([128, 512], F32, tag=f'pmm{i}', name=f'pc{i}',
                                bufs=1) for i in range(4)]
                for g in range(9):
                    ky, kx = g // 3, g % 3
                    for i in range(4):
                        ch = 4 * blk + i
                        rhs = _rows(hpad2[0:64, :],
                                    (2 + ky) * PP3 + 2 + kx + 8 * PP3 * ch,
                                    PP3, 8, W)
                        PE.matmul(pcs[i][0:18, :],
                                  C['wC'][0:64, 18 * g:18 * g + 18],
                                  rhs, start=(g == 0), stop=(g == 8))
                for i in range(4):
                    ch = 4 * blk + i
                    o3 = oev.tile([18, 512], F32, tag='o3', name='o3')
                    S.activation(o3[:, :], pcs[i][0:18, :], AF.Identity,
                                 bias=C['boff3'])
                    pt = psp.tile([128, 128], F32, tag='ptr', name='pt',
                                  bufs=1)
                    for t in range(4):
                        PE.transpose(pt[:, 18 * t:18 * t + 18],
                                     o3[:, 128 * t:128 * t + 128],
                                     idt32[0:18, 0:18])
                    V.tensor_copy(out=off3T[:, 4 * ch:4 * ch + 4, :]
                                  .rearrange('p a b -> p (a b)'),
                                  in_=pt[:, 0:72])

            # coordinates / coefficients / gather rows
            sc = [ph1.tile([128, 288], F32, tag=f'sc{i}', name=f'sc{i}')
                  for i in range(8)]
            rowf3 = ph1.tile([128, 2, 9, 16], F32, tag='rowf3', name='rowf3')
            w3f = ph1.tile([16, 2304], F32, tag='w3f', name='w3f')
            w3fv = w3f[:, :].rearrange('p (k c) -> p k c', k=9)
            scv = [s[:, :].rearrange('p (a b) -> p a b', b=9) for s in sc]
            for hf in range(2):
                cs = slice(16 * hf, 16 * hf + 16)
                _coords(nc,
                        [sv[:, cs, :] for sv in scv],
                        off3T[:, cs, 0:18:2], off3T[:, cs, 1:18:2],
                        C['yb3'].rearrange('p (a b) -> p a b', b=9)[:, cs, :],
                        C['xb3'].rearrange('p (a b) -> p a b', b=9)[:, cs, :],
                        C['pb3y'].rearrange('p (a b) -> p a b', b=9)[:, cs, :],
                        C['pb3x'].rearrange('p (a b) -> p a b', b=9)[:, cs, :],
                        coefD[:, :, cs, :, :].transpose([0, 2, 1, 3, 4]),
                        rowf3[:, hf, :, :].transpose([0, 2, 1]))
                w3fB = w3f[:, :].rearrange('p (h k c) -> p h k c',
                                           h=2, k=9)
                for r in range(8):
                    SY.dma_start(out=w3fB[0:16, hf, :, r::8],
                                 in_=rowf3[16 * r:16 * r + 16, hf, :, :])
            wr3v = wrapped3[:, :, :].rearrange('p k (h c) -> p h k c', h=2)
            for j in range(6):
                ptag = ['pmm0', 'pmm1', 'pmm2', 'pmm3', 'pc1', 'pmm0'][j]
                prep3 = psp.tile([128, 512], F32, tag=ptag, name='prep3',
                                 bufs=1)
                PE.matmul(prep3[:, 0:384], C['repl'][0:16, :],
                          w3f[0:16, 384 * j:384 * j + 384],
                          start=True, stop=True)
                V.tensor_copy(out=wr3v[:, j // 3, 3 * (j % 3):3 * (j % 3) + 3,
                                       :],
                              in_=prep3[:, 0:384])

            # h1 x-part: relu(bn1(x)) into h1pad interior (ACT, off path)
            S.activation(_rows(h1pad[0:64, :], PAD1 * PP1 + PAD1, PP1, H, W),
                         xv[0:64], AF.Relu, bias=C['t1x'], scale=C['s1x'])
            # h1Tst x-half transposes (PE idle window; evacs on ACT)
            for t0 in range(0, 37, 4):
                nt = min(4, 37 - t0)
                pv = psv.tile([128, 512], F16, tag='pv', name='pv')
                for j in range(nt):
                    PE.transpose(pv[:, 64 * j:64 * j + 64],
                                 h1pad[0:64,
                                       128 * (t0 + j):128 * (t0 + j) + 128],
                                 idt128h[0:64, 0:64])
                S.activation(h1Tst[:, t0:t0 + nt, 0:64], pv[:, 0:64 * nt],
                             AF.Identity)

        # ---------------- phase C: gathers + in-place combine ------------
        vhgs = {}
        with tc.tile_pool(name='gpool', bufs=4) as gp, \
             tc.tile_pool(name='vhp', bufs=3) as vhp:
            for pos, k in enumerate([0, 1, 8, 2, 3, 4, 5, 6, 7]):
                gp_i, sl = k // 2, k % 2
                if gp_i not in vhgs:
                    nt = 2 if gp_i < 4 else 1
                    tag = 'vh' if gp_i < 4 else 'vh4'
                    vhgs[gp_i] = vhp.tile([128, NCH, nt, 64], F16, tag=tag,
                                          name=f'vh{gp_i}')
                vhg = vhgs[gp_i]
                g = gp.tile([128, 8192], F16, tag='g', name='g')
                G.dma_gather(g[:, :].rearrange('p (a c) -> p a c', c=256),
                             quad3[:], wrapped3[:, k, :], 4096, 4096,
                             256, queue_num=0, single_packet=False)
                if pos == 8:
                    # gate: identity-rewrite of one h1Tst cell that reads
                    # tap 6's combine output, so the x-side table DMAs (which
                    # read h1Tst) can't preempt the gather stream
                    V.scalar_tensor_tensor(out=h1Tst[0:1, 0, 0:16],
                                           in0=vhgs[2][0:1, 0, 1, 0:16],
                                           scalar=0.0,
                                           in1=h1Tst[0:1, 0, 0:16],
                                           op0=ALU.mult, op1=ALU.add)
                    SY.dma_start(out=h1Tx[0:4736, :]
                                 .rearrange('(a p) c -> p a c', p=128),
                                 in_=h1Tst[:, :, 0:64])
                    _quad_build(nc, h1Tx, quad1x, PP1, 64)
                gq = g[:, :].rearrange('p (a b c d) -> p a b c d',
                                       a=NCH, b=4, c=32)
                cf = coefD[:, k][:, :, :, None, :].broadcast_to(
                    [128, NCH, 4, 32, 2])
                gw = g[:, :].rearrange('p (a b c) -> p a b c', a=NCH, b=4)
                nspl = 2 if pos == 8 else 1
                nh = NCH // nspl
                for sp in range(nspl):
                    chs = slice(nh * sp, nh * sp + nh)
                    V.tensor_tensor(out=gq[:, chs], in0=gq[:, chs],
                                    in1=cf[:, chs], op=ALU.mult)
                    with nc.allow_low_precision('fp16 middle precision'):
                        V.tensor_tensor(out=gw[:, chs, 0:2, :],
                                        in0=gw[:, chs, 0:2, :],
                                        in1=gw[:, chs, 2:4, :], op=ALU.add)
                        V.tensor_tensor(out=vhg[:, chs, sl, :],
                                        in0=gw[:, chs, 0, :],
                                        in1=gw[:, chs, 1, :], op=ALU.add)

            # ------------ phase C2/D: transposes + einsum3 ---------------
            with tc.tile_pool(name='vpool', bufs=1) as vp:
                v = vp.tile([128, 5, HW], F16, tag='v', name='v')
                pms = {}

                def _mm3(ch, gp_i):
                    if ch not in pms:
                        pms[ch] = psp.tile([128, 512], F32,
                                           tag=f'pmm{ch % 4}',
                                           name=f'pm{ch}', bufs=1)
                    if gp_i < 4:
                        PE.matmul(pms[ch][64:128, :],
                                  C['wd3T'][:, 64 * gp_i:64 * gp_i + 64],
                                  v[:, gp_i, 512 * ch:512 * ch + 512],
                                  start=(gp_i == 0), stop=False)
                    else:
                        PE.matmul(pms[ch][64:128, :],
                                  C['wd3T'][0:64, 256:320],
                                  v[0:64, 4, 512 * ch:512 * ch + 512],
                                  start=False, stop=True)

                for gp_i in (0, 4, 1, 2, 3):
                    for ch4 in range(8):
                        pv = psv.tile([128, 512], F16, tag='pv', name='pv')
                        for sub in range(4):
                            ch = 4 * ch4 + sub
                            if gp_i < 4:
                                PE.transpose(
                                    pv[:, 128 * sub:128 * sub + 128],
                                    vhgs[gp_i][:, ch, :, :]
                                    .rearrange('p a b -> p (a b)'),
                                    idt128h[:, :])
                            else:
                                PE.transpose(
                                    pv[0:64, 128 * sub:128 * sub + 128],
                                    vhgs[4][:, ch, 0, :], idt128h[:, :])
                        np_ = 128 if gp_i < 4 else 64
                        S.activation(v[0:np_, gp_i,
                                       512 * ch4:512 * ch4 + 512],
                                     pv[0:np_, :], AF.Identity)
                    for ch in range(6):
                        _mm3(ch, gp_i)

                # ---- phase E part 1: evacs 0..5, off1, h1T-m ------------
                estate = {'tch': 0, 'pt1': None}
                with tc.tile_pool(name='oev1', bufs=2) as oev1:

                    def _evac_chunk(ch):
                        hv = _rows(h1pad[64:128, :],
                                   (8 * ch + PAD1) * PP1 + PAD1, PP1, 8, W)
                        pmv = pms[ch][64:128, :] \
                            .rearrange('p (r c) -> p r c', r=8)
                        S.activation(hv, pmv, AF.Relu, bias=C['t1m'],
                                     scale=C['s1m'])
                        pc1 = psp.tile([128, 512], F32, tag='pc1',
                                       name='pc1', bufs=1)
                        PE.matmul(pc1[0:2, :], C['woff1T'],
                                  _rows(h1pad[0:128, :],
                                        (8 * ch + PAD1) * PP1 + PAD1,
                                        PP1, 8, W),
                                  start=True, stop=True)
                        o1 = oev1.tile([2, 512], F32, tag='o1', name='o1')
                        if ch % 2 == 0:
                            S.activation(o1[:, :], pc1[0:2, :], AF.Identity,
                                         bias=C['boff1'])
                        else:
                            V.tensor_scalar(out=o1[:, :], in0=pc1[0:2, :],
                                            scalar1=C['boff1'], scalar2=None,
                                            op0=ALU.add)
                        if ch % 2 == 0:
                            estate['pt1'] = psp.tile([128, 128], F32,
                                                     tag='ptr', name='pt1',
                                                     bufs=1)
                        pt1 = estate['pt1']
                        for t in range(4):
                            PE.transpose(pt1[:, 8 * (ch % 2) + 2 * t:
                                             8 * (ch % 2) + 2 * t + 2],
                                         o1[:, 128 * t:128 * t + 128],
                                         idt32[0:2, 0:2])
                        if ch % 2 == 1:
                            V.tensor_copy(out=off1T[:, 8 * (ch // 2):
                                                    8 * (ch // 2) + 8, :]
                                          .rearrange('p a b -> p (a b)'),
                                          in_=pt1[:, 0:16])
                        # h1Tst mid-half transposes ready with this chunk
                        r_hi_ready = 2 + 8 * ch + 8
                        ready = []
                        while estate['tch'] < 37:
                            tch = estate['tch']
                            r_hi = (128 * tch + 127) // PP1
                            if r_hi >= r_hi_ready and ch < 7:
                                break
                            ready.append(tch)
                            estate['tch'] += 1
                        for i0 in range(0, len(ready), 4):
                            grp = ready[i0:i0 + 4]
                            pv = psv.tile([128, 512], F16, tag='pv',
                                          name='pv')
                            for j, tch in enumerate(grp):
                                PE.transpose(pv[:, 64 * j:64 * j + 64],
                                             h1pad[64:128,
                                                   128 * tch:128 * tch + 128],
                                             idt128h[64:128, 64:128])
                            t0g = grp[0]
                            if (i0 // 4) % 2 == 0:
                                S.activation(h1Tst[:, t0g:t0g + len(grp),
                                                   64:128],
                                             pv[:, 0:64 * len(grp)]
                                             .rearrange('p (a b) -> p a b',
                                                        b=64),
                                             AF.Identity)
                            else:
                                V.tensor_copy(out=h1Tst[:, t0g:t0g + len(grp),
                                                        64:128],
                                              in_=pv[:, 0:64 * len(grp)])
                        if ch == 4:
                            SY.dma_start(out=h1Tm[0:2688, :]
                                         .rearrange('(a p) c -> p a c',
                                                    p=128),
                                         in_=h1Tst[:, 0:21, 64:128])
                            _quad_build(nc, h1Tm, quad1m, PP1, 64, 0, 18)

                    _evac_chunk(0)
                    _evac_chunk(1)
                    for ch in (6, 7):
                        for gp_i in (0, 4, 1, 2, 3):
                            _mm3(ch, gp_i)
                    for ch in range(2, 8):
                        _evac_chunk(ch)

        # ---------------- phase E part 2: tables + coords1 ---------------
        SY.dma_start(out=h1Tm[2688:4736, :]
                     .rearrange('(a p) c -> p a c', p=128),
                     in_=h1Tst[:, 21:37, 64:128])

        with tc.tile_pool(name='tailp', bufs=1) as tp:
            with tc.tile_pool(name='sc1p', bufs=1) as sc1p:
                sc1 = [sc1p.tile([128, 32], F32, tag=f't1s{i}',
                                 name=f't1s{i}') for i in range(8)]
                rowf = sc1p.tile([128, 32], F32, tag='rowf', name='rowf')
                w1f = sc1p.tile([16, 256], F32, tag='w1f', name='w1f')
                _coords(nc, [s[:, :] for s in sc1],
                        off1T[:, :, 0], off1T[:, :, 1],
                        C['yb1'], C['xb1'], C['pb1y'], C['pb1x'],
                        coef1D[:, :, :, :], rowf[:, :])
                for r in range(8):
                    SY.dma_start(out=w1f[0:16, r::8],
                                 in_=rowf[16 * r:16 * r + 16, :])
                prep = psp.tile([128, 512], F32, tag='pc1', name='prep',
                                bufs=1)
                PE.matmul(prep[:, 0:256], C['repl'][0:16, :], w1f[:, :],
                          start=True, stop=True)
                V.tensor_copy(out=wrapped1[:, 0, :], in_=prep[:, 0:256])
            _quad_build(nc, h1Tm, quad1m, PP1, 64, 18, NB)

            # gathers + in-place combine for the 1x1 deform
            vhat1 = tp.tile([128, NCH, 2, 64], F16, tag='vhat1',
                            name='vhat1')
            with tc.tile_pool(name='g1pool', bufs=3) as g1p:
                for hx, quad, half, nchh in ((0, quad1x, 0, 16),
                                             (0, quad1x, 1, 16),
                                             (1, quad1m, 0, 16),
                                             (1, quad1m, 1, 16)):
                    g1 = g1p.tile([128, 32, 4, 64], F16, tag='g1', name='g1')
                    g1v = g1[:, 0:nchh, :, :]
                    G.dma_gather(g1v.rearrange('p a b c -> p a (b c)'),
                                 quad[:],
                                 wrapped1[:, 0,
                                          128 * half:128 * (half + nchh // 16)],
                                 128 * nchh, 128 * nchh, 256, queue_num=0,
                                 single_packet=False)
                    gq = g1v.rearrange('p a b (c d) -> p a b c d', d=2)
                    cf = coef1D[:, 16 * half:16 * half + nchh][
                        :, :, :, None, :].broadcast_to(
                        [128, nchh, 4, 32, 2])
                    V.tensor_tensor(out=gq, in0=gq, in1=cf, op=ALU.mult)
                    with nc.allow_low_precision('fp16 by design'):
                        V.tensor_tensor(out=g1v[:, :, 0:2, :],
                                        in0=g1v[:, :, 0:2, :],
                                        in1=g1v[:, :, 2:4, :], op=ALU.add)
                        V.tensor_tensor(
                            out=vhat1[:, 16 * half:16 * half + nchh, hx, :],
                            in0=g1v[:, :, 0, :], in1=g1v[:, :, 1, :],
                            op=ALU.add)

            # ---------------- v1 transposes + einsum1 + upsample ---------
            yd = tp.tile([32, H, 2 * W], F32, tag='yd', name='yd')
            v1s = tp.tile([128, 8, 512], F16, tag='v1s', name='v1s')
            for ch4 in range(8):
                pv = psv.tile([128, 512], F16, tag='pv', name='pv')
                for sub in range(4):
                    PE.transpose(pv[:, 128 * sub:128 * sub + 128],
                                 vhat1[:, 4 * ch4 + sub, :, :]
                                 .rearrange('p a b -> p (a b)'),
                                 idt128h[:, :])
                if ch4 % 2 == 0:
                    S.activation(v1s[:, ch4, :], pv[:, :], AF.Identity)
                else:
                    V.tensor_copy(out=v1s[:, ch4, :], in_=pv[:, :])
                pmy = psp.tile([128, 512], F32, tag=f'pmm{ch4 % 4}',
                               name='pmy', bufs=1)
                PE.matmul(pmy[0:32, :], C['wd1T'], v1s[:, ch4, :],
                          start=True, stop=True)
                pmv = pmy[0:32, :].rearrange('p (r c) -> p r c', r=8)
                S.activation(yd[:, 8 * ch4:8 * ch4 + 8, 0::2], pmv,
                             AF.Identity, bias=C['bd1'])
                V.tensor_scalar(out=yd[:, 8 * ch4:8 * ch4 + 8, 1::2],
                                in0=pmv, scalar1=C['bd1'], scalar2=None,
                                op0=ALU.add)
                if ch4 % 2 == 1:
                    gq = ch4 // 2
                    SY.dma_start(out=out_ext[:, 32 * gq:32 * gq + 32:2, :],
                                 in_=yd[:, 16 * gq:16 * gq + 16, :])
                    SY.dma_start(out=out_ext[:,
                                             32 * gq + 1:32 * gq + 32:2, :],
                                 in_=yd[:, 16 * gq:16 * gq + 16, :])



# --------------------------------------------------------------------------
# host entry point
# --------------------------------------------------------------------------

_CACHE = {}


def kernel(**inputs):
    x = np.ascontiguousarray(inputs['x'], np.float32)      # [8, 64, 64, 64]
    B = x.shape[0]
    consts = host_constants(inputs)

    if 'nc' not in _CACHE:
        _CACHE['nc'] = build_nc()
    nc = _CACHE['nc']

    packed = pack_constants(consts)
    in_maps = []
    for b in range(B):
        m = {'x': x[b].reshape(64, HW)}
        for name, shape, dt in CONST_SPECS:
            m['c_' + name] = packed[name]
        in_maps.append(m)

    res = run_bass_kernel_spmd(nc, in_maps, list(range(B)))
    out = np.stack([res.results[b]['out'] for b in range(B)])
    return out.astype(np.float32)



# revision 2
# speedup vs baseline: 1.0121x; 1.0121x over previous
"""Trainium2 Bass kernel v2 for nn_DeformableUpsampleBlock (fixed instance).

Same algorithm family as the baseline (quad-parity gather tables + on-PE
einsums), restructured around the TimelineSim cost model:
  - x loaded once ([64, HW]); conv3x3 uses K=64 contraction (no shifted copy)
  - pad-strip memsets instead of full-tile memsets
  - bilinear corner coefficients stored pair-duplicated so the big combine
    multiplies hit the DVE 2x fast mode; all combine work on DVE
  - second deform's table split into x/mid halves so the x half is built
    during phase C and only the mid half sits on the post-einsum tail
  - einsum1 / yd evacuation / output DMA pipelined per chunk
"""

import numpy as np

import concourse.bass as bass
import concourse.mybir as mybir
from concourse import bacc
import concourse.tile as tile
from concourse.bass_utils import run_bass_kernel_spmd
from concourse.masks import make_identity

F32 = mybir.dt.float32
F16 = mybir.dt.float16
I16 = mybir.dt.int16
AF = mybir.ActivationFunctionType
ALU = mybir.AluOpType

H = W = 64
HW = H * W              # 4096
NCH = 32                # pixel chunks of 128; pixel p -> [p % 128, p // 128]
PAD3 = 3
PP3 = H + 2 * PAD3      # 70
PAD1 = 2
PP1 = H + 2 * PAD1      # 68
NB = 34                 # quad blocks per side (both tables)
NROW = 4 * NB * NB      # 4624
HT3_COLS = 4992         # 39*128 >= 70*70 (+ quad-build overread)
HT1_COLS = 4864         # 38*128; quad build reads to 4761
EPS = 1e-5


# --------------------------------------------------------------------------
# host-side constants
# --------------------------------------------------------------------------

def _f16(a):
    return np.ascontiguousarray(a).astype(np.float16)


def host_constants(p):
    c = {}
    inv3 = (1.0 / np.sqrt(p['bn3_var'].astype(np.float64) + EPS)).astype(np.float32)
    s3 = (p['bn3_gamma'] * inv3).astype(np.float32)
    t3 = (p['bn3_beta'] - p['bn3_mean'] * s3).astype(np.float32)
    c['s3'] = s3.reshape(64, 1).copy()
    c['t3'] = t3.reshape(64, 1).copy()

    inv1 = (1.0 / np.sqrt(p['bn1_var'].astype(np.float64) + EPS)).astype(np.float32)
    s1 = (p['bn1_gamma'] * inv1).astype(np.float32)
    t1 = (p['bn1_beta'] - p['bn1_mean'] * s1).astype(np.float32)
    c['s1x'] = s1[:64].reshape(64, 1).copy()
    c['t1x'] = t1[:64].reshape(64, 1).copy()
    c['s1m'] = s1[64:].reshape(64, 1).copy()
    c['t1m'] = (t1[64:] + s1[64:] * p['b_d3']).reshape(64, 1).astype(np.float32)

    w3 = p['w_off3'].astype(np.float32)          # [18, 64, 3, 3]
    # 6 conv groups: per kx a K=128 pair (ky=0 in parts 0:64, ky=1 in the
    # row-shifted parts 64:128) plus a K=64 single (ky=2)
    wC = np.zeros((128, 162), np.float32)
    for kx in range(3):
        wC[:64, 36 * kx:36 * kx + 18] = w3[:, :, 0, kx].T
        wC[64:, 36 * kx:36 * kx + 18] = w3[:, :, 1, kx].T
        wC[:64, 36 * kx + 18:36 * kx + 36] = w3[:, :, 2, kx].T
    c['wC'] = _f16(wC)
    c['boff3'] = p['b_off3'].astype(np.float32).reshape(18, 1)
    c['boff1'] = p['b_off1'].astype(np.float32).reshape(2, 1)

    wd3 = p['w_d3'].astype(np.float32).reshape(64, 64, 9)    # [o, c, k]
    wt = np.zeros((128, 320), np.float32)
    for g in range(5):
        for part in range(128):
            kap = 128 * g + part
            if kap < 576:
                wt[part, 64 * g:64 * g + 64] = wd3[:, kap % 64, kap // 64]
    c['wd3T'] = _f16(wt)

    c['woff1T'] = _f16(p['w_off1'].reshape(2, 128).T)
    c['wd1T'] = _f16(p['w_d1'].reshape(32, 128).T)
    c['bd1'] = p['b_d1'].astype(np.float32).reshape(32, 1)

    part = np.arange(128)[:, None]
    chunk = np.arange(NCH)[None, :]
    pix = chunk * 128 + part
    ymap = (pix // W).astype(np.float32)
    xmap = (pix % W).astype(np.float32)
    yb3 = np.zeros((128, NCH, 9), np.float32)
    xb3 = np.zeros((128, NCH, 9), np.float32)
    for k in range(9):
        yb3[:, :, k] = ymap + (k // 3 + PAD3 - 2)
        xb3[:, :, k] = xmap + (k % 3 + PAD3 - 2)
    yb1 = ymap + (PAD1 - 1)
    xb1 = xmap + (PAD1 - 1)

    # quad tables are laid out by-outer: row = 136*by + 34*(2*ay+ax) + bx.
    # With Y0 = yb+sy, X0 = xb+sx, ay = Y0%2, ax = X0%2 the ay terms cancel:
    # row = 68*Y0 + 0.5*X0 + 33.5*ax = C0 + 68*sy + sx*CB (exact in f32)
    def _rowconsts(yb, xb):
        pbx = np.mod(xb, 2.0)
        c0 = 68.0 * yb + 0.5 * xb + 33.5 * pbx
        cb = 34.0 - 67.0 * pbx
        return c0.astype(np.float32), cb.astype(np.float32)

    c03, cB3 = _rowconsts(yb3, xb3)
    c['c03'] = c03.reshape(128, 288)
    c['cB3'] = cB3.reshape(128, 288)
    c01, cB1 = _rowconsts(yb1, xb1)
    c['c01'] = c01
    c['cB1'] = cB1
    # wrap matrices: wrapR[p, 128*r + q] = 1 iff p == 16*r + q%16
    wrapR = np.zeros((128, 1024), np.float32)
    for r in range(8):
        for q in range(128):
            wrapR[16 * r + q % 16, 128 * r + q] = 1.0
    c['wrapR'] = wrapR
    return c


_VEC_SPECS = [   # [P<=128, 1] f32 per-partition vectors -> blob 'cvec'
    ('s3', 64), ('t3', 64), ('s1x', 64), ('t1x', 64), ('s1m', 64),
    ('t1m', 64), ('boff3', 18), ('boff1', 2), ('bd1', 32),
]
_MAP_SPECS = [   # [128, N] f32 coordinate maps -> blob 'cmap'
    ('c03', 288), ('cB3', 288),
    ('c01', 32), ('cB1', 32),
    ('wrapR', 1024),
]
_W_SPECS = [     # [128, N] f16 weights -> blob 'cw'
    ('wC', 162), ('wd3T', 320), ('woff1T', 2), ('wd1T', 32),
]
CONST_SPECS = [
    ('cvec', (128, len(_VEC_SPECS)), F32),
    ('cmap', (128, sum(n for _, n in _MAP_SPECS)), F32),
    ('cw', (128, sum(n for _, n in _W_SPECS)), F16),
]


def pack_constants(c):
    cvec = np.zeros((128, len(_VEC_SPECS)), np.float32)
    for i, (n, p) in enumerate(_VEC_SPECS):
        cvec[:p, i] = c[n].reshape(-1)
    cmap = np.concatenate([c[n].reshape(128, sz) for n, sz in _MAP_SPECS], axis=1)
    cw = np.concatenate([c[n].reshape(128, sz) for n, sz in _W_SPECS],
                        axis=1).astype(np.float16)
    return {'cvec': cvec.astype(np.float32), 'cmap': cmap.astype(np.float32),
            'cw': cw}


# --------------------------------------------------------------------------
# AP helpers
# --------------------------------------------------------------------------

def _rows(ap2d, off, rstride, nr, ncols):
    """[P, nr, ncols] view of a [P, N] AP: rows of length ncols, stride rstride."""
    v = ap2d[:, off:off + nr * rstride].rearrange('p (r q) -> p r q', q=rstride)
    return v[:, :, 0:ncols]


# --------------------------------------------------------------------------
# device program
# --------------------------------------------------------------------------

def build_nc():
    nc = bacc.Bacc()
    x_in = nc.declare_dram_parameter('x', [64, HW], F32, isOutput=False)
    consts = {}
    for name, shape, dt in CONST_SPECS:
        consts[name] = nc.declare_dram_parameter('c_' + name, list(shape), dt,
                                                 isOutput=False)
    out_ext = nc.declare_dram_parameter('out', [32, 2 * H, 2 * W], F32,
                                        isOutput=True)

    hT_dram = nc.dram_tensor('hT_dram', [HT3_COLS, 64], F16)
    quad3 = nc.dram_tensor('quad3', [NROW, 256], F16)
    h1Tx = nc.dram_tensor('h1Tx', [HT1_COLS, 64], F16)
    h1Tm = nc.dram_tensor('h1Tm', [HT1_COLS, 64], F16)
    quad1x = nc.dram_tensor('quad1x', [NROW, 128], F16)
    quad1m = nc.dram_tensor('quad1m', [NROW, 128], F16)
    gate = nc.dram_tensor('gate', [1, 16], F16)

    with tile.TileContext(nc) as tc:
        _body(nc, tc, x_in, consts, out_ext, hT_dram, quad3,
              h1Tx, h1Tm, quad1x, quad1m, gate)
    nc.finalize()
    return nc


def _coords_rows(nc, scratch, dyv, dxv, c0v, cBv, row_out, eng=None):
    """row = C0 + 68*sy + sx*CB; sy/sx persist in scratch for _coords_coefs."""
    sy, sx, ta, tb = scratch
    V = eng or nc.vector
    V.tensor_scalar(out=sy, in0=dyv, scalar1=0.0, scalar2=None, op0=ALU.is_ge)
    V.tensor_scalar(out=sx, in0=dxv, scalar1=0.0, scalar2=None, op0=ALU.is_ge)
    V.scalar_tensor_tensor(out=ta, in0=sy, scalar=68.0, in1=c0v,
                           op0=ALU.mult, op1=ALU.add)
    V.tensor_tensor(out=tb, in0=sx, in1=cBv, op=ALU.mult)
    V.tensor_tensor(out=row_out, in0=ta, in1=tb, op=ALU.add)


def _coords_coefs(nc, scratch, dyv, dxv, coef_out, eng=None,
                  split_corners=False):
    """Corner coefficients from dy/dx and the sy/sx left in scratch."""
    sy, sx, fy, fx = scratch
    V = eng or nc.vector
    # fy = dy + 1 - sy; gy = 1 - fy = sy - dy (reuse sy/sx slots for gy/gx)
    V.scalar_tensor_tensor(out=fy, in0=dyv, scalar=1.0, in1=sy,
                           op0=ALU.add, op1=ALU.subtract)
    V.scalar_tensor_tensor(out=fx, in0=dxv, scalar=1.0, in1=sx,
                           op0=ALU.add, op1=ALU.subtract)
    V.tensor_tensor(out=sy, in0=sy, in1=dyv, op=ALU.subtract)
    V.tensor_tensor(out=sx, in0=sx, in1=dxv, op=ALU.subtract)
    nd = coef_out.ndim - (3 if split_corners else 2)
    for i, (a, b) in enumerate([(sy, sx), (sy, fx), (fy, sx), (fy, fx)]):
        for j in range(2):
            idx = (i // 2, i % 2, j) if split_corners else (i, j)
            V.tensor_tensor(out=coef_out[(slice(None),) * nd + idx],
                            in0=a, in1=b, op=ALU.mult)


def _wrap_idx(nc, rowi16_v, wrapped, eng=None):
    """rowi16_v: [128, nk, nch] i16 (contiguous) -> wrapped [128, nk, 256]:
    wrapped[q, k, chunk*8 + r] = row[16r+q, k, chunk], replicated to the 8
    16-partition groups."""
    eng = eng or nc.sync
    for r in range(8):
        eng.dma_start(out=wrapped[0:16, :, r::8],
                      in_=rowi16_v[16 * r:16 * r + 16, :, :])
    for gsz in (16, 32, 64):
        eng.dma_start(out=wrapped[gsz:2 * gsz, :, :],
                      in_=wrapped[0:gsz, :, :])


def _quad_build(nc, src_dram, dst_dram, pp, chans, b0=0, b1=NB,
                parities=None, eng=None):
    """DRAM->DRAM DMAs (3-dim APs) building the quad-parity block table
    (by-outer layout: row = 4*NB*by + NB*(2*ay+ax) + bx) for block rows
    by in [b0, b1); optionally only some (ay, ax) parities."""
    q = 2 * pp
    nb = b1 - b0
    for ay in range(2):
        for ax in range(2):
            if parities is not None and (ay, ax) not in parities:
                continue
            s = ay * 2 + ax
            for yy in range(2):
                r0 = (ay + yy) * pp + ax + b0 * q
                sv = src_dram[:][r0:r0 + nb * q, :] \
                    .rearrange('(by q) c -> by q c', q=q)[:, 0:2 * NB, :] \
                    .rearrange('by (bx xx) c -> by bx (xx c)', xx=2)
                dv = dst_dram[4 * NB * b0:4 * NB * b1,
                              2 * chans * yy:2 * chans * (yy + 1)] \
                    .rearrange('(by sx) e -> by sx e', sx=4 * NB)[
                        :, NB * s:NB * s + NB, :]
                (eng or nc.sync).dma_start(out=dv, in_=sv)


def _xpar_build(nc, src_dram, dst_dram, p0, p1, eng=None):
    """x-parity table for the 1x1 deform: dst row 68*py + 34*ax + px2 holds
    the 128 f16 of padded positions (68*py + 2*px2 + ax, +1). One DMA per ax
    with 8.5KB-contiguous dst runs; builds py in [p0, p1)."""
    sflat = src_dram[:].rearrange('r c -> (r c)')
    for ax in range(2):
        base = 64 * (PP1 * p0 + ax)
        sv = sflat[base:base + (p1 - p0) * 64 * PP1] \
            .rearrange('(py q) -> py q', q=64 * PP1) \
            .rearrange('py (px2 e) -> py px2 e', e=128)[:, 0:NB, :]
        dv = dst_dram[PP1 * p0:PP1 * p1, :] \
            .rearrange('(py sx) e -> py sx e', sx=PP1)[:, NB * ax:
                                                       NB * ax + NB, :]
        (eng or nc.sync).dma_start(out=dv, in_=sv)


def _pad_memset(nc, t, npart, pp, pad, w, ncols):
    """Zero only the padding cells of a padded image tile t [npart, ncols]."""
    head = pad * pp + pad
    nc.gpsimd.memset(t[0:npart, 0:head], 0.0)
    gapw = pp - w
    r0, r1 = pad, pad + w  # gap r covers trail of row r / lead of row r+1
    ngap = r1 - r0 - 1
    gv = _rows(t[0:npart, :], r0 * pp + pad + w, pp, ngap, gapw)
    nc.gpsimd.memset(gv, 0.0)
    tail0 = (r1 - 1) * pp + pad + w
    nc.gpsimd.memset(t[0:npart, tail0:ncols], 0.0)



def _body(nc, tc, x_in, consts, out_ext, hT_dram, quad3,
          h1Tx, h1Tm, quad1x, quad1m, gate):
    V, S, G, PE, SY = nc.vector, nc.scalar, nc.gpsimd, nc.tensor, nc.sync

    with (
        tc.tile_pool(name='persist', bufs=1) as pp,
        tc.tile_pool(name='psum', bufs=2, space='PSUM') as psp,
        tc.tile_pool(name='psumv', bufs=2, space='PSUM') as psv,
    ):
        # ---------------- constants / persistent tiles -------------------
        blobs = {}
        for name, shape, dt in CONST_SPECS:
            t = pp.tile(list(shape), dt, tag='c_' + name, name='c_' + name)
            blobs[name] = t
        C = {}
        for i, (n, p_) in enumerate(_VEC_SPECS):
            C[n] = blobs['cvec'][0:p_, i:i + 1]
        col = 0
        for n, sz in _MAP_SPECS:
            C[n] = blobs['cmap'][:, col:col + sz]
            col += sz
        col = 0
        for n, sz in _W_SPECS:
            C[n] = blobs['cw'][:, col:col + sz]
            col += sz
        idt32 = pp.tile([32, 32], F32, tag='idt32', name='idt32')
        idt128h = pp.tile([128, 128], F16, tag='idt128h', name='idt128h')
        h1pad = pp.tile([128, HT1_COLS], F16, tag='h1pad', name='h1pad')
        coefD = pp.tile([128, 9, NCH, 4, 2], F16, tag='coefD', name='coefD')
        coef1D = pp.tile([128, 2, NCH, 2, 2], F16, tag='coef1D',
                 name='coef1D')
        wrapped3 = pp.tile([128, 9, 256], I16, tag='wrapped3', name='wrapped3')
        wrapped1 = pp.tile([128, 2, 256], I16, tag='wrapped1', name='wrapped1')
        h1Tst = pp.tile([128, 37, 128], F16, tag='h1Tst', name='h1Tst')
        off3T = pp.tile([128, NCH, 18], F32, tag='off3T', name='off3T')
        off1T = pp.tile([128, NCH, 2], F32, tag='off1T', name='off1T')

        # ---------------- phase A: bn3, transposes, conv, coords ---------
        with tc.tile_pool(name='ph1', bufs=1) as ph1, \
             tc.tile_pool(name='oev', bufs=2) as oev:
            x2 = ph1.tile([64, HW], F32, tag='x2', name='x2')
            hpad2 = ph1.tile([128, HT3_COLS], F16, tag='hpad2',
                             name='hpad2')
            # cvec first (gates bn3), then x, then cw/cmap (needed later)
            SY.dma_start(out=blobs['cvec'][:, :], in_=consts['cvec'][:])
            SY.dma_start(out=x2[:, 0:2048], in_=x_in[:][:, 0:2048])
            SY.dma_start(out=x2[:, 2048:HW], in_=x_in[:][:, 2048:HW])
            SY.dma_start(out=blobs['cw'][:, :], in_=consts['cw'][:])
            SY.dma_start(out=blobs['cmap'][:, :], in_=consts['cmap'][:])
            make_identity(nc, idt32[:, :])
            make_identity(nc, idt128h[:, :])
            _pad_memset(nc, hpad2, 128, PP3, PAD3, W, HT3_COLS)
            _pad_memset(nc, h1pad, 128, PP1, PAD1, W, HT1_COLS)
            xv = x2[:, :].rearrange('p (r c) -> p r c', r=H)

            # bn3 in two row-chunks (pipelines with the x DMA halves);
            # partitions 64:128 hold the same rows shifted up one padded
            # row so the conv can pair taps (ky, ky+1) with K=128
            for half in range(2):
                S.activation(_rows(hpad2[0:64, :],
                                   (PAD3 + 32 * half) * PP3 + PAD3,
                                   PP3, 32, W),
                             xv[0:64, 32 * half:32 * half + 32, :], AF.Relu,
                             bias=C['t3'], scale=C['s3'])
            for half in range(2):
                c0 = (PAD3 - 1 + 32 * half) * PP3
                SY.dma_start(out=hpad2[64:128, c0:c0 + 32 * PP3],
                             in_=hpad2[0:64, c0 + PP3:c0 + 33 * PP3])
            # hT transposes: [64, 128] -> [128, 64] per 128-col chunk;
            # store + quad3 build in two stages so the table is ready early
            hTst = ph1.tile([128, 39, 64], F16, tag='hTst', name='hTst')
            for t0 in range(0, 39, 4):
                nt = min(4, 39 - t0)
                pv = psv.tile([128, 512], F16, tag='pv', name='pv')
                for j in range(nt):
                    PE.transpose(pv[:, 64 * j:64 * j + 64],
                                 hpad2[0:64,
                                       128 * (t0 + j):128 * (t0 + j) + 128],
                                 idt128h[0:64, 0:64])
                if (t0 // 4) % 2 == 1:
                    V.tensor_copy(out=hTst[:, t0:t0 + nt, :],
                                  in_=pv[:, 0:64 * nt])
                else:
                    S.activation(hTst[:, t0:t0 + nt, :],
                                 pv[:, 0:64 * nt]
                                 .rearrange('p (a b) -> p a b', b=64),
                                 AF.Identity)
                if t0 == 16:
                    SY.dma_start(out=hT_dram[0:2560, :]
                                 .rearrange('(a p) c -> p a c', p=128),
                                 in_=hTst[:, 0:20, :])
                    _quad_build(nc, hT_dram, quad3, PP3, 64, 0, 17)
            SY.dma_start(out=hT_dram[2560:4992, :]
                         .rearrange('(a p) c -> p a c', p=128),
                         in_=hTst[:, 20:39, :])
            _quad_build(nc, hT_dram, quad3, PP3, 64, 17, NB)

            # conv3x3: 2 blocks x 4 psum chunks x 6 groups
            # (K=128 tap-pairs (ky=0,1) + K=64 singles (ky=2) per kx)
            for blk in range(2):
                pcs = [psp.tile([128, 512], F32, tag=f'pmm{i}', name=f'pc{i}',
                                bufs=1) for i in range(4)]
                for g in range(6):
                    kx, sub = g // 2, g % 2
                    ky = 0 if sub == 0 else 2
                    npt = 128 if sub == 0 else 64
                    for i in range(4):
                        ch = 4 * blk + i
                        rhs = _rows(hpad2[0:npt, :],
                                    (2 + ky) * PP3 + 2 + kx + 8 * PP3 * ch,
                                    PP3, 8, W)
                        PE.matmul(pcs[i][0:18, :],
                                  C['wC'][0:npt, 18 * g:18 * g + 18],
                                  rhs, start=(g == 0), stop=(g == 5))
                for i in range(4):
                    ch = 4 * blk + i
                    o3 = oev.tile([18, 512], F32, tag='o3', name='o3')
                    S.activation(o3[:, :], pcs[i][0:18, :], AF.Identity,
                                 bias=C['boff3'])
                    pt = psp.tile([128, 128], F32, tag='ptr', name='pt',
                                  bufs=1)
                    for t in range(4):
                        PE.transpose(pt[:, 18 * t:18 * t + 18],
                                     o3[:, 128 * t:128 * t + 128],
                                     idt32[0:18, 0:18])
                    V.tensor_copy(out=off3T[:, 4 * ch:4 * ch + 4, :]
                                  .rearrange('p a b -> p (a b)'),
                                  in_=pt[:, 0:72])

            # coordinates / gather rows -> wrap matmuls / coefficients
            sc = [ph1.tile([128, 288], F32, tag=f'sc{i}', name=f'sc{i}')
                  for i in range(4)]
            rowf3 = ph1.tile([128, 2, 9, 16], F32, tag='rowf3', name='rowf3')
            scv = [s[:, :].rearrange('p (a b) -> p a b', b=9) for s in sc]
            wr3v = wrapped3[:, :, :].rearrange('p k (h c r) -> p h k c r',
                                               h=2, r=8)
            cmv = [C[n].rearrange('p (a b) -> p a b', b=9)
                   for n in ('c03', 'cB3')]
            for hf in range(2):
                cs = slice(16 * hf, 16 * hf + 16)
                _coords_rows(nc,
                             [sv[:, cs, :] for sv in scv],
                             off3T[:, cs, 0:18:2], off3T[:, cs, 1:18:2],
                             cmv[0][:, cs, :], cmv[1][:, cs, :],
                             rowf3[:, hf, :, :].transpose([0, 2, 1]))
                for r in range(8):
                    ptag = ['pmm0', 'pmm1', 'pmm2', 'pmm3'][r % 4]
                    pw = psp.tile([128, 512], F32, tag=ptag, name='pw',
                                  bufs=1)
                    PE.matmul(pw[:, 0:144],
                              C['wrapR'][:, 128 * r:128 * r + 128],
                              rowf3[:, hf, :, :], start=True, stop=True)
                    if r % 2 == 0:
                        S.activation(wr3v[:, hf, :, :, r], pw[:, 0:144],
                                     AF.Identity)
                    else:
                        V.tensor_copy(out=wr3v[:, hf, :, :, r],
                                      in_=pw[:, 0:144])
                _coords_coefs(nc,
                              [sv[:, cs, :] for sv in scv],
                              off3T[:, cs, 0:18:2], off3T[:, cs, 1:18:2],
                              coefD[:, :, cs, :, :].transpose([0, 2, 1, 3, 4]))

            # h1 x-part: relu(bn1(x)) into h1pad interior (ACT, off path)
            S.activation(_rows(h1pad[0:64, :], PAD1 * PP1 + PAD1, PP1, H, W),
                         xv[0:64], AF.Relu, bias=C['t1x'], scale=C['s1x'])
            # h1Tst x-half transposes + x-side table build (DMA idle here)
            for t0 in range(0, 37, 4):
                nt = min(4, 37 - t0)
                pv = psv.tile([128, 512], F16, tag='pv', name='pv')
                for j in range(nt):
                    PE.transpose(pv[:, 64 * j:64 * j + 64],
                                 h1pad[0:64, 128 * (t0 + j):
                                       128 * (t0 + j) + 128],
                                 idt128h[0:64, 0:64])
                S.activation(h1Tst[:, t0:t0 + nt, 0:64],
                             pv[:, 0:64 * nt], AF.Identity)
            SY.dma_start(out=h1Tx[0:4736, :]
                         .rearrange('(a p) c -> p a c', p=128),
                         in_=h1Tst[:, :, 0:64])
            _xpar_build(nc, h1Tx, quad1x, 0, PP1)

        # ---------------- phase C: gathers + in-place combine ------------
        vhgs = {}
        pend_fadd = []
        with tc.tile_pool(name='gpool', bufs=4) as gp, \
             tc.tile_pool(name='vhp', bufs=3) as vhp:
            for pos, k in enumerate([0, 1, 8, 2, 3, 4, 5, 6, 7]):
                gp_i, sl = k // 2, k % 2
                if gp_i not in vhgs:
                    nt = 2 if gp_i < 4 else 1
                    tag = 'vh' if gp_i < 4 else 'vh4'
                    vhgs[gp_i] = vhp.tile([128, NCH, nt, 64], F16, tag=tag,
                                          name=f'vh{gp_i}')
                vhg = vhgs[gp_i]
                g = gp.tile([128, 8192], F16, tag='g', name='g')
                G.dma_gather(g[:, :].rearrange('p (a c) -> p a c', c=256),
                             quad3[:], wrapped3[:, k, :], 4096, 4096,
                             256, queue_num=0, single_packet=False)

                gq = g[:, :].rearrange('p (a b c d) -> p a b c d',
                                       a=NCH, b=4, c=32)
                cf = coefD[:, k][:, :, :, None, :].broadcast_to(
                    [128, NCH, 4, 32, 2])
                gw = g[:, :].rearrange('p (a b c) -> p a b c', a=NCH, b=4)
                nspl = 2 if pos == 8 else 1
                nh = NCH // nspl
                for sp in range(nspl):
                    chs = slice(nh * sp, nh * sp + nh)
                    V.tensor_tensor(out=gq[:, chs], in0=gq[:, chs],
                                    in1=cf[:, chs], op=ALU.mult)
                    with nc.allow_low_precision('fp16 middle precision'):
                        V.tensor_tensor(out=gw[:, chs, 0:2, :],
                                        in0=gw[:, chs, 0:2, :],
                                        in1=gw[:, chs, 2:4, :], op=ALU.add)
                        V.tensor_tensor(out=vhg[:, chs, sl, :],
                                        in0=gw[:, chs, 0, :],
                                        in1=gw[:, chs, 1, :], op=ALU.add)

            # ------------ phase C2/D: transposes + einsum3 ---------------
            sc1 = [pp.tile([128, 32], F32, tag=f't1s{i}', name=f't1s{i}')
                   for i in range(4)]
            rowf = pp.tile([128, 32], F32, tag='rowf', name='rowf')
            rowt = pp.tile([128, 16, 8], F32, tag='rowt', name='rowt')
            vhat1 = pp.tile([128, NCH, 2, 64], F16, tag='vhat1',
                            name='vhat1')
            wr1v = wrapped1[:, :, :].rearrange('p a (c r) -> p a c r', r=8)

            def _coords1_half(hf):
                hs = slice(16 * hf, 16 * hf + 16)
                _coords_rows(nc, [s[:, hs] for s in sc1],
                             off1T[:, hs, 0], off1T[:, hs, 1],
                             C['c01'][:, hs], C['cB1'][:, hs], rowf[:, hs])
                pw1 = psp.tile([128, 512], F32, tag='ptr', name='pw1',
                               bufs=1)
                for r in range(8):
                    PE.matmul(pw1[:, 16 * r:16 * r + 16],
                              C['wrapR'][:, 128 * r:128 * r + 128],
                              rowf[:, hs], start=True, stop=True)
                pwv = pw1[:, 0:128].rearrange('p (r c) -> p c r', c=16)
                V.tensor_copy(out=wr1v[:, 0, hs, :], in_=pwv)
                # second y-corner row sits one padded row (+PP1) below
                V.tensor_scalar(out=rowt[:, 0:16, :], in0=pwv,
                                scalar1=float(PP1), scalar2=None,
                                op0=ALU.add)
                V.tensor_copy(out=wr1v[:, 1, hs, :], in_=rowt[:, 0:16, :])
                _coords_coefs(nc, [s[:, hs] for s in sc1],
                              off1T[:, hs, 0], off1T[:, hs, 1],
                              coef1D[:, :, hs, :, :]
                              .transpose([0, 2, 1, 3, 4]),
                              split_corners=True)

            def _gather1(quadap, c0, nch):
                """Gather chunks [c0, c0+nch) of the 1x1 deform from the
                x-parity table: one gather per y-corner a, each row holding
                the two x-corners for 64 channels."""
                g1 = gp.tile([128, 8192], F16, tag='g', name='g1')
                for a in range(2):
                    G.dma_gather(g1[:, 4096 * a:4096 * a + 128 * nch]
                                 .rearrange('p (c e) -> p c e', e=128),
                                 quadap,
                                 wrapped1[:, a, 8 * c0:8 * (c0 + nch)],
                                 128 * nch, 128 * nch, 128, queue_num=0,
                                 single_packet=False)
                return g1[:, :].rearrange('p (a q) -> p a q', a=2)[
                    :, :, 0:128 * nch] \
                    .rearrange('p a (c b e) -> p a c b e', b=2, e=64)

            def _combine1(g1v, hx, c0, nch):
                for a in range(2):
                    gq = g1v[:, a].rearrange('p c b (f d) -> p c b f d', d=2)
                    cf = coef1D[:, a, c0:c0 + nch, :, :][
                        :, :, :, None, :].broadcast_to([128, nch, 2, 32, 2])
                    V.tensor_tensor(out=gq, in0=gq, in1=cf, op=ALU.mult)
                with nc.allow_low_precision('fp16 by design'):
                    V.tensor_tensor(out=g1v[:, 0], in0=g1v[:, 0],
                                    in1=g1v[:, 1], op=ALU.add)
                    V.tensor_tensor(
                        out=vhat1[:, c0:c0 + nch, hx, :],
                        in0=g1v[:, 0, :, 0, :], in1=g1v[:, 0, :, 1, :],
                        op=ALU.add)

            with tc.tile_pool(name='vpool', bufs=1) as vp:
                v = vp.tile([128, 5, HW], F16, tag='v', name='v')
                pms = {}

                def _mm3(ch, gp_i):
                    if ch not in pms:
                        pms[ch] = psp.tile([128, 512], F32,
                                           tag=f'pmm{ch % 4}',
                                           name=f'pm{ch}', bufs=1)
                    if gp_i < 4:
                        PE.matmul(pms[ch][64:128, :],
                                  C['wd3T'][:, 64 * gp_i:64 * gp_i + 64],
                                  v[:, gp_i, 512 * ch:512 * ch + 512],
                                  start=(gp_i == 0), stop=False)
                    else:
                        PE.matmul(pms[ch][64:128, :],
                                  C['wd3T'][0:64, 256:320],
                                  v[0:64, 4, 512 * ch:512 * ch + 512],
                                  start=False, stop=True)

                for gp_i in (0, 4, 1, 2, 3):
                    for ch4 in range(8):
                        pv = psv.tile([128, 512], F16, tag='pv', name='pv')
                        for sub in range(4):
                            ch = 4 * ch4 + sub
                            if gp_i < 4:
                                PE.transpose(
                                    pv[:, 128 * sub:128 * sub + 128],
                                    vhgs[gp_i][:, ch, :, :]
                                    .rearrange('p a b -> p (a b)'),
                                    idt128h[:, :])
                            else:
                                PE.transpose(
                                    pv[0:64, 128 * sub:128 * sub + 128],
                                    vhgs[4][:, ch, 0, :], idt128h[:, :])
                        np_ = 128 if gp_i < 4 else 64
                        if gp_i == 3 and ch4 % 2 == 1:
                            # last group lands post-combine; DVE is free then
                            V.tensor_copy(out=v[0:np_, gp_i,
                                                512 * ch4:512 * ch4 + 512],
                                          in_=pv[0:np_, :])
                        else:
                            S.activation(v[0:np_, gp_i,
                                           512 * ch4:512 * ch4 + 512],
                                         pv[0:np_, :], AF.Identity)
                    for ch in range(4):
                        _mm3(ch, gp_i)

                # ---- phase E: evacs + off1 + tables + 1x1 gathers -------
                estate = {'tch': 0, 'pt1': None}
                with tc.tile_pool(name='oev1', bufs=2) as oev1:

                    def _evac_chunk(ch):
                        hv = _rows(h1pad[64:128, :],
                                   (8 * ch + PAD1) * PP1 + PAD1, PP1, 8, W)
                        pmv = pms[ch][64:128, :] \
                            .rearrange('p (r c) -> p r c', r=8)
                        S.activation(hv, pmv, AF.Relu, bias=C['t1m'],
                                     scale=C['s1m'])
                        pc1 = psp.tile([128, 512], F32, tag='pc1',
                                       name='pc1', bufs=1)
                        PE.matmul(pc1[0:2, :], C['woff1T'],
                                  _rows(h1pad[0:128, :],
                                        (8 * ch + PAD1) * PP1 + PAD1,
                                        PP1, 8, W),
                                  start=True, stop=True)
                        o1 = oev1.tile([2, 512], F32, tag='o1', name='o1')
                        if ch % 2 == 0:
                            S.activation(o1[:, :], pc1[0:2, :], AF.Identity,
                                         bias=C['boff1'])
                        else:
                            V.tensor_scalar(out=o1[:, :], in0=pc1[0:2, :],
                                            scalar1=C['boff1'], scalar2=None,
                                            op0=ALU.add)
                        if ch % 2 == 0:
                            estate['pt1'] = psp.tile([128, 128], F32,
                                                     tag='ptr', name='pt1',
                                                     bufs=1)
                        pt1 = estate['pt1']
                        for t in range(4):
                            PE.transpose(pt1[:, 8 * (ch % 2) + 2 * t:
                                             8 * (ch % 2) + 2 * t + 2],
                                         o1[:, 128 * t:128 * t + 128],
                                         idt32[0:2, 0:2])
                        if ch % 2 == 1:
                            V.tensor_copy(out=off1T[:, 8 * (ch // 2):
                                                    8 * (ch // 2) + 8, :]
                                          .rearrange('p a b -> p (a b)'),
                                          in_=pt1[:, 0:16])
                        # h1Tst mid-half transposes ready with this chunk
                        r_hi_ready = 2 + 8 * ch + 8
                        ready = []
                        while estate['tch'] < 37:
                            tch = estate['tch']
                            r_hi = (128 * tch + 127) // PP1
                            if r_hi >= r_hi_ready and ch < 7:
                                break
                            ready.append(tch)
                            estate['tch'] += 1
                        for i0 in range(0, len(ready), 4):
                            grp = ready[i0:i0 + 4]
                            pv = psv.tile([128, 512], F16, tag='pv',
                                          name='pv')
                            for j, tch in enumerate(grp):
                                PE.transpose(pv[:, 64 * j:64 * j + 64],
                                             h1pad[64:128,
                                                   128 * tch:128 * tch + 128],
                                             idt128h[64:128, 64:128])
                            t0g = grp[0]
                            estate['evp'] = estate.get('evp', 0) + 1
                            if estate['evp'] % 2 == 1:
                                S.activation(h1Tst[:, t0g:t0g + len(grp),
                                                   64:128],
                                             pv[:, 0:64 * len(grp)]
                                             .rearrange('p (a b) -> p a b',
                                                        b=64),
                                             AF.Identity)
                            else:
                                V.tensor_copy(out=h1Tst[:, t0g:t0g + len(grp),
                                                        64:128],
                                              in_=pv[:, 0:64 * len(grp)])
                        # staged h1Tm stores + x-parity table builds: stage s
                        # covers padded rows [QB[s], QB[s+1]) and needs h1Tst
                        # cols [QC[s], QC[s+1])
                        QB = [0, 35, PP1]
                        QC = [0, 19, 37]
                        st = {4: 0, 7: 1}.get(ch)
                        if st is not None:
                            SY.dma_start(
                                out=h1Tm[128 * QC[st]:128 * QC[st + 1], :]
                                .rearrange('(a p) c -> p a c', p=128),
                                in_=h1Tst[:, QC[st]:QC[st + 1], 64:128])
                            _xpar_build(nc, h1Tm, quad1m, QB[st], QB[st + 1])

                    for ch in range(4):
                        _evac_chunk(ch)
                    # 1x1 half 0: coords + x gather issued ASAP
                    _coords1_half(0)
                    g1xh0 = _gather1(quad1x[:], 0, 16)
                    for gp_i in (0, 4, 1, 2, 3):
                        for ch in range(4, 8):
                            _mm3(ch, gp_i)
                    _evac_chunk(4)
                    g1mh0 = _gather1(quad1m[:][0:PP1 * 35, :], 0, 16)
                    _evac_chunk(5)
                    _combine1(g1xh0, 0, 0, 16)
                    _evac_chunk(6)
                    _evac_chunk(7)
                    _combine1(g1mh0, 1, 0, 16)
                    _coords1_half(1)
                    g1xh1 = _gather1(quad1x[:], 16, 16)
                    g1mh1 = _gather1(quad1m[:], 16, 16)
                    _combine1(g1xh1, 0, 16, 16)
                    _combine1(g1mh1, 1, 16, 16)

            # ---------------- v1 transposes + einsum1 + upsample ---------
            with tc.tile_pool(name='tailp', bufs=1) as tp:
                yd = tp.tile([32, H, 2 * W], F32, tag='yd', name='yd')
                v1s = tp.tile([128, 8, 512], F16, tag='v1s', name='v1s')
                for ch4 in range(8):
                    pv = psv.tile([128, 512], F16, tag='pv', name='pv')
                    for sub in range(4):
                        PE.transpose(pv[:, 128 * sub:128 * sub + 128],
                                     vhat1[:, 4 * ch4 + sub, :, :]
                                     .rearrange('p a b -> p (a b)'),
                                     idt128h[:, :])
                    if ch4 % 2 == 0:
                        S.activation(v1s[:, ch4, :], pv[:, :], AF.Identity)
                    else:
                        V.tensor_copy(out=v1s[:, ch4, :], in_=pv[:, :])
                    pmy = psp.tile([128, 512], F32, tag=f'pmm{ch4 % 4}',
                                   name='pmy', bufs=1)
                    PE.matmul(pmy[0:32, :], C['wd1T'], v1s[:, ch4, :],
                              start=True, stop=True)
                    pmv = pmy[0:32, :].rearrange('p (r c) -> p r c', r=8)
                    S.activation(yd[:, 8 * ch4:8 * ch4 + 8, 0::2], pmv,
                                 AF.Identity, bias=C['bd1'])
                    V.tensor_scalar(out=yd[:, 8 * ch4:8 * ch4 + 8, 1::2],
                                    in0=pmv, scalar1=C['bd1'], scalar2=None,
                                    op0=ALU.add)
                    if ch4 % 2 == 1:
                        gq = ch4 // 2
                        SY.dma_start(out=out_ext[:, 32 * gq:32 * gq + 32:2,
                                                 :],
                                     in_=yd[:, 16 * gq:16 * gq + 16, :])
                        SY.dma_start(out=out_ext[:,
                                                 32 * gq + 1:32 * gq + 32:2,
                                                 :],
                                     in_=yd[:, 16 * gq:16 * gq + 16, :])



# --------------------------------------------------------------------------
# host entry point
# --------------------------------------------------------------------------

_CACHE = {}


def kernel(**inputs):
    x = np.ascontiguousarray(inputs['x'], np.float32)      # [8, 64, 64, 64]
    B = x.shape[0]
    consts = host_constants(inputs)

    if 'nc' not in _CACHE:
        _CACHE['nc'] = build_nc()
    nc = _CACHE['nc']

    packed = pack_constants(consts)
    in_maps = []
    for b in range(B):
        m = {'x': x[b].reshape(64, HW)}
        for name, shape, dt in CONST_SPECS:
            m['c_' + name] = packed[name]
        in_maps.append(m)

    res = run_bass_kernel_spmd(nc, in_maps, list(range(B)))
    out = np.stack([res.results[b]['out'] for b in range(B)])
    return out.astype(np.float32)



# revision 3
# speedup vs baseline: 1.0317x; 1.0194x over previous
"""Trainium2 Bass kernel v2 for nn_DeformableUpsampleBlock (fixed instance).

Same algorithm family as the baseline (quad-parity gather tables + on-PE
einsums), restructured around the TimelineSim cost model:
  - x loaded once ([64, HW]); conv3x3 uses K=64 contraction (no shifted copy)
  - pad-strip memsets instead of full-tile memsets
  - bilinear corner coefficients stored pair-duplicated so the big combine
    multiplies hit the DVE 2x fast mode; all combine work on DVE
  - second deform's table split into x/mid halves so the x half is built
    during phase C and only the mid half sits on the post-einsum tail
  - einsum1 / yd evacuation / output DMA pipelined per chunk
"""

import numpy as np

import concourse.bass as bass
import concourse.mybir as mybir
from concourse import bacc
import concourse.tile as tile
from concourse.bass_utils import run_bass_kernel_spmd
from concourse.masks import make_identity

F32 = mybir.dt.float32
F16 = mybir.dt.float16
I16 = mybir.dt.int16
AF = mybir.ActivationFunctionType
ALU = mybir.AluOpType

H = W = 64
HW = H * W              # 4096
NCH = 32                # pixel chunks of 128; pixel p -> [p % 128, p // 128]
PAD3 = 3
PP3 = H + 2 * PAD3      # 70
PAD1 = 2
PP1 = H + 2 * PAD1      # 68
NB = 34                 # quad blocks per side (both tables)
NROW = 4 * NB * NB      # 4624
HT3_COLS = 4992         # 39*128 >= 70*70 (+ quad-build overread)
HT1_COLS = 4864         # 38*128; quad build reads to 4761
EPS = 1e-5


# --------------------------------------------------------------------------
# host-side constants
# --------------------------------------------------------------------------

def _f16(a):
    return np.ascontiguousarray(a).astype(np.float16)


def host_constants(p):
    c = {}
    inv3 = (1.0 / np.sqrt(p['bn3_var'].astype(np.float64) + EPS)).astype(np.float32)
    s3 = (p['bn3_gamma'] * inv3).astype(np.float32)
    t3 = (p['bn3_beta'] - p['bn3_mean'] * s3).astype(np.float32)
    c['s3'] = s3.reshape(64, 1).copy()
    c['t3'] = t3.reshape(64, 1).copy()

    inv1 = (1.0 / np.sqrt(p['bn1_var'].astype(np.float64) + EPS)).astype(np.float32)
    s1 = (p['bn1_gamma'] * inv1).astype(np.float32)
    t1 = (p['bn1_beta'] - p['bn1_mean'] * s1).astype(np.float32)
    c['s1x'] = s1[:64].reshape(64, 1).copy()
    c['t1x'] = t1[:64].reshape(64, 1).copy()
    c['s1m'] = s1[64:].reshape(64, 1).copy()
    c['t1m'] = (t1[64:] + s1[64:] * p['b_d3']).reshape(64, 1).astype(np.float32)

    w3 = p['w_off3'].astype(np.float32)          # [18, 64, 3, 3]
    # 6 conv groups: per kx a K=128 pair (ky=0 in parts 0:64, ky=1 in the
    # row-shifted parts 64:128) plus a K=64 single (ky=2)
    wC = np.zeros((128, 162), np.float32)
    for kx in range(3):
        wC[:64, 36 * kx:36 * kx + 18] = w3[:, :, 0, kx].T
        wC[64:, 36 * kx:36 * kx + 18] = w3[:, :, 1, kx].T
        wC[:64, 36 * kx + 18:36 * kx + 36] = w3[:, :, 2, kx].T
    c['wC'] = _f16(wC)
    c['boff3'] = p['b_off3'].astype(np.float32).reshape(18, 1)
    c['boff1'] = p['b_off1'].astype(np.float32).reshape(2, 1)

    wd3 = p['w_d3'].astype(np.float32).reshape(64, 64, 9)    # [o, c, k]
    wt = np.zeros((128, 320), np.float32)
    for g in range(5):
        for part in range(128):
            kap = 128 * g + part
            if kap < 576:
                wt[part, 64 * g:64 * g + 64] = wd3[:, kap % 64, kap // 64]
    c['wd3T'] = _f16(wt)

    c['woff1T'] = _f16(p['w_off1'].reshape(2, 128).T)
    c['wd1T'] = _f16(p['w_d1'].reshape(32, 128).T)
    c['bd1'] = p['b_d1'].astype(np.float32).reshape(32, 1)

    part = np.arange(128)[:, None]
    chunk = np.arange(NCH)[None, :]
    pix = chunk * 128 + part
    ymap = (pix // W).astype(np.float32)
    xmap = (pix % W).astype(np.float32)
    yb3 = np.zeros((128, NCH, 9), np.float32)
    xb3 = np.zeros((128, NCH, 9), np.float32)
    for k in range(9):
        yb3[:, :, k] = ymap + (k // 3 + PAD3 - 2)
        xb3[:, :, k] = xmap + (k % 3 + PAD3 - 2)
    yb1 = ymap + (PAD1 - 1)
    xb1 = xmap + (PAD1 - 1)

    # quad tables are laid out by-outer: row = 136*by + 34*(2*ay+ax) + bx.
    # With Y0 = yb+sy, X0 = xb+sx, ay = Y0%2, ax = X0%2 the ay terms cancel:
    # row = 68*Y0 + 0.5*X0 + 33.5*ax = C0 + 68*sy + sx*CB (exact in f32)
    def _rowconsts(yb, xb):
        pbx = np.mod(xb, 2.0)
        c0 = 68.0 * yb + 0.5 * xb + 33.5 * pbx
        cb = 34.0 - 67.0 * pbx
        return c0.astype(np.float32), cb.astype(np.float32)

    c03, cB3 = _rowconsts(yb3, xb3)
    c['c03'] = c03.reshape(128, 288)
    c['cB3'] = cB3.reshape(128, 288)
    c01, cB1 = _rowconsts(yb1, xb1)
    c['c01'] = c01
    c['cB1'] = cB1
    # wrap matrices: wrapR[p, 128*r + q] = 1 iff p == 16*r + q%16
    wrapR = np.zeros((128, 1024), np.float32)
    for r in range(8):
        for q in range(128):
            wrapR[16 * r + q % 16, 128 * r + q] = 1.0
    c['wrapR'] = wrapR
    return c


_VEC_SPECS = [   # [P<=128, 1] f32 per-partition vectors -> blob 'cvec'
    ('s3', 64), ('t3', 64), ('s1x', 64), ('t1x', 64), ('s1m', 64),
    ('t1m', 64), ('boff3', 18), ('boff1', 2), ('bd1', 32),
]
_MAP_SPECS = [   # [128, N] f32 coordinate maps -> blob 'cmap'
    ('c03', 288), ('cB3', 288),
    ('c01', 32), ('cB1', 32),
    ('wrapR', 1024),
]
_W_SPECS = [     # [128, N] f16 weights -> blob 'cw'
    ('wC', 162), ('wd3T', 320), ('woff1T', 2), ('wd1T', 32),
]
CONST_SPECS = [
    ('cvec', (128, len(_VEC_SPECS)), F32),
    ('cmap', (128, sum(n for _, n in _MAP_SPECS)), F32),
    ('cw', (128, sum(n for _, n in _W_SPECS)), F16),
]


def pack_constants(c):
    cvec = np.zeros((128, len(_VEC_SPECS)), np.float32)
    for i, (n, p) in enumerate(_VEC_SPECS):
        cvec[:p, i] = c[n].reshape(-1)
    cmap = np.concatenate([c[n].reshape(128, sz) for n, sz in _MAP_SPECS], axis=1)
    cw = np.concatenate([c[n].reshape(128, sz) for n, sz in _W_SPECS],
                        axis=1).astype(np.float16)
    return {'cvec': cvec.astype(np.float32), 'cmap': cmap.astype(np.float32),
            'cw': cw}


# --------------------------------------------------------------------------
# AP helpers
# --------------------------------------------------------------------------

def _rows(ap2d, off, rstride, nr, ncols):
    """[P, nr, ncols] view of a [P, N] AP: rows of length ncols, stride rstride."""
    v = ap2d[:, off:off + nr * rstride].rearrange('p (r q) -> p r q', q=rstride)
    return v[:, :, 0:ncols]


# --------------------------------------------------------------------------
# device program
# --------------------------------------------------------------------------

def build_nc():
    nc = bacc.Bacc()
    x_in = nc.declare_dram_parameter('x', [64, HW], F32, isOutput=False)
    consts = {}
    for name, shape, dt in CONST_SPECS:
        consts[name] = nc.declare_dram_parameter('c_' + name, list(shape), dt,
                                                 isOutput=False)
    out_ext = nc.declare_dram_parameter('out', [32, 2 * H, 2 * W], F32,
                                        isOutput=True)

    hT_dram = nc.dram_tensor('hT_dram', [HT3_COLS, 64], F16)
    quad3 = nc.dram_tensor('quad3', [NROW, 256], F16)
    h1Tx = nc.dram_tensor('h1Tx', [HT1_COLS, 64], F16)
    h1Tm = nc.dram_tensor('h1Tm', [HT1_COLS, 64], F16)
    quad1x = nc.dram_tensor('quad1x', [NROW, 128], F16)
    quad1m = nc.dram_tensor('quad1m', [NROW, 128], F16)
    gate = nc.dram_tensor('gate', [1, 16], F16)

    with tile.TileContext(nc) as tc:
        _body(nc, tc, x_in, consts, out_ext, hT_dram, quad3,
              h1Tx, h1Tm, quad1x, quad1m, gate)
    nc.finalize()
    return nc


def _coords_rows(nc, scratch, dyv, dxv, c0v, cBv, row_out, eng=None):
    """row = C0 + 68*sy + sx*CB; sy/sx persist in scratch for _coords_coefs."""
    sy, sx, ta, tb = scratch
    V = eng or nc.vector
    V.tensor_scalar(out=sy, in0=dyv, scalar1=0.0, scalar2=None, op0=ALU.is_ge)
    V.tensor_scalar(out=sx, in0=dxv, scalar1=0.0, scalar2=None, op0=ALU.is_ge)
    V.scalar_tensor_tensor(out=ta, in0=sy, scalar=68.0, in1=c0v,
                           op0=ALU.mult, op1=ALU.add)
    V.tensor_tensor(out=tb, in0=sx, in1=cBv, op=ALU.mult)
    V.tensor_tensor(out=row_out, in0=ta, in1=tb, op=ALU.add)


def _coords_coefs(nc, scratch, dyv, dxv, coef_out, eng=None,
                  split_corners=False):
    """Corner coefficients from dy/dx and the sy/sx left in scratch."""
    sy, sx, fy, fx = scratch
    V = eng or nc.vector
    # fy = dy + 1 - sy; gy = 1 - fy = sy - dy (reuse sy/sx slots for gy/gx)
    V.scalar_tensor_tensor(out=fy, in0=dyv, scalar=1.0, in1=sy,
                           op0=ALU.add, op1=ALU.subtract)
    V.scalar_tensor_tensor(out=fx, in0=dxv, scalar=1.0, in1=sx,
                           op0=ALU.add, op1=ALU.subtract)
    V.tensor_tensor(out=sy, in0=sy, in1=dyv, op=ALU.subtract)
    V.tensor_tensor(out=sx, in0=sx, in1=dxv, op=ALU.subtract)
    nd = coef_out.ndim - (3 if split_corners else 2)
    for i, (a, b) in enumerate([(sy, sx), (sy, fx), (fy, sx), (fy, fx)]):
        for j in range(2):
            idx = (i // 2, i % 2, j) if split_corners else (i, j)
            V.tensor_tensor(out=coef_out[(slice(None),) * nd + idx],
                            in0=a, in1=b, op=ALU.mult)


def _wrap_idx(nc, rowi16_v, wrapped, eng=None):
    """rowi16_v: [128, nk, nch] i16 (contiguous) -> wrapped [128, nk, 256]:
    wrapped[q, k, chunk*8 + r] = row[16r+q, k, chunk], replicated to the 8
    16-partition groups."""
    eng = eng or nc.sync
    for r in range(8):
        eng.dma_start(out=wrapped[0:16, :, r::8],
                      in_=rowi16_v[16 * r:16 * r + 16, :, :])
    for gsz in (16, 32, 64):
        eng.dma_start(out=wrapped[gsz:2 * gsz, :, :],
                      in_=wrapped[0:gsz, :, :])


def _quad_build(nc, src_dram, dst_dram, pp, chans, b0=0, b1=NB,
                parities=None, eng=None):
    """DRAM->DRAM DMAs (3-dim APs) building the quad-parity block table
    (by-outer layout: row = 4*NB*by + NB*(2*ay+ax) + bx) for block rows
    by in [b0, b1); optionally only some (ay, ax) parities."""
    q = 2 * pp
    nb = b1 - b0
    for ay in range(2):
        for ax in range(2):
            if parities is not None and (ay, ax) not in parities:
                continue
            s = ay * 2 + ax
            for yy in range(2):
                r0 = (ay + yy) * pp + ax + b0 * q
                sv = src_dram[:][r0:r0 + nb * q, :] \
                    .rearrange('(by q) c -> by q c', q=q)[:, 0:2 * NB, :] \
                    .rearrange('by (bx xx) c -> by bx (xx c)', xx=2)
                dv = dst_dram[4 * NB * b0:4 * NB * b1,
                              2 * chans * yy:2 * chans * (yy + 1)] \
                    .rearrange('(by sx) e -> by sx e', sx=4 * NB)[
                        :, NB * s:NB * s + NB, :]
                (eng or nc.sync).dma_start(out=dv, in_=sv)


def _xpar_build(nc, src_dram, dst_dram, p0, p1, eng=None):
    """x-parity table for the 1x1 deform: dst row 68*py + 34*ax + px2 holds
    the 128 f16 of padded positions (68*py + 2*px2 + ax, +1). One DMA per ax
    with 8.5KB-contiguous dst runs; builds py in [p0, p1)."""
    sflat = src_dram[:].rearrange('r c -> (r c)')
    for ax in range(2):
        base = 64 * (PP1 * p0 + ax)
        sv = sflat[base:base + (p1 - p0) * 64 * PP1] \
            .rearrange('(py q) -> py q', q=64 * PP1) \
            .rearrange('py (px2 e) -> py px2 e', e=128)[:, 0:NB, :]
        dv = dst_dram[PP1 * p0:PP1 * p1, :] \
            .rearrange('(py sx) e -> py sx e', sx=PP1)[:, NB * ax:
                                                       NB * ax + NB, :]
        (eng or nc.sync).dma_start(out=dv, in_=sv)


def _pad_memset(nc, t, npart, pp, pad, w, ncols):
    """Zero only the padding cells of a padded image tile t [npart, ncols]."""
    head = pad * pp + pad
    nc.gpsimd.memset(t[0:npart, 0:head], 0.0)
    gapw = pp - w
    r0, r1 = pad, pad + w  # gap r covers trail of row r / lead of row r+1
    ngap = r1 - r0 - 1
    gv = _rows(t[0:npart, :], r0 * pp + pad + w, pp, ngap, gapw)
    nc.gpsimd.memset(gv, 0.0)
    tail0 = (r1 - 1) * pp + pad + w
    nc.gpsimd.memset(t[0:npart, tail0:ncols], 0.0)



def _body(nc, tc, x_in, consts, out_ext, hT_dram, quad3,
          h1Tx, h1Tm, quad1x, quad1m, gate):
    V, S, G, PE, SY = nc.vector, nc.scalar, nc.gpsimd, nc.tensor, nc.sync

    with (
        tc.tile_pool(name='persist', bufs=1) as pp,
        tc.tile_pool(name='psum', bufs=2, space='PSUM') as psp,
        tc.tile_pool(name='psumv', bufs=2, space='PSUM') as psv,
    ):
        # ---------------- constants / persistent tiles -------------------
        blobs = {}
        for name, shape, dt in CONST_SPECS:
            t = pp.tile(list(shape), dt, tag='c_' + name, name='c_' + name)
            blobs[name] = t
        C = {}
        for i, (n, p_) in enumerate(_VEC_SPECS):
            C[n] = blobs['cvec'][0:p_, i:i + 1]
        col = 0
        for n, sz in _MAP_SPECS:
            C[n] = blobs['cmap'][:, col:col + sz]
            col += sz
        col = 0
        for n, sz in _W_SPECS:
            C[n] = blobs['cw'][:, col:col + sz]
            col += sz
        idt32 = pp.tile([32, 32], F32, tag='idt32', name='idt32')
        idt128h = pp.tile([128, 128], F16, tag='idt128h', name='idt128h')
        h1pad = pp.tile([128, HT1_COLS], F16, tag='h1pad', name='h1pad')
        coefD = pp.tile([128, 9, NCH, 4, 2], F16, tag='coefD', name='coefD')
        coef1D = pp.tile([128, 2, NCH, 2, 2], F16, tag='coef1D',
                 name='coef1D')
        wrapped3 = pp.tile([128, 9, 256], I16, tag='wrapped3', name='wrapped3')
        wrapped1 = pp.tile([128, 2, 256], I16, tag='wrapped1', name='wrapped1')
        h1Tst = pp.tile([128, 37, 128], F16, tag='h1Tst', name='h1Tst')
        off3T = pp.tile([128, NCH, 18], F32, tag='off3T', name='off3T')
        off1T = pp.tile([128, NCH, 2], F32, tag='off1T', name='off1T')

        # ---------------- phase A: bn3, transposes, conv, coords ---------
        with tc.tile_pool(name='ph1', bufs=1) as ph1, \
             tc.tile_pool(name='oev', bufs=2) as oev:
            x2 = ph1.tile([64, HW], F32, tag='x2', name='x2')
            hpad2 = ph1.tile([128, HT3_COLS], F16, tag='hpad2',
                             name='hpad2')
            # cvec first (gates bn3), then x, then cw/cmap (needed later)
            SY.dma_start(out=blobs['cvec'][:, :], in_=consts['cvec'][:])
            SY.dma_start(out=x2[:, 0:2048], in_=x_in[:][:, 0:2048])
            SY.dma_start(out=x2[:, 2048:HW], in_=x_in[:][:, 2048:HW])
            SY.dma_start(out=blobs['cw'][:, :], in_=consts['cw'][:])
            SY.dma_start(out=blobs['cmap'][:, :], in_=consts['cmap'][:])
            make_identity(nc, idt32[:, :])
            make_identity(nc, idt128h[:, :])
            _pad_memset(nc, hpad2, 128, PP3, PAD3, W, HT3_COLS)
            _pad_memset(nc, h1pad, 128, PP1, PAD1, W, HT1_COLS)
            xv = x2[:, :].rearrange('p (r c) -> p r c', r=H)

            # bn3 in two row-chunks (pipelines with the x DMA halves);
            # partitions 64:128 hold the same rows shifted up one padded
            # row so the conv can pair taps (ky, ky+1) with K=128
            for half in range(2):
                S.activation(_rows(hpad2[0:64, :],
                                   (PAD3 + 32 * half) * PP3 + PAD3,
                                   PP3, 32, W),
                             xv[0:64, 32 * half:32 * half + 32, :], AF.Relu,
                             bias=C['t3'], scale=C['s3'])
            for half in range(2):
                c0 = (PAD3 - 1 + 32 * half) * PP3
                SY.dma_start(out=hpad2[64:128, c0:c0 + 32 * PP3],
                             in_=hpad2[0:64, c0 + PP3:c0 + 33 * PP3])
            # hT transposes: [64, 128] -> [128, 64] per 128-col chunk;
            # store + quad3 build in two stages so the table is ready early
            hTst = ph1.tile([128, 39, 64], F16, tag='hTst', name='hTst')
            for t0 in range(0, 39, 4):
                nt = min(4, 39 - t0)
                pv = psv.tile([128, 512], F16, tag='pv', name='pv')
                for j in range(nt):
                    PE.transpose(pv[:, 64 * j:64 * j + 64],
                                 hpad2[0:64,
                                       128 * (t0 + j):128 * (t0 + j) + 128],
                                 idt128h[0:64, 0:64])
                if (t0 // 4) % 2 == 1:
                    V.tensor_copy(out=hTst[:, t0:t0 + nt, :],
                                  in_=pv[:, 0:64 * nt])
                else:
                    S.activation(hTst[:, t0:t0 + nt, :],
                                 pv[:, 0:64 * nt]
                                 .rearrange('p (a b) -> p a b', b=64),
                                 AF.Identity)
                if t0 == 16:
                    SY.dma_start(out=hT_dram[0:2560, :]
                                 .rearrange('(a p) c -> p a c', p=128),
                                 in_=hTst[:, 0:20, :])
                    _quad_build(nc, hT_dram, quad3, PP3, 64, 0, 17)
            SY.dma_start(out=hT_dram[2560:4992, :]
                         .rearrange('(a p) c -> p a c', p=128),
                         in_=hTst[:, 20:39, :])
            _quad_build(nc, hT_dram, quad3, PP3, 64, 17, NB)

            # conv3x3: 2 blocks x 4 psum chunks x 6 groups
            # (K=128 tap-pairs (ky=0,1) + K=64 singles (ky=2) per kx)
            for blk in range(2):
                pcs = [psp.tile([128, 512], F32, tag=f'pmm{i}', name=f'pc{i}',
                                bufs=1) for i in range(4)]
                for g in range(6):
                    kx, sub = g // 2, g % 2
                    ky = 0 if sub == 0 else 2
                    npt = 128 if sub == 0 else 64
                    for i in range(4):
                        ch = 4 * blk + i
                        rhs = _rows(hpad2[0:npt, :],
                                    (2 + ky) * PP3 + 2 + kx + 8 * PP3 * ch,
                                    PP3, 8, W)
                        PE.matmul(pcs[i][0:18, :],
                                  C['wC'][0:npt, 18 * g:18 * g + 18],
                                  rhs, start=(g == 0), stop=(g == 5))
                for i in range(4):
                    ch = 4 * blk + i
                    o3 = oev.tile([18, 512], F32, tag='o3', name='o3')
                    S.activation(o3[:, :], pcs[i][0:18, :], AF.Identity,
                                 bias=C['boff3'])
                    pt = psp.tile([128, 128], F32, tag='ptr', name='pt',
                                  bufs=1)
                    for t in range(4):
                        PE.transpose(pt[:, 18 * t:18 * t + 18],
                                     o3[:, 128 * t:128 * t + 128],
                                     idt32[0:18, 0:18])
                    V.tensor_copy(out=off3T[:, 4 * ch:4 * ch + 4, :]
                                  .rearrange('p a b -> p (a b)'),
                                  in_=pt[:, 0:72])

            # coordinates / gather rows -> wrap matmuls / coefficients
            sc = [ph1.tile([128, 288], F32, tag=f'sc{i}', name=f'sc{i}')
                  for i in range(4)]
            rowf3 = ph1.tile([128, 2, 9, 16], F32, tag='rowf3', name='rowf3')
            scv = [s[:, :].rearrange('p (a b) -> p a b', b=9) for s in sc]
            wr3v = wrapped3[:, :, :].rearrange('p k (h c r) -> p h k c r',
                                               h=2, r=8)
            cmv = [C[n].rearrange('p (a b) -> p a b', b=9)
                   for n in ('c03', 'cB3')]
            for hf in range(2):
                cs = slice(16 * hf, 16 * hf + 16)
                _coords_rows(nc,
                             [sv[:, cs, :] for sv in scv],
                             off3T[:, cs, 0:18:2], off3T[:, cs, 1:18:2],
                             cmv[0][:, cs, :], cmv[1][:, cs, :],
                             rowf3[:, hf, :, :].transpose([0, 2, 1]))
                for r in range(8):
                    ptag = ['pmm0', 'pmm1', 'pmm2', 'pmm3'][r % 4]
                    pw = psp.tile([128, 512], F32, tag=ptag, name='pw',
                                  bufs=1)
                    PE.matmul(pw[:, 0:144],
                              C['wrapR'][:, 128 * r:128 * r + 128],
                              rowf3[:, hf, :, :], start=True, stop=True)
                    if r % 2 == 0:
                        S.activation(wr3v[:, hf, :, :, r], pw[:, 0:144],
                                     AF.Identity)
                    else:
                        V.tensor_copy(out=wr3v[:, hf, :, :, r],
                                      in_=pw[:, 0:144])
                _coords_coefs(nc,
                              [sv[:, cs, :] for sv in scv],
                              off3T[:, cs, 0:18:2], off3T[:, cs, 1:18:2],
                              coefD[:, :, cs, :, :].transpose([0, 2, 1, 3, 4]))

            # h1 x-part: relu(bn1(x)) into h1pad interior (ACT, off path)
            S.activation(_rows(h1pad[0:64, :], PAD1 * PP1 + PAD1, PP1, H, W),
                         xv[0:64], AF.Relu, bias=C['t1x'], scale=C['s1x'])
            # h1Tst x-half transposes + x-side table build (DMA idle here)
            for t0 in range(0, 37, 4):
                nt = min(4, 37 - t0)
                pv = psv.tile([128, 512], F16, tag='pv', name='pv')
                for j in range(nt):
                    PE.transpose(pv[:, 64 * j:64 * j + 64],
                                 h1pad[0:64, 128 * (t0 + j):
                                       128 * (t0 + j) + 128],
                                 idt128h[0:64, 0:64])
                S.activation(h1Tst[:, t0:t0 + nt, 0:64],
                             pv[:, 0:64 * nt], AF.Identity)
            SY.dma_start(out=h1Tx[0:4736, :]
                         .rearrange('(a p) c -> p a c', p=128),
                         in_=h1Tst[:, :, 0:64])
            _xpar_build(nc, h1Tx, quad1x, 0, PP1)

        # ---------------- phase C: gathers + in-place combine ------------
        vhgs = {}
        pend_fadd = []
        with tc.tile_pool(name='gpool', bufs=4) as gp, \
             tc.tile_pool(name='vhp', bufs=3) as vhp:
            for pos, k in enumerate([0, 1, 8, 2, 3, 4, 5, 6, 7]):
                gp_i, sl = k // 2, k % 2
                if gp_i not in vhgs:
                    nt = 2 if gp_i < 4 else 1
                    tag = 'vh' if gp_i < 4 else 'vh4'
                    vhgs[gp_i] = vhp.tile([128, NCH, nt, 64], F16, tag=tag,
                                          name=f'vh{gp_i}')
                vhg = vhgs[gp_i]
                g = gp.tile([128, 8192], F16, tag='g', name='g')
                G.dma_gather(g[:, :].rearrange('p (a c) -> p a c', c=256),
                             quad3[:], wrapped3[:, k, :], 4096, 4096,
                             256, queue_num=0, single_packet=False)

                gq = g[:, :].rearrange('p (a b c d) -> p a b c d',
                                       a=NCH, b=4, c=32)
                cf = coefD[:, k][:, :, :, None, :].broadcast_to(
                    [128, NCH, 4, 32, 2])
                gw = g[:, :].rearrange('p (a b c) -> p a b c', a=NCH, b=4)
                nspl = 2 if pos == 8 else 1
                nh = NCH // nspl
                for sp in range(nspl):
                    chs = slice(nh * sp, nh * sp + nh)
                    V.tensor_tensor(out=gq[:, chs], in0=gq[:, chs],
                                    in1=cf[:, chs], op=ALU.mult)
                    with nc.allow_low_precision('fp16 middle precision'):
                        V.tensor_tensor(out=gw[:, chs, 0:2, :],
                                        in0=gw[:, chs, 0:2, :],
                                        in1=gw[:, chs, 2:4, :], op=ALU.add)
                        V.tensor_tensor(out=vhg[:, chs, sl, :],
                                        in0=gw[:, chs, 0, :],
                                        in1=gw[:, chs, 1, :], op=ALU.add)

            # ------------ phase C2/D: transposes + einsum3 ---------------
            sc1 = [pp.tile([128, 32], F32, tag=f't1s{i}', name=f't1s{i}')
                   for i in range(4)]
            rowf = pp.tile([128, 32], F32, tag='rowf', name='rowf')
            rowt = pp.tile([128, 16, 8], F32, tag='rowt', name='rowt')
            vhat1 = pp.tile([128, NCH, 2, 64], F16, tag='vhat1',
                            name='vhat1')
            wr1v = wrapped1[:, :, :].rearrange('p a (c r) -> p a c r', r=8)

            def _coords1_half(hf):
                hs = slice(16 * hf, 16 * hf + 16)
                _coords_rows(nc, [s[:, hs] for s in sc1],
                             off1T[:, hs, 0], off1T[:, hs, 1],
                             C['c01'][:, hs], C['cB1'][:, hs], rowf[:, hs])
                pw1 = psp.tile([128, 512], F32, tag='ptr', name='pw1',
                               bufs=1)
                for r in range(8):
                    PE.matmul(pw1[:, 16 * r:16 * r + 16],
                              C['wrapR'][:, 128 * r:128 * r + 128],
                              rowf[:, hs], start=True, stop=True)
                pwv = pw1[:, 0:128].rearrange('p (r c) -> p c r', c=16)
                V.tensor_copy(out=wr1v[:, 0, hs, :], in_=pwv)
                # second y-corner row sits one padded row (+PP1) below
                V.tensor_scalar(out=rowt[:, 0:16, :], in0=pwv,
                                scalar1=float(PP1), scalar2=None,
                                op0=ALU.add)
                V.tensor_copy(out=wr1v[:, 1, hs, :], in_=rowt[:, 0:16, :])
                _coords_coefs(nc, [s[:, hs] for s in sc1],
                              off1T[:, hs, 0], off1T[:, hs, 1],
                              coef1D[:, :, hs, :, :]
                              .transpose([0, 2, 1, 3, 4]),
                              split_corners=True)

            def _gather1(quadap, c0, nch):
                """Gather chunks [c0, c0+nch) of the 1x1 deform from the
                x-parity table: one gather per y-corner a, each row holding
                the two x-corners for 64 channels."""
                g1 = gp.tile([128, 8192], F16, tag='g', name='g1')
                for a in range(2):
                    G.dma_gather(g1[:, 4096 * a:4096 * a + 128 * nch]
                                 .rearrange('p (c e) -> p c e', e=128),
                                 quadap,
                                 wrapped1[:, a, 8 * c0:8 * (c0 + nch)],
                                 128 * nch, 128 * nch, 128, queue_num=0,
                                 single_packet=False)
                return g1[:, :].rearrange('p (a q) -> p a q', a=2)[
                    :, :, 0:128 * nch] \
                    .rearrange('p a (c b e) -> p a c b e', b=2, e=64)

            def _combine1(g1v, hx, c0, nch):
                for a in range(2):
                    gq = g1v[:, a].rearrange('p c b (f d) -> p c b f d', d=2)
                    cf = coef1D[:, a, c0:c0 + nch, :, :][
                        :, :, :, None, :].broadcast_to([128, nch, 2, 32, 2])
                    V.tensor_tensor(out=gq, in0=gq, in1=cf, op=ALU.mult)
                with nc.allow_low_precision('fp16 by design'):
                    V.tensor_tensor(out=g1v[:, 0], in0=g1v[:, 0],
                                    in1=g1v[:, 1], op=ALU.add)
                    V.tensor_tensor(
                        out=vhat1[:, c0:c0 + nch, hx, :],
                        in0=g1v[:, 0, :, 0, :], in1=g1v[:, 0, :, 1, :],
                        op=ALU.add)

            with tc.tile_pool(name='vpool', bufs=1) as vp:
                v = vp.tile([128, 5, HW], F16, tag='v', name='v')
                pms = {}

                def _mm3(ch, gp_i):
                    if ch not in pms:
                        pms[ch] = psp.tile([128, 512], F32,
                                           tag=f'pmm{ch % 4}',
                                           name=f'pm{ch}', bufs=1)
                    if gp_i < 4:
                        PE.matmul(pms[ch][64:128, :],
                                  C['wd3T'][:, 64 * gp_i:64 * gp_i + 64],
                                  v[:, gp_i, 512 * ch:512 * ch + 512],
                                  start=(gp_i == 0), stop=False)
                    else:
                        PE.matmul(pms[ch][64:128, :],
                                  C['wd3T'][0:64, 256:320],
                                  v[0:64, 4, 512 * ch:512 * ch + 512],
                                  start=False, stop=True)

                for gp_i in (0, 4, 1, 2, 3):
                    for ch4 in range(8):
                        pv = psv.tile([128, 512], F16, tag='pv', name='pv')
                        for sub in range(4):
                            ch = 4 * ch4 + sub
                            if gp_i < 4:
                                PE.transpose(
                                    pv[:, 128 * sub:128 * sub + 128],
                                    vhgs[gp_i][:, ch, :, :]
                                    .rearrange('p a b -> p (a b)'),
                                    idt128h[:, :])
                            else:
                                PE.transpose(
                                    pv[0:64, 128 * sub:128 * sub + 128],
                                    vhgs[4][:, ch, 0, :], idt128h[:, :])
                        np_ = 128 if gp_i < 4 else 64
                        if gp_i == 3 and ch4 % 2 == 1:
                            # last group lands post-combine; DVE is free then
                            V.tensor_copy(out=v[0:np_, gp_i,
                                                512 * ch4:512 * ch4 + 512],
                                          in_=pv[0:np_, :])
                        else:
                            S.activation(v[0:np_, gp_i,
                                           512 * ch4:512 * ch4 + 512],
                                         pv[0:np_, :], AF.Identity)
                    for ch in range(4):
                        _mm3(ch, gp_i)

                # ---- phase E: evacs + off1 + tables + 1x1 gathers -------
                estate = {'tch': 0, 'pt1': None}
                with tc.tile_pool(name='oev1', bufs=2) as oev1:

                    def _evac_chunk(ch):
                        hv = _rows(h1pad[64:128, :],
                                   (8 * ch + PAD1) * PP1 + PAD1, PP1, 8, W)
                        pmv = pms[ch][64:128, :] \
                            .rearrange('p (r c) -> p r c', r=8)
                        S.activation(hv, pmv, AF.Relu, bias=C['t1m'],
                                     scale=C['s1m'])
                        pc1 = psp.tile([128, 512], F32, tag='pc1',
                                       name='pc1', bufs=1)
                        PE.matmul(pc1[0:2, :], C['woff1T'],
                                  _rows(h1pad[0:128, :],
                                        (8 * ch + PAD1) * PP1 + PAD1,
                                        PP1, 8, W),
                                  start=True, stop=True)
                        o1 = oev1.tile([2, 512], F32, tag='o1', name='o1')
                        if ch % 2 == 0:
                            S.activation(o1[:, :], pc1[0:2, :], AF.Identity,
                                         bias=C['boff1'])
                        else:
                            V.tensor_scalar(out=o1[:, :], in0=pc1[0:2, :],
                                            scalar1=C['boff1'], scalar2=None,
                                            op0=ALU.add)
                        if ch % 2 == 0:
                            estate['pt1'] = psp.tile([128, 128], F32,
                                                     tag='ptr', name='pt1',
                                                     bufs=1)
                        pt1 = estate['pt1']
                        for t in range(4):
                            PE.transpose(pt1[:, 8 * (ch % 2) + 2 * t:
                                             8 * (ch % 2) + 2 * t + 2],
                                         o1[:, 128 * t:128 * t + 128],
                                         idt32[0:2, 0:2])
                        if ch % 2 == 1:
                            V.tensor_copy(out=off1T[:, 8 * (ch // 2):
                                                    8 * (ch // 2) + 8, :]
                                          .rearrange('p a b -> p (a b)'),
                                          in_=pt1[:, 0:16])
                        # h1Tst mid-half transposes ready with this chunk
                        r_hi_ready = 2 + 8 * ch + 8
                        ready = []
                        while estate['tch'] < 37:
                            tch = estate['tch']
                            r_hi = (128 * tch + 127) // PP1
                            if r_hi >= r_hi_ready and ch < 7:
                                break
                            ready.append(tch)
                            estate['tch'] += 1
                        for i0 in range(0, len(ready), 4):
                            grp = ready[i0:i0 + 4]
                            pv = psv.tile([128, 512], F16, tag='pv',
                                          name='pv')
                            for j, tch in enumerate(grp):
                                PE.transpose(pv[:, 64 * j:64 * j + 64],
                                             h1pad[64:128,
                                                   128 * tch:128 * tch + 128],
                                             idt128h[64:128, 64:128])
                            t0g = grp[0]
                            estate['evp'] = estate.get('evp', 0) + 1
                            if estate['evp'] % 2 == 1:
                                S.activation(h1Tst[:, t0g:t0g + len(grp),
                                                   64:128],
                                             pv[:, 0:64 * len(grp)]
                                             .rearrange('p (a b) -> p a b',
                                                        b=64),
                                             AF.Identity)
                            else:
                                V.tensor_copy(out=h1Tst[:, t0g:t0g + len(grp),
                                                        64:128],
                                              in_=pv[:, 0:64 * len(grp)])
                        # staged h1Tm stores + x-parity table builds: stage s
                        # covers padded rows [QB[s], QB[s+1]) and needs h1Tst
                        # cols [QC[s], QC[s+1])
                        QB = [0, 35, PP1]
                        QC = [0, 19, 37]
                        st = {4: 0, 7: 1}.get(ch)
                        if st is not None:
                            SY.dma_start(
                                out=h1Tm[128 * QC[st]:128 * QC[st + 1], :]
                                .rearrange('(a p) c -> p a c', p=128),
                                in_=h1Tst[:, QC[st]:QC[st + 1], 64:128])
                            _xpar_build(nc, h1Tm, quad1m, QB[st], QB[st + 1])

                    for ch in range(4):
                        _evac_chunk(ch)
                    # 1x1 half 0: coords + x gather issued ASAP
                    _coords1_half(0)
                    g1xh0 = _gather1(quad1x[:], 0, 16)
                    for gp_i in (0, 4, 1, 2, 3):
                        for ch in range(4, 8):
                            _mm3(ch, gp_i)
                    _evac_chunk(4)
                    g1mq1 = _gather1(quad1m[:][0:PP1 * 35, :], 0, 8)
                    g1mq2 = _gather1(quad1m[:][0:PP1 * 35, :], 8, 8)
                    _evac_chunk(5)
                    _combine1(g1xh0, 0, 0, 16)
                    _evac_chunk(6)
                    _combine1(g1mq1, 1, 0, 8)
                    _evac_chunk(7)
                    _combine1(g1mq2, 1, 8, 8)
                    _coords1_half(1)
                    g1xh1 = _gather1(quad1x[:], 16, 16)
                    g1mq3 = _gather1(quad1m[:], 16, 8)
                    g1mq4 = _gather1(quad1m[:], 24, 8)
                    _combine1(g1xh1, 0, 16, 16)
                    _combine1(g1mq3, 1, 16, 8)
                    _combine1(g1mq4, 1, 24, 8)

            # ---------------- v1 transposes + einsum1 + upsample ---------
            with tc.tile_pool(name='tailp', bufs=1) as tp:
                yd = tp.tile([32, H, 2 * W], F32, tag='yd', name='yd')
                v1s = tp.tile([128, 8, 512], F16, tag='v1s', name='v1s')
                for ch4 in range(8):
                    pv = psv.tile([128, 512], F16, tag='pv', name='pv')
                    for sub in range(4):
                        PE.transpose(pv[:, 128 * sub:128 * sub + 128],
                                     vhat1[:, 4 * ch4 + sub, :, :]
                                     .rearrange('p a b -> p (a b)'),
                                     idt128h[:, :])
                    if ch4 % 2 == 0:
                        S.activation(v1s[:, ch4, :], pv[:, :], AF.Identity)
                    else:
                        V.tensor_copy(out=v1s[:, ch4, :], in_=pv[:, :])
                    pmy = psp.tile([128, 512], F32, tag=f'pmm{ch4 % 4}',
                                   name='pmy', bufs=1)
                    PE.matmul(pmy[0:32, :], C['wd1T'], v1s[:, ch4, :],
                              start=True, stop=True)
                    pmv = pmy[0:32, :].rearrange('p (r c) -> p r c', r=8)
                    S.activation(yd[:, 8 * ch4:8 * ch4 + 8, 0::2], pmv,
                                 AF.Identity, bias=C['bd1'])
                    V.tensor_scalar(out=yd[:, 8 * ch4:8 * ch4 + 8, 1::2],
                                    in0=pmv, scalar1=C['bd1'], scalar2=None,
                                    op0=ALU.add)
                    if ch4 % 2 == 1:
                        gq = ch4 // 2
                        SY.dma_start(out=out_ext[:, 32 * gq:32 * gq + 32:2,
                                                 :],
                                     in_=yd[:, 16 * gq:16 * gq + 16, :])
                        SY.dma_start(out=out_ext[:,
                                                 32 * gq + 1:32 * gq + 32:2,
                                                 :],
                                     in_=yd[:, 16 * gq:16 * gq + 16, :])



# --------------------------------------------------------------------------
# host entry point
# --------------------------------------------------------------------------

_CACHE = {}


def kernel(**inputs):
    x = np.ascontiguousarray(inputs['x'], np.float32)      # [8, 64, 64, 64]
    B = x.shape[0]
    consts = host_constants(inputs)

    if 'nc' not in _CACHE:
        _CACHE['nc'] = build_nc()
    nc = _CACHE['nc']

    packed = pack_constants(consts)
    in_maps = []
    for b in range(B):
        m = {'x': x[b].reshape(64, HW)}
        for name, shape, dt in CONST_SPECS:
            m['c_' + name] = packed[name]
        in_maps.append(m)

    res = run_bass_kernel_spmd(nc, in_maps, list(range(B)))
    out = np.stack([res.results[b]['out'] for b in range(B)])
    return out.astype(np.float32)



# revision 4
# speedup vs baseline: 1.0328x; 1.0010x over previous
"""Trainium2 Bass kernel v2 for nn_DeformableUpsampleBlock (fixed instance).

Same algorithm family as the baseline (quad-parity gather tables + on-PE
einsums), restructured around the TimelineSim cost model:
  - x loaded once ([64, HW]); conv3x3 uses K=64 contraction (no shifted copy)
  - pad-strip memsets instead of full-tile memsets
  - bilinear corner coefficients stored pair-duplicated so the big combine
    multiplies hit the DVE 2x fast mode; all combine work on DVE
  - second deform's table split into x/mid halves so the x half is built
    during phase C and only the mid half sits on the post-einsum tail
  - einsum1 / yd evacuation / output DMA pipelined per chunk
"""

import numpy as np

import concourse.bass as bass
import concourse.mybir as mybir
from concourse import bacc
import concourse.tile as tile
from concourse.bass_utils import run_bass_kernel_spmd
from concourse.masks import make_identity

F32 = mybir.dt.float32
F16 = mybir.dt.float16
I16 = mybir.dt.int16
AF = mybir.ActivationFunctionType
ALU = mybir.AluOpType

H = W = 64
HW = H * W              # 4096
NCH = 32                # pixel chunks of 128; pixel p -> [p % 128, p // 128]
PAD3 = 3
PP3 = H + 2 * PAD3      # 70
PAD1 = 2
PP1 = H + 2 * PAD1      # 68
NB = 34                 # quad blocks per side (both tables)
NROW = 4 * NB * NB      # 4624
HT3_COLS = 4992         # 39*128 >= 70*70 (+ quad-build overread)
HT1_COLS = 4864         # 38*128; quad build reads to 4761
EPS = 1e-5


# --------------------------------------------------------------------------
# host-side constants
# --------------------------------------------------------------------------

def _f16(a):
    return np.ascontiguousarray(a).astype(np.float16)


def host_constants(p):
    c = {}
    inv3 = (1.0 / np.sqrt(p['bn3_var'].astype(np.float64) + EPS)).astype(np.float32)
    s3 = (p['bn3_gamma'] * inv3).astype(np.float32)
    t3 = (p['bn3_beta'] - p['bn3_mean'] * s3).astype(np.float32)
    c['s3'] = s3.reshape(64, 1).copy()
    c['t3'] = t3.reshape(64, 1).copy()

    inv1 = (1.0 / np.sqrt(p['bn1_var'].astype(np.float64) + EPS)).astype(np.float32)
    s1 = (p['bn1_gamma'] * inv1).astype(np.float32)
    t1 = (p['bn1_beta'] - p['bn1_mean'] * s1).astype(np.float32)
    c['s1x'] = s1[:64].reshape(64, 1).copy()
    c['t1x'] = t1[:64].reshape(64, 1).copy()
    c['s1m'] = s1[64:].reshape(64, 1).copy()
    c['t1m'] = (t1[64:] + s1[64:] * p['b_d3']).reshape(64, 1).astype(np.float32)

    w3 = p['w_off3'].astype(np.float32)          # [18, 64, 3, 3]
    # 6 conv groups: per kx a K=128 pair (ky=0 in parts 0:64, ky=1 in the
    # row-shifted parts 64:128) plus a K=64 single (ky=2)
    wC = np.zeros((128, 162), np.float32)
    for kx in range(3):
        wC[:64, 36 * kx:36 * kx + 18] = w3[:, :, 0, kx].T
        wC[64:, 36 * kx:36 * kx + 18] = w3[:, :, 1, kx].T
        wC[:64, 36 * kx + 18:36 * kx + 36] = w3[:, :, 2, kx].T
    c['wC'] = _f16(wC)
    c['boff3'] = p['b_off3'].astype(np.float32).reshape(18, 1)
    c['boff1'] = p['b_off1'].astype(np.float32).reshape(2, 1)

    wd3 = p['w_d3'].astype(np.float32).reshape(64, 64, 9)    # [o, c, k]
    wt = np.zeros((128, 320), np.float32)
    for g in range(5):
        for part in range(128):
            kap = 128 * g + part
            if kap < 576:
                wt[part, 64 * g:64 * g + 64] = wd3[:, kap % 64, kap // 64]
    c['wd3T'] = _f16(wt)

    c['woff1T'] = _f16(p['w_off1'].reshape(2, 128).T)
    c['wd1T'] = _f16(p['w_d1'].reshape(32, 128).T)
    c['bd1'] = p['b_d1'].astype(np.float32).reshape(32, 1)

    part = np.arange(128)[:, None]
    chunk = np.arange(NCH)[None, :]
    pix = chunk * 128 + part
    ymap = (pix // W).astype(np.float32)
    xmap = (pix % W).astype(np.float32)
    yb3 = np.zeros((128, NCH, 9), np.float32)
    xb3 = np.zeros((128, NCH, 9), np.float32)
    for k in range(9):
        yb3[:, :, k] = ymap + (k // 3 + PAD3 - 2)
        xb3[:, :, k] = xmap + (k % 3 + PAD3 - 2)
    yb1 = ymap + (PAD1 - 1)
    xb1 = xmap + (PAD1 - 1)

    # quad tables are laid out by-outer: row = 136*by + 34*(2*ay+ax) + bx.
    # With Y0 = yb+sy, X0 = xb+sx, ay = Y0%2, ax = X0%2 the ay terms cancel:
    # row = 68*Y0 + 0.5*X0 + 33.5*ax = C0 + 68*sy + sx*CB (exact in f32)
    def _rowconsts(yb, xb):
        pbx = np.mod(xb, 2.0)
        c0 = 68.0 * yb + 0.5 * xb + 33.5 * pbx
        cb = 34.0 - 67.0 * pbx
        return c0.astype(np.float32), cb.astype(np.float32)

    c03, cB3 = _rowconsts(yb3, xb3)
    c['c03'] = c03.reshape(128, 288)
    c['cB3'] = cB3.reshape(128, 288)
    c01, cB1 = _rowconsts(yb1, xb1)
    c['c01'] = c01
    c['cB1'] = cB1
    # wrap matrices: wrapR[p, 128*r + q] = 1 iff p == 16*r + q%16
    wrapR = np.zeros((128, 1024), np.float32)
    for r in range(8):
        for q in range(128):
            wrapR[16 * r + q % 16, 128 * r + q] = 1.0
    c['wrapR'] = wrapR
    return c


_VEC_SPECS = [   # [P<=128, 1] f32 per-partition vectors -> blob 'cvec'
    ('s3', 64), ('t3', 64), ('s1x', 64), ('t1x', 64), ('s1m', 64),
    ('t1m', 64), ('boff3', 18), ('boff1', 2), ('bd1', 32),
]
_MAP_SPECS = [   # [128, N] f32 coordinate maps -> blob 'cmap'
    ('c03', 288), ('cB3', 288),
    ('c01', 32), ('cB1', 32),
    ('wrapR', 1024),
]
_W_SPECS = [     # [128, N] f16 weights -> blob 'cw'
    ('wC', 162), ('wd3T', 320), ('woff1T', 2), ('wd1T', 32),
]
CONST_SPECS = [
    ('cvec', (128, len(_VEC_SPECS)), F32),
    ('cmap', (128, sum(n for _, n in _MAP_SPECS)), F32),
    ('cw', (128, sum(n for _, n in _W_SPECS)), F16),
]


def pack_constants(c):
    cvec = np.zeros((128, len(_VEC_SPECS)), np.float32)
    for i, (n, p) in enumerate(_VEC_SPECS):
        cvec[:p, i] = c[n].reshape(-1)
    cmap = np.concatenate([c[n].reshape(128, sz) for n, sz in _MAP_SPECS], axis=1)
    cw = np.concatenate([c[n].reshape(128, sz) for n, sz in _W_SPECS],
                        axis=1).astype(np.float16)
    return {'cvec': cvec.astype(np.float32), 'cmap': cmap.astype(np.float32),
            'cw': cw}


# --------------------------------------------------------------------------
# AP helpers
# --------------------------------------------------------------------------

def _rows(ap2d, off, rstride, nr, ncols):
    """[P, nr, ncols] view of a [P, N] AP: rows of length ncols, stride rstride."""
    v = ap2d[:, off:off + nr * rstride].rearrange('p (r q) -> p r q', q=rstride)
    return v[:, :, 0:ncols]


# --------------------------------------------------------------------------
# device program
# --------------------------------------------------------------------------

def build_nc():
    nc = bacc.Bacc()
    x_in = nc.declare_dram_parameter('x', [64, HW], F32, isOutput=False)
    consts = {}
    for name, shape, dt in CONST_SPECS:
        consts[name] = nc.declare_dram_parameter('c_' + name, list(shape), dt,
                                                 isOutput=False)
    out_ext = nc.declare_dram_parameter('out', [32, 2 * H, 2 * W], F32,
                                        isOutput=True)

    hT_dram = nc.dram_tensor('hT_dram', [HT3_COLS, 64], F16)
    quad3 = nc.dram_tensor('quad3', [NROW, 256], F16)
    h1Tx = nc.dram_tensor('h1Tx', [HT1_COLS, 64], F16)
    h1Tm = nc.dram_tensor('h1Tm', [HT1_COLS, 64], F16)
    quad1x = nc.dram_tensor('quad1x', [NROW, 128], F16)
    quad1m = nc.dram_tensor('quad1m', [NROW, 128], F16)
    gate = nc.dram_tensor('gate', [1, 16], F16)

    with tile.TileContext(nc) as tc:
        _body(nc, tc, x_in, consts, out_ext, hT_dram, quad3,
              h1Tx, h1Tm, quad1x, quad1m, gate)
    nc.finalize()
    return nc


def _coords_rows(nc, scratch, dyv, dxv, c0v, cBv, row_out, eng=None):
    """row = C0 + 68*sy + sx*CB; sy/sx persist in scratch for _coords_coefs."""
    sy, sx, ta, tb = scratch
    V = eng or nc.vector
    V.tensor_scalar(out=sy, in0=dyv, scalar1=0.0, scalar2=None, op0=ALU.is_ge)
    V.tensor_scalar(out=sx, in0=dxv, scalar1=0.0, scalar2=None, op0=ALU.is_ge)
    V.scalar_tensor_tensor(out=ta, in0=sy, scalar=68.0, in1=c0v,
                           op0=ALU.mult, op1=ALU.add)
    V.tensor_tensor(out=tb, in0=sx, in1=cBv, op=ALU.mult)
    V.tensor_tensor(out=row_out, in0=ta, in1=tb, op=ALU.add)


def _coords_coefs(nc, scratch, dyv, dxv, coef_out, eng=None,
                  split_corners=False):
    """Corner coefficients from dy/dx and the sy/sx left in scratch."""
    sy, sx, fy, fx = scratch
    V = eng or nc.vector
    # fy = dy + 1 - sy; gy = 1 - fy = sy - dy (reuse sy/sx slots for gy/gx)
    V.scalar_tensor_tensor(out=fy, in0=dyv, scalar=1.0, in1=sy,
                           op0=ALU.add, op1=ALU.subtract)
    V.scalar_tensor_tensor(out=fx, in0=dxv, scalar=1.0, in1=sx,
                           op0=ALU.add, op1=ALU.subtract)
    V.tensor_tensor(out=sy, in0=sy, in1=dyv, op=ALU.subtract)
    V.tensor_tensor(out=sx, in0=sx, in1=dxv, op=ALU.subtract)
    nd = coef_out.ndim - (3 if split_corners else 2)
    for i, (a, b) in enumerate([(sy, sx), (sy, fx), (fy, sx), (fy, fx)]):
        for j in range(2):
            idx = (i // 2, i % 2, j) if split_corners else (i, j)
            V.tensor_tensor(out=coef_out[(slice(None),) * nd + idx],
                            in0=a, in1=b, op=ALU.mult)


def _wrap_idx(nc, rowi16_v, wrapped, eng=None):
    """rowi16_v: [128, nk, nch] i16 (contiguous) -> wrapped [128, nk, 256]:
    wrapped[q, k, chunk*8 + r] = row[16r+q, k, chunk], replicated to the 8
    16-partition groups."""
    eng = eng or nc.sync
    for r in range(8):
        eng.dma_start(out=wrapped[0:16, :, r::8],
                      in_=rowi16_v[16 * r:16 * r + 16, :, :])
    for gsz in (16, 32, 64):
        eng.dma_start(out=wrapped[gsz:2 * gsz, :, :],
                      in_=wrapped[0:gsz, :, :])


def _quad_build(nc, src_dram, dst_dram, pp, chans, b0=0, b1=NB,
                parities=None, eng=None):
    """DRAM->DRAM DMAs (3-dim APs) building the quad-parity block table
    (by-outer layout: row = 4*NB*by + NB*(2*ay+ax) + bx) for block rows
    by in [b0, b1); optionally only some (ay, ax) parities."""
    q = 2 * pp
    nb = b1 - b0
    for ay in range(2):
        for ax in range(2):
            if parities is not None and (ay, ax) not in parities:
                continue
            s = ay * 2 + ax
            for yy in range(2):
                r0 = (ay + yy) * pp + ax + b0 * q
                sv = src_dram[:][r0:r0 + nb * q, :] \
                    .rearrange('(by q) c -> by q c', q=q)[:, 0:2 * NB, :] \
                    .rearrange('by (bx xx) c -> by bx (xx c)', xx=2)
                dv = dst_dram[4 * NB * b0:4 * NB * b1,
                              2 * chans * yy:2 * chans * (yy + 1)] \
                    .rearrange('(by sx) e -> by sx e', sx=4 * NB)[
                        :, NB * s:NB * s + NB, :]
                (eng or nc.sync).dma_start(out=dv, in_=sv)


def _xpar_build(nc, src_dram, dst_dram, p0, p1, eng=None):
    """x-parity table for the 1x1 deform: dst row 68*py + 34*ax + px2 holds
    the 128 f16 of padded positions (68*py + 2*px2 + ax, +1). One DMA per ax
    with 8.5KB-contiguous dst runs; builds py in [p0, p1)."""
    sflat = src_dram[:].rearrange('r c -> (r c)')
    for ax in range(2):
        base = 64 * (PP1 * p0 + ax)
        sv = sflat[base:base + (p1 - p0) * 64 * PP1] \
            .rearrange('(py q) -> py q', q=64 * PP1) \
            .rearrange('py (px2 e) -> py px2 e', e=128)[:, 0:NB, :]
        dv = dst_dram[PP1 * p0:PP1 * p1, :] \
            .rearrange('(py sx) e -> py sx e', sx=PP1)[:, NB * ax:
                                                       NB * ax + NB, :]
        (eng or nc.sync).dma_start(out=dv, in_=sv)


def _pad_memset(nc, t, npart, pp, pad, w, ncols):
    """Zero only the padding cells of a padded image tile t [npart, ncols]."""
    head = pad * pp + pad
    nc.gpsimd.memset(t[0:npart, 0:head], 0.0)
    gapw = pp - w
    r0, r1 = pad, pad + w  # gap r covers trail of row r / lead of row r+1
    ngap = r1 - r0 - 1
    gv = _rows(t[0:npart, :], r0 * pp + pad + w, pp, ngap, gapw)
    nc.gpsimd.memset(gv, 0.0)
    tail0 = (r1 - 1) * pp + pad + w
    nc.gpsimd.memset(t[0:npart, tail0:ncols], 0.0)



def _body(nc, tc, x_in, consts, out_ext, hT_dram, quad3,
          h1Tx, h1Tm, quad1x, quad1m, gate):
    V, S, G, PE, SY = nc.vector, nc.scalar, nc.gpsimd, nc.tensor, nc.sync

    with (
        tc.tile_pool(name='persist', bufs=1) as pp,
        tc.tile_pool(name='psum', bufs=2, space='PSUM') as psp,
        tc.tile_pool(name='psumv', bufs=2, space='PSUM') as psv,
    ):
        # ---------------- constants / persistent tiles -------------------
        blobs = {}
        for name, shape, dt in CONST_SPECS:
            t = pp.tile(list(shape), dt, tag='c_' + name, name='c_' + name)
            blobs[name] = t
        C = {}
        for i, (n, p_) in enumerate(_VEC_SPECS):
            C[n] = blobs['cvec'][0:p_, i:i + 1]
        col = 0
        for n, sz in _MAP_SPECS:
            C[n] = blobs['cmap'][:, col:col + sz]
            col += sz
        col = 0
        for n, sz in _W_SPECS:
            C[n] = blobs['cw'][:, col:col + sz]
            col += sz
        idt32 = pp.tile([32, 32], F32, tag='idt32', name='idt32')
        idt128h = pp.tile([128, 128], F16, tag='idt128h', name='idt128h')
        h1pad = pp.tile([128, HT1_COLS], F16, tag='h1pad', name='h1pad')
        coefD = pp.tile([128, 9, NCH, 4, 2], F16, tag='coefD', name='coefD')
        coef1D = pp.tile([128, 2, NCH, 2, 2], F16, tag='coef1D',
                 name='coef1D')
        wrapped3 = pp.tile([128, 9, 256], I16, tag='wrapped3', name='wrapped3')
        wrapped1 = pp.tile([128, 2, 256], I16, tag='wrapped1', name='wrapped1')
        h1Tst = pp.tile([128, 37, 128], F16, tag='h1Tst', name='h1Tst')
        off3T = pp.tile([128, NCH, 18], F32, tag='off3T', name='off3T')
        off1T = pp.tile([128, NCH, 2], F32, tag='off1T', name='off1T')

        # ---------------- phase A: bn3, transposes, conv, coords ---------
        with tc.tile_pool(name='ph1', bufs=1) as ph1, \
             tc.tile_pool(name='oev', bufs=2) as oev:
            x2 = ph1.tile([64, HW], F32, tag='x2', name='x2')
            hpad2 = ph1.tile([128, HT3_COLS], F16, tag='hpad2',
                             name='hpad2')
            # cvec first (gates bn3), then x, then cw/cmap (needed later)
            SY.dma_start(out=blobs['cvec'][:, :], in_=consts['cvec'][:])
            SY.dma_start(out=x2[:, 0:2048], in_=x_in[:][:, 0:2048])
            SY.dma_start(out=x2[:, 2048:HW], in_=x_in[:][:, 2048:HW])
            SY.dma_start(out=blobs['cw'][:, :], in_=consts['cw'][:])
            SY.dma_start(out=blobs['cmap'][:, :], in_=consts['cmap'][:])
            make_identity(nc, idt32[:, :])
            make_identity(nc, idt128h[:, :])
            _pad_memset(nc, hpad2, 128, PP3, PAD3, W, HT3_COLS)
            _pad_memset(nc, h1pad, 128, PP1, PAD1, W, HT1_COLS)
            xv = x2[:, :].rearrange('p (r c) -> p r c', r=H)

            # bn3 in two row-chunks (pipelines with the x DMA halves);
            # partitions 64:128 hold the same rows shifted up one padded
            # row so the conv can pair taps (ky, ky+1) with K=128
            for half in range(2):
                S.activation(_rows(hpad2[0:64, :],
                                   (PAD3 + 32 * half) * PP3 + PAD3,
                                   PP3, 32, W),
                             xv[0:64, 32 * half:32 * half + 32, :], AF.Relu,
                             bias=C['t3'], scale=C['s3'])
            for half in range(2):
                c0 = (PAD3 - 1 + 32 * half) * PP3
                SY.dma_start(out=hpad2[64:128, c0:c0 + 32 * PP3],
                             in_=hpad2[0:64, c0 + PP3:c0 + 33 * PP3])
            # hT transposes: [64, 128] -> [128, 64] per 128-col chunk;
            # store + quad3 build in two stages so the table is ready early
            hTst = ph1.tile([128, 39, 64], F16, tag='hTst', name='hTst')
            for t0 in range(0, 39, 4):
                nt = min(4, 39 - t0)
                pv = psv.tile([128, 512], F16, tag='pv', name='pv')
                for j in range(nt):
                    PE.transpose(pv[:, 64 * j:64 * j + 64],
                                 hpad2[0:64,
                                       128 * (t0 + j):128 * (t0 + j) + 128],
                                 idt128h[0:64, 0:64])
                if (t0 // 4) % 2 == 1:
                    V.tensor_copy(out=hTst[:, t0:t0 + nt, :],
                                  in_=pv[:, 0:64 * nt])
                else:
                    S.activation(hTst[:, t0:t0 + nt, :],
                                 pv[:, 0:64 * nt]
                                 .rearrange('p (a b) -> p a b', b=64),
                                 AF.Identity)
                if t0 == 16:
                    SY.dma_start(out=hT_dram[0:2560, :]
                                 .rearrange('(a p) c -> p a c', p=128),
                                 in_=hTst[:, 0:20, :])
                    _quad_build(nc, hT_dram, quad3, PP3, 64, 0, 17)
            SY.dma_start(out=hT_dram[2560:4992, :]
                         .rearrange('(a p) c -> p a c', p=128),
                         in_=hTst[:, 20:39, :])
            _quad_build(nc, hT_dram, quad3, PP3, 64, 17, NB)

            # conv3x3: 2 blocks x 4 psum chunks x 6 groups
            # (K=128 tap-pairs (ky=0,1) + K=64 singles (ky=2) per kx)
            for blk in range(2):
                pcs = [psp.tile([128, 512], F32, tag=f'pmm{i}', name=f'pc{i}',
                                bufs=1) for i in range(4)]
                for g in range(6):
                    kx, sub = g // 2, g % 2
                    ky = 0 if sub == 0 else 2
                    npt = 128 if sub == 0 else 64
                    for i in range(4):
                        ch = 4 * blk + i
                        rhs = _rows(hpad2[0:npt, :],
                                    (2 + ky) * PP3 + 2 + kx + 8 * PP3 * ch,
                                    PP3, 8, W)
                        PE.matmul(pcs[i][0:18, :],
                                  C['wC'][0:npt, 18 * g:18 * g + 18],
                                  rhs, start=(g == 0), stop=(g == 5))
                for i in range(4):
                    ch = 4 * blk + i
                    o3 = oev.tile([18, 512], F32, tag='o3', name='o3')
                    S.activation(o3[:, :], pcs[i][0:18, :], AF.Identity,
                                 bias=C['boff3'])
                    pt = psp.tile([128, 128], F32, tag='ptr', name='pt',
                                  bufs=1)
                    for t in range(4):
                        PE.transpose(pt[:, 18 * t:18 * t + 18],
                                     o3[:, 128 * t:128 * t + 128],
                                     idt32[0:18, 0:18])
                    V.tensor_copy(out=off3T[:, 4 * ch:4 * ch + 4, :]
                                  .rearrange('p a b -> p (a b)'),
                                  in_=pt[:, 0:72])

            # coordinates / gather rows -> wrap matmuls / coefficients
            sc = [ph1.tile([128, 288], F32, tag=f'sc{i}', name=f'sc{i}')
                  for i in range(4)]
            rowf3 = ph1.tile([128, 2, 9, 16], F32, tag='rowf3', name='rowf3')
            scv = [s[:, :].rearrange('p (a b) -> p a b', b=9) for s in sc]
            wr3v = wrapped3[:, :, :].rearrange('p k (h c r) -> p h k c r',
                                               h=2, r=8)
            cmv = [C[n].rearrange('p (a b) -> p a b', b=9)
                   for n in ('c03', 'cB3')]
            for hf in range(2):
                cs = slice(16 * hf, 16 * hf + 16)
                _coords_rows(nc,
                             [sv[:, cs, :] for sv in scv],
                             off3T[:, cs, 0:18:2], off3T[:, cs, 1:18:2],
                             cmv[0][:, cs, :], cmv[1][:, cs, :],
                             rowf3[:, hf, :, :].transpose([0, 2, 1]))
                for r in range(8):
                    ptag = ['pmm0', 'pmm1', 'pmm2', 'pmm3'][r % 4]
                    pw = psp.tile([128, 512], F32, tag=ptag, name='pw',
                                  bufs=1)
                    PE.matmul(pw[:, 0:144],
                              C['wrapR'][:, 128 * r:128 * r + 128],
                              rowf3[:, hf, :, :], start=True, stop=True)
                    if r % 2 == 0:
                        S.activation(wr3v[:, hf, :, :, r], pw[:, 0:144],
                                     AF.Identity)
                    else:
                        V.tensor_copy(out=wr3v[:, hf, :, :, r],
                                      in_=pw[:, 0:144])
                _coords_coefs(nc,
                              [sv[:, cs, :] for sv in scv],
                              off3T[:, cs, 0:18:2], off3T[:, cs, 1:18:2],
                              coefD[:, :, cs, :, :].transpose([0, 2, 1, 3, 4]))

            # h1 x-part: relu(bn1(x)) into h1pad interior (ACT, off path)
            S.activation(_rows(h1pad[0:64, :], PAD1 * PP1 + PAD1, PP1, H, W),
                         xv[0:64], AF.Relu, bias=C['t1x'], scale=C['s1x'])
            # h1Tst x-half transposes + x-side table build (DMA idle here)
            for t0 in range(0, 37, 4):
                nt = min(4, 37 - t0)
                pv = psv.tile([128, 512], F16, tag='pv', name='pv')
                for j in range(nt):
                    PE.transpose(pv[:, 64 * j:64 * j + 64],
                                 h1pad[0:64, 128 * (t0 + j):
                                       128 * (t0 + j) + 128],
                                 idt128h[0:64, 0:64])
                S.activation(h1Tst[:, t0:t0 + nt, 0:64],
                             pv[:, 0:64 * nt], AF.Identity)
            SY.dma_start(out=h1Tx[0:4736, :]
                         .rearrange('(a p) c -> p a c', p=128),
                         in_=h1Tst[:, :, 0:64])
            _xpar_build(nc, h1Tx, quad1x, 0, PP1)

        # ---------------- phase C: gathers + in-place combine ------------
        vhgs = {}
        pend_fadd = []
        with tc.tile_pool(name='gpool', bufs=4) as gp, \
             tc.tile_pool(name='vhp', bufs=3) as vhp:
            for pos, k in enumerate([0, 1, 8, 2, 3, 4, 5, 6, 7]):
                gp_i, sl = k // 2, k % 2
                if gp_i not in vhgs:
                    nt = 2 if gp_i < 4 else 1
                    tag = 'vh' if gp_i < 4 else 'vh4'
                    vhgs[gp_i] = vhp.tile([128, NCH, nt, 64], F16, tag=tag,
                                          name=f'vh{gp_i}')
                vhg = vhgs[gp_i]
                g = gp.tile([128, 8192], F16, tag='g', name='g')
                G.dma_gather(g[:, :].rearrange('p (a c) -> p a c', c=256),
                             quad3[:], wrapped3[:, k, :], 4096, 4096,
                             256, queue_num=0, single_packet=False)

                gq = g[:, :].rearrange('p (a b c d) -> p a b c d',
                                       a=NCH, b=4, c=32)
                cf = coefD[:, k][:, :, :, None, :].broadcast_to(
                    [128, NCH, 4, 32, 2])
                gw = g[:, :].rearrange('p (a b c) -> p a b c', a=NCH, b=4)
                nspl = 2 if pos == 8 else 1
                nh = NCH // nspl
                for sp in range(nspl):
                    chs = slice(nh * sp, nh * sp + nh)
                    V.tensor_tensor(out=gq[:, chs], in0=gq[:, chs],
                                    in1=cf[:, chs], op=ALU.mult)
                    with nc.allow_low_precision('fp16 middle precision'):
                        V.tensor_tensor(out=gw[:, chs, 0:2, :],
                                        in0=gw[:, chs, 0:2, :],
                                        in1=gw[:, chs, 2:4, :], op=ALU.add)
                        V.tensor_tensor(out=vhg[:, chs, sl, :],
                                        in0=gw[:, chs, 0, :],
                                        in1=gw[:, chs, 1, :], op=ALU.add)

            # ------------ phase C2/D: transposes + einsum3 ---------------
            sc1 = [pp.tile([128, 32], F32, tag=f't1s{i}', name=f't1s{i}')
                   for i in range(4)]
            rowf = pp.tile([128, 32], F32, tag='rowf', name='rowf')
            rowt = pp.tile([128, 16, 8], F32, tag='rowt', name='rowt')
            vhat1 = pp.tile([128, NCH, 2, 64], F16, tag='vhat1',
                            name='vhat1')
            wr1v = wrapped1[:, :, :].rearrange('p a (c r) -> p a c r', r=8)

            def _coords1_q(q):
                hs = slice(8 * q, 8 * q + 8)
                _coords_rows(nc, [s[:, hs] for s in sc1],
                             off1T[:, hs, 0], off1T[:, hs, 1],
                             C['c01'][:, hs], C['cB1'][:, hs], rowf[:, hs])
                pw1 = psp.tile([128, 512], F32, tag='ptr', name='pw1',
                               bufs=1)
                for r in range(8):
                    PE.matmul(pw1[:, 8 * r:8 * r + 8],
                              C['wrapR'][:, 128 * r:128 * r + 128],
                              rowf[:, hs], start=True, stop=True)
                pwv = pw1[:, 0:64].rearrange('p (r c) -> p c r', c=8)
                V.tensor_copy(out=wr1v[:, 0, hs, :], in_=pwv)
                # second y-corner row sits one padded row (+PP1) below
                V.tensor_scalar(out=rowt[:, 0:8, :], in0=pwv,
                                scalar1=float(PP1), scalar2=None,
                                op0=ALU.add)
                V.tensor_copy(out=wr1v[:, 1, hs, :], in_=rowt[:, 0:8, :])
                _coords_coefs(nc, [s[:, hs] for s in sc1],
                              off1T[:, hs, 0], off1T[:, hs, 1],
                              coef1D[:, :, hs, :, :]
                              .transpose([0, 2, 1, 3, 4]),
                              split_corners=True)

            def _gather1(quadap, c0, nch):
                """Gather chunks [c0, c0+nch) of the 1x1 deform from the
                x-parity table: one gather per y-corner a, each row holding
                the two x-corners for 64 channels."""
                g1 = gp.tile([128, 8192], F16, tag='g', name='g1')
                for a in range(2):
                    G.dma_gather(g1[:, 4096 * a:4096 * a + 128 * nch]
                                 .rearrange('p (c e) -> p c e', e=128),
                                 quadap,
                                 wrapped1[:, a, 8 * c0:8 * (c0 + nch)],
                                 128 * nch, 128 * nch, 128, queue_num=0,
                                 single_packet=False)
                return g1[:, :].rearrange('p (a q) -> p a q', a=2)[
                    :, :, 0:128 * nch] \
                    .rearrange('p a (c b e) -> p a c b e', b=2, e=64)

            def _combine1(g1v, hx, c0, nch):
                for a in range(2):
                    gq = g1v[:, a].rearrange('p c b (f d) -> p c b f d', d=2)
                    cf = coef1D[:, a, c0:c0 + nch, :, :][
                        :, :, :, None, :].broadcast_to([128, nch, 2, 32, 2])
                    V.tensor_tensor(out=gq, in0=gq, in1=cf, op=ALU.mult)
                with nc.allow_low_precision('fp16 by design'):
                    V.tensor_tensor(out=g1v[:, 0], in0=g1v[:, 0],
                                    in1=g1v[:, 1], op=ALU.add)
                    V.tensor_tensor(
                        out=vhat1[:, c0:c0 + nch, hx, :],
                        in0=g1v[:, 0, :, 0, :], in1=g1v[:, 0, :, 1, :],
                        op=ALU.add)

            with tc.tile_pool(name='vpool', bufs=1) as vp:
                v = vp.tile([128, 5, HW], F16, tag='v', name='v')
                pms = {}

                def _mm3(ch, gp_i):
                    if ch not in pms:
                        pms[ch] = psp.tile([128, 512], F32,
                                           tag=f'pmm{ch % 4}',
                                           name=f'pm{ch}', bufs=1)
                    if gp_i < 4:
                        PE.matmul(pms[ch][64:128, :],
                                  C['wd3T'][:, 64 * gp_i:64 * gp_i + 64],
                                  v[:, gp_i, 512 * ch:512 * ch + 512],
                                  start=(gp_i == 0), stop=False)
                    else:
                        PE.matmul(pms[ch][64:128, :],
                                  C['wd3T'][0:64, 256:320],
                                  v[0:64, 4, 512 * ch:512 * ch + 512],
                                  start=False, stop=True)

                def _vtrans(gp_i, ch4):
                    pv = psv.tile([128, 512], F16, tag='pv', name='pv')
                    for sub in range(4):
                        ch = 4 * ch4 + sub
                        if gp_i < 4:
                            PE.transpose(
                                pv[:, 128 * sub:128 * sub + 128],
                                vhgs[gp_i][:, ch, :, :]
                                .rearrange('p a b -> p (a b)'),
                                idt128h[:, :])
                        else:
                            PE.transpose(
                                pv[0:64, 128 * sub:128 * sub + 128],
                                vhgs[4][:, ch, 0, :], idt128h[:, :])
                    np_ = 128 if gp_i < 4 else 64
                    if gp_i == 3 and ch4 % 2 == 1:
                        # last group lands post-combine; DVE is free then
                        V.tensor_copy(out=v[0:np_, gp_i,
                                            512 * ch4:512 * ch4 + 512],
                                      in_=pv[0:np_, :])
                    else:
                        S.activation(v[0:np_, gp_i,
                                       512 * ch4:512 * ch4 + 512],
                                     pv[0:np_, :], AF.Identity)

                for gp_i in (0, 4, 1, 2, 3):
                    for ch4 in range(8):
                        _vtrans(gp_i, ch4)
                    for ch in range(4):
                        _mm3(ch, gp_i)

                # ---- phase E: evacs + off1 + tables + 1x1 gathers -------
                estate = {'tch': 0, 'pt1': None}
                with tc.tile_pool(name='oev1', bufs=2) as oev1:

                    def _evac_chunk(ch):
                        hv = _rows(h1pad[64:128, :],
                                   (8 * ch + PAD1) * PP1 + PAD1, PP1, 8, W)
                        pmv = pms[ch][64:128, :] \
                            .rearrange('p (r c) -> p r c', r=8)
                        S.activation(hv, pmv, AF.Relu, bias=C['t1m'],
                                     scale=C['s1m'])
                        pc1 = psp.tile([128, 512], F32, tag='pc1',
                                       name='pc1', bufs=1)
                        PE.matmul(pc1[0:2, :], C['woff1T'],
                                  _rows(h1pad[0:128, :],
                                        (8 * ch + PAD1) * PP1 + PAD1,
                                        PP1, 8, W),
                                  start=True, stop=True)
                        o1 = oev1.tile([2, 512], F32, tag='o1', name='o1')
                        if ch % 2 == 0:
                            S.activation(o1[:, :], pc1[0:2, :], AF.Identity,
                                         bias=C['boff1'])
                        else:
                            V.tensor_scalar(out=o1[:, :], in0=pc1[0:2, :],
                                            scalar1=C['boff1'], scalar2=None,
                                            op0=ALU.add)
                        if ch % 2 == 0:
                            estate['pt1'] = psp.tile([128, 128], F32,
                                                     tag='ptr', name='pt1',
                                                     bufs=1)
                        pt1 = estate['pt1']
                        for t in range(4):
                            PE.transpose(pt1[:, 8 * (ch % 2) + 2 * t:
                                             8 * (ch % 2) + 2 * t + 2],
                                         o1[:, 128 * t:128 * t + 128],
                                         idt32[0:2, 0:2])
                        if ch % 2 == 1:
                            V.tensor_copy(out=off1T[:, 8 * (ch // 2):
                                                    8 * (ch // 2) + 8, :]
                                          .rearrange('p a b -> p (a b)'),
                                          in_=pt1[:, 0:16])
                        # h1Tst mid-half transposes ready with this chunk
                        r_hi_ready = 2 + 8 * ch + 8
                        ready = []
                        while estate['tch'] < 37:
                            tch = estate['tch']
                            r_hi = (128 * tch + 127) // PP1
                            if r_hi >= r_hi_ready and ch < 7:
                                break
                            ready.append(tch)
                            estate['tch'] += 1
                        for i0 in range(0, len(ready), 4):
                            grp = ready[i0:i0 + 4]
                            pv = psv.tile([128, 512], F16, tag='pv',
                                          name='pv')
                            for j, tch in enumerate(grp):
                                PE.transpose(pv[:, 64 * j:64 * j + 64],
                                             h1pad[64:128,
                                                   128 * tch:128 * tch + 128],
                                             idt128h[64:128, 64:128])
                            t0g = grp[0]
                            estate['evp'] = estate.get('evp', 0) + 1
                            if estate['evp'] % 2 == 1:
                                S.activation(h1Tst[:, t0g:t0g + len(grp),
                                                   64:128],
                                             pv[:, 0:64 * len(grp)]
                                             .rearrange('p (a b) -> p a b',
                                                        b=64),
                                             AF.Identity)
                            else:
                                V.tensor_copy(out=h1Tst[:, t0g:t0g + len(grp),
                                                        64:128],
                                              in_=pv[:, 0:64 * len(grp)])
                        # staged h1Tm stores + x-parity table builds: stage s
                        # covers padded rows [QB[s], QB[s+1]) and needs h1Tst
                        # cols [QC[s], QC[s+1])
                        QB = [0, 19, 35, PP1]
                        QC = [0, 11, 19, 37]
                        st = {2: 0, 4: 1, 7: 2}.get(ch)
                        if st is not None:
                            SY.dma_start(
                                out=h1Tm[128 * QC[st]:128 * QC[st + 1], :]
                                .rearrange('(a p) c -> p a c', p=128),
                                in_=h1Tst[:, QC[st]:QC[st + 1], 64:128])
                            _xpar_build(nc, h1Tm, quad1m, QB[st], QB[st + 1])

                    _evac_chunk(0)
                    _evac_chunk(1)
                    _coords1_q(0)
                    gx0 = _gather1(quad1x[:], 0, 8)
                    _evac_chunk(2)             # store-a + build-a
                    gm0 = _gather1(quad1m[:][0:PP1 * 19, :], 0, 8)
                    _evac_chunk(3)
                    _coords1_q(1)
                    gx1 = _gather1(quad1x[:], 8, 8)
                    for gp_i in (0, 4, 1, 2, 3):
                        for ch in range(4, 8):
                            _mm3(ch, gp_i)
                    _evac_chunk(4)             # store-b + build-b
                    gm1 = _gather1(quad1m[:][0:PP1 * 35, :], 8, 8)
                    _combine1(gx0, 0, 0, 8)
                    _combine1(gm0, 1, 0, 8)
                    _evac_chunk(5)
                    _coords1_q(2)
                    gx2 = _gather1(quad1x[:], 16, 8)
                    _combine1(gx1, 0, 8, 8)
                    _combine1(gm1, 1, 8, 8)
                    _evac_chunk(6)
                    _evac_chunk(7)             # store-c + build-c
                    gm2 = _gather1(quad1m[:], 16, 8)
                    _coords1_q(3)
                    gx3 = _gather1(quad1x[:], 24, 8)
                    _combine1(gx2, 0, 16, 8)
                    _combine1(gm2, 1, 16, 8)
                    gm3 = _gather1(quad1m[:], 24, 8)
                    _combine1(gx3, 0, 24, 8)
                    _combine1(gm3, 1, 24, 8)

            # ---------------- v1 transposes + einsum1 + upsample ---------
            with tc.tile_pool(name='tailp', bufs=1) as tp:
                yd = tp.tile([32, H, 2 * W], F32, tag='yd', name='yd')
                v1s = tp.tile([128, 8, 512], F16, tag='v1s', name='v1s')
                for ch4 in range(8):
                    pv = psv.tile([128, 512], F16, tag='pv', name='pv')
                    for sub in range(4):
                        PE.transpose(pv[:, 128 * sub:128 * sub + 128],
                                     vhat1[:, 4 * ch4 + sub, :, :]
                                     .rearrange('p a b -> p (a b)'),
                                     idt128h[:, :])
                    if ch4 % 2 == 0:
                        S.activation(v1s[:, ch4, :], pv[:, :], AF.Identity)
                    else:
                        V.tensor_copy(out=v1s[:, ch4, :], in_=pv[:, :])
                    pmy = psp.tile([128, 512], F32, tag=f'pmm{ch4 % 4}',
                                   name='pmy', bufs=1)
                    PE.matmul(pmy[0:32, :], C['wd1T'], v1s[:, ch4, :],
                              start=True, stop=True)
                    pmv = pmy[0:32, :].rearrange('p (r c) -> p r c', r=8)
                    S.activation(yd[:, 8 * ch4:8 * ch4 + 8, 0::2], pmv,
                                 AF.Identity, bias=C['bd1'])
                    V.tensor_scalar(out=yd[:, 8 * ch4:8 * ch4 + 8, 1::2],
                                    in0=pmv, scalar1=C['bd1'], scalar2=None,
                                    op0=ALU.add)
                    if ch4 % 2 == 1:
                        gq = ch4 // 2
                        SY.dma_start(out=out_ext[:, 32 * gq:32 * gq + 32:2,
                                                 :],
                                     in_=yd[:, 16 * gq:16 * gq + 16, :])
                        SY.dma_start(out=out_ext[:,
                                                 32 * gq + 1:32 * gq + 32:2,
                                                 :],
                                     in_=yd[:, 16 * gq:16 * gq + 16, :])



# --------------------------------------------------------------------------
# host entry point
# --------------------------------------------------------------------------

_CACHE = {}


def kernel(**inputs):
    x = np.ascontiguousarray(inputs['x'], np.float32)      # [8, 64, 64, 64]
    B = x.shape[0]
    consts = host_constants(inputs)

    if 'nc' not in _CACHE:
        _CACHE['nc'] = build_nc()
    nc = _CACHE['nc']

    packed = pack_constants(consts)
    in_maps = []
    for b in range(B):
        m = {'x': x[b].reshape(64, HW)}
        for name, shape, dt in CONST_SPECS:
            m['c_' + name] = packed[name]
        in_maps.append(m)

    res = run_bass_kernel_spmd(nc, in_maps, list(range(B)))
    out = np.stack([res.results[b]['out'] for b in range(B)])
    return out.astype(np.float32)



# revision 6
# speedup vs baseline: 1.0421x; 1.0091x over previous
"""Trainium2 Bass kernel for nn_DeformableUpsampleBlock (fixed instance).

Quad-parity gather tables + on-PE einsums, tuned against TimelineSim:
  - conv3x3 as 6 K=128/K=64 groups via a row-shifted copy of bn3(x) in
    partitions 64:127 (one SBUF->SBUF DMA builds the shifted half)
  - gather-row indices simplified to row = C0 + 68*sy + sx*CB (by-outer
    table layout cancels the y-parity term); index wrap/replication done
    with 8 permutation matmuls on PE instead of DMA round-trips
  - 3x3 deform: single-gather quad-parity table (512B descriptors),
    gather stream paced by the DVE bilinear combine (~7.6us/tap)
  - 1x1 deform: x-parity-only tables (2-DMA builds, 8.5KB-contiguous
    runs) with two gathers per quarter (one per y-corner row, +68 row
    offset); mid table built in 3 stages keyed to the einsum evacuation
    ladder; coords/gather/combine/einsum1/output quartered and
    interleaved so the output DMAs overlap the gather tail
"""

import numpy as np

import concourse.bass as bass
import concourse.mybir as mybir
from concourse import bacc
import concourse.tile as tile
from concourse.bass_utils import run_bass_kernel_spmd
from concourse.masks import make_identity

F32 = mybir.dt.float32
F16 = mybir.dt.float16
I16 = mybir.dt.int16
AF = mybir.ActivationFunctionType
ALU = mybir.AluOpType

H = W = 64
HW = H * W              # 4096
NCH = 32                # pixel chunks of 128; pixel p -> [p % 128, p // 128]
PAD3 = 3
PP3 = H + 2 * PAD3      # 70
PAD1 = 2
PP1 = H + 2 * PAD1      # 68
NB = 34                 # quad blocks per side (both tables)
NROW = 4 * NB * NB      # 4624
HT3_COLS = 4992         # 39*128 >= 70*70 (+ quad-build overread)
HT1_COLS = 4864         # 38*128; quad build reads to 4761
EPS = 1e-5


# --------------------------------------------------------------------------
# host-side constants
# --------------------------------------------------------------------------

def _f16(a):
    return np.ascontiguousarray(a).astype(np.float16)


def host_constants(p):
    c = {}
    inv3 = (1.0 / np.sqrt(p['bn3_var'].astype(np.float64) + EPS)).astype(np.float32)
    s3 = (p['bn3_gamma'] * inv3).astype(np.float32)
    t3 = (p['bn3_beta'] - p['bn3_mean'] * s3).astype(np.float32)
    c['s3'] = s3.reshape(64, 1).copy()
    c['t3'] = t3.reshape(64, 1).copy()

    inv1 = (1.0 / np.sqrt(p['bn1_var'].astype(np.float64) + EPS)).astype(np.float32)
    s1 = (p['bn1_gamma'] * inv1).astype(np.float32)
    t1 = (p['bn1_beta'] - p['bn1_mean'] * s1).astype(np.float32)
    c['s1x'] = s1[:64].reshape(64, 1).copy()
    c['t1x'] = t1[:64].reshape(64, 1).copy()
    c['s1m'] = s1[64:].reshape(64, 1).copy()
    c['t1m'] = (t1[64:] + s1[64:] * p['b_d3']).reshape(64, 1).astype(np.float32)

    w3 = p['w_off3'].astype(np.float32)          # [18, 64, 3, 3]
    # 6 conv groups: per kx a K=128 pair (ky=0 in parts 0:64, ky=1 in the
    # row-shifted parts 64:128) plus a K=64 single (ky=2)
    wC = np.zeros((128, 162), np.float32)
    for kx in range(3):
        wC[:64, 36 * kx:36 * kx + 18] = w3[:, :, 0, kx].T
        wC[64:, 36 * kx:36 * kx + 18] = w3[:, :, 1, kx].T
        wC[:64, 36 * kx + 18:36 * kx + 36] = w3[:, :, 2, kx].T
    c['wC'] = _f16(wC)
    c['boff3'] = p['b_off3'].astype(np.float32).reshape(18, 1)
    c['boff1'] = p['b_off1'].astype(np.float32).reshape(2, 1)

    wd3 = p['w_d3'].astype(np.float32).reshape(64, 64, 9)    # [o, c, k]
    wt = np.zeros((128, 320), np.float32)
    for g in range(5):
        for part in range(128):
            kap = 128 * g + part
            if kap < 576:
                wt[part, 64 * g:64 * g + 64] = wd3[:, kap % 64, kap // 64]
    c['wd3T'] = _f16(wt)

    c['woff1T'] = _f16(p['w_off1'].reshape(2, 128).T)
    c['wd1T'] = _f16(p['w_d1'].reshape(32, 128).T)
    c['bd1'] = p['b_d1'].astype(np.float32).reshape(32, 1)

    part = np.arange(128)[:, None]
    chunk = np.arange(NCH)[None, :]
    pix = chunk * 128 + part
    ymap = (pix // W).astype(np.float32)
    xmap = (pix % W).astype(np.float32)
    yb3 = np.zeros((128, NCH, 9), np.float32)
    xb3 = np.zeros((128, NCH, 9), np.float32)
    for k in range(9):
        yb3[:, :, k] = ymap + (k // 3 + PAD3 - 2)
        xb3[:, :, k] = xmap + (k % 3 + PAD3 - 2)
    yb1 = ymap + (PAD1 - 1)
    xb1 = xmap + (PAD1 - 1)

    # quad tables are laid out by-outer: row = 136*by + 34*(2*ay+ax) + bx.
    # With Y0 = yb+sy, X0 = xb+sx, ay = Y0%2, ax = X0%2 the ay terms cancel:
    # row = 68*Y0 + 0.5*X0 + 33.5*ax = C0 + 68*sy + sx*CB (exact in f32)
    def _rowconsts(yb, xb):
        pbx = np.mod(xb, 2.0)
        c0 = 68.0 * yb + 0.5 * xb + 33.5 * pbx
        cb = 34.0 - 67.0 * pbx
        return c0.astype(np.float32), cb.astype(np.float32)

    c03, cB3 = _rowconsts(yb3, xb3)
    c['c03'] = c03.reshape(128, 288)
    c['cB3'] = cB3.reshape(128, 288)
    c01, cB1 = _rowconsts(yb1, xb1)
    c['c01'] = c01
    c['cB1'] = cB1
    # wrap matrices: wrapR[p, 128*r + q] = 1 iff p == 16*r + q%16
    wrapR = np.zeros((128, 1024), np.float32)
    for r in range(8):
        for q in range(128):
            wrapR[16 * r + q % 16, 128 * r + q] = 1.0
    c['wrapR'] = wrapR
    return c


_VEC_SPECS = [   # [P<=128, 1] f32 per-partition vectors -> blob 'cvec'
    ('s3', 64), ('t3', 64), ('s1x', 64), ('t1x', 64), ('s1m', 64),
    ('t1m', 64), ('boff3', 18), ('boff1', 2), ('bd1', 32),
]
_MAP_SPECS = [   # [128, N] f32 coordinate maps -> blob 'cmap'
    ('c03', 288), ('cB3', 288),
    ('c01', 32), ('cB1', 32),
    ('wrapR', 1024),
]
_W_SPECS = [     # [128, N] f16 weights -> blob 'cw'
    ('wC', 162), ('wd3T', 320), ('woff1T', 2), ('wd1T', 32),
]
CONST_SPECS = [
    ('cvec', (128, len(_VEC_SPECS)), F32),
    ('cmap', (128, sum(n for _, n in _MAP_SPECS)), F32),
    ('cw', (128, sum(n for _, n in _W_SPECS)), F16),
]


def pack_constants(c):
    cvec = np.zeros((128, len(_VEC_SPECS)), np.float32)
    for i, (n, p) in enumerate(_VEC_SPECS):
        cvec[:p, i] = c[n].reshape(-1)
    cmap = np.concatenate([c[n].reshape(128, sz) for n, sz in _MAP_SPECS], axis=1)
    cw = np.concatenate([c[n].reshape(128, sz) for n, sz in _W_SPECS],
                        axis=1).astype(np.float16)
    return {'cvec': cvec.astype(np.float32), 'cmap': cmap.astype(np.float32),
            'cw': cw}


# --------------------------------------------------------------------------
# AP helpers
# --------------------------------------------------------------------------

def _rows(ap2d, off, rstride, nr, ncols):
    """[P, nr, ncols] view of a [P, N] AP: rows of length ncols, stride rstride."""
    v = ap2d[:, off:off + nr * rstride].rearrange('p (r q) -> p r q', q=rstride)
    return v[:, :, 0:ncols]


# --------------------------------------------------------------------------
# device program
# --------------------------------------------------------------------------

def build_nc():
    nc = bacc.Bacc()
    x_in = nc.declare_dram_parameter('x', [64, HW], F32, isOutput=False)
    consts = {}
    for name, shape, dt in CONST_SPECS:
        consts[name] = nc.declare_dram_parameter('c_' + name, list(shape), dt,
                                                 isOutput=False)
    out_ext = nc.declare_dram_parameter('out', [32, 2 * H, 2 * W], F32,
                                        isOutput=True)

    hT_dram = nc.dram_tensor('hT_dram', [HT3_COLS, 64], F16)
    quad3 = nc.dram_tensor('quad3', [NROW, 256], F16)
    h1Tx = nc.dram_tensor('h1Tx', [HT1_COLS, 64], F16)
    h1Tm = nc.dram_tensor('h1Tm', [HT1_COLS, 64], F16)
    quad1x = nc.dram_tensor('quad1x', [NROW, 128], F16)
    quad1m = nc.dram_tensor('quad1m', [NROW, 128], F16)
    gate = nc.dram_tensor('gate', [1, 16], F16)

    with tile.TileContext(nc) as tc:
        _body(nc, tc, x_in, consts, out_ext, hT_dram, quad3,
              h1Tx, h1Tm, quad1x, quad1m, gate)
    nc.finalize()
    return nc


def _coords_rows(nc, scratch, dyv, dxv, c0v, cBv, row_out, eng=None):
    """row = C0 + 68*sy + sx*CB; sy/sx persist in scratch for _coords_coefs."""
    sy, sx, ta, tb = scratch
    V = eng or nc.vector
    V.tensor_scalar(out=sy, in0=dyv, scalar1=0.0, scalar2=None, op0=ALU.is_ge)
    V.tensor_scalar(out=sx, in0=dxv, scalar1=0.0, scalar2=None, op0=ALU.is_ge)
    V.scalar_tensor_tensor(out=ta, in0=sy, scalar=68.0, in1=c0v,
                           op0=ALU.mult, op1=ALU.add)
    V.tensor_tensor(out=tb, in0=sx, in1=cBv, op=ALU.mult)
    V.tensor_tensor(out=row_out, in0=ta, in1=tb, op=ALU.add)


def _coords_coefs(nc, scratch, dyv, dxv, coef_out, eng=None,
                  split_corners=False):
    """Corner coefficients from dy/dx and the sy/sx left in scratch."""
    sy, sx, fy, fx = scratch
    V = eng or nc.vector
    # fy = dy + 1 - sy; gy = 1 - fy = sy - dy (reuse sy/sx slots for gy/gx)
    V.scalar_tensor_tensor(out=fy, in0=dyv, scalar=1.0, in1=sy,
                           op0=ALU.add, op1=ALU.subtract)
    V.scalar_tensor_tensor(out=fx, in0=dxv, scalar=1.0, in1=sx,
                           op0=ALU.add, op1=ALU.subtract)
    V.tensor_tensor(out=sy, in0=sy, in1=dyv, op=ALU.subtract)
    V.tensor_tensor(out=sx, in0=sx, in1=dxv, op=ALU.subtract)
    nd = coef_out.ndim - (3 if split_corners else 2)
    for i, (a, b) in enumerate([(sy, sx), (sy, fx), (fy, sx), (fy, fx)]):
        for j in range(2):
            idx = (i // 2, i % 2, j) if split_corners else (i, j)
            V.tensor_tensor(out=coef_out[(slice(None),) * nd + idx],
                            in0=a, in1=b, op=ALU.mult)


def _wrap_idx(nc, rowi16_v, wrapped, eng=None):
    """rowi16_v: [128, nk, nch] i16 (contiguous) -> wrapped [128, nk, 256]:
    wrapped[q, k, chunk*8 + r] = row[16r+q, k, chunk], replicated to the 8
    16-partition groups."""
    eng = eng or nc.sync
    for r in range(8):
        eng.dma_start(out=wrapped[0:16, :, r::8],
                      in_=rowi16_v[16 * r:16 * r + 16, :, :])
    for gsz in (16, 32, 64):
        eng.dma_start(out=wrapped[gsz:2 * gsz, :, :],
                      in_=wrapped[0:gsz, :, :])


def _quad_build(nc, src_dram, dst_dram, pp, chans, b0=0, b1=NB,
                parities=None, eng=None):
    """DRAM->DRAM DMAs (3-dim APs) building the quad-parity block table
    (by-outer layout: row = 4*NB*by + NB*(2*ay+ax) + bx) for block rows
    by in [b0, b1); optionally only some (ay, ax) parities."""
    q = 2 * pp
    nb = b1 - b0
    for ay in range(2):
        for ax in range(2):
            if parities is not None and (ay, ax) not in parities:
                continue
            s = ay * 2 + ax
            for yy in range(2):
                r0 = (ay + yy) * pp + ax + b0 * q
                sv = src_dram[:][r0:r0 + nb * q, :] \
                    .rearrange('(by q) c -> by q c', q=q)[:, 0:2 * NB, :] \
                    .rearrange('by (bx xx) c -> by bx (xx c)', xx=2)
                dv = dst_dram[4 * NB * b0:4 * NB * b1,
                              2 * chans * yy:2 * chans * (yy + 1)] \
                    .rearrange('(by sx) e -> by sx e', sx=4 * NB)[
                        :, NB * s:NB * s + NB, :]
                (eng or nc.sync).dma_start(out=dv, in_=sv)


def _xpar_build(nc, src_dram, dst_dram, p0, p1, eng=None):
    """x-parity table for the 1x1 deform: dst row 68*py + 34*ax + px2 holds
    the 128 f16 of padded positions (68*py + 2*px2 + ax, +1). One DMA per ax
    with 8.5KB-contiguous dst runs; builds py in [p0, p1)."""
    sflat = src_dram[:].rearrange('r c -> (r c)')
    for ax in range(2):
        base = 64 * (PP1 * p0 + ax)
        sv = sflat[base:base + (p1 - p0) * 64 * PP1] \
            .rearrange('(py q) -> py q', q=64 * PP1) \
            .rearrange('py (px2 e) -> py px2 e', e=128)[:, 0:NB, :]
        dv = dst_dram[PP1 * p0:PP1 * p1, :] \
            .rearrange('(py sx) e -> py sx e', sx=PP1)[:, NB * ax:
                                                       NB * ax + NB, :]
        (eng or nc.sync).dma_start(out=dv, in_=sv)


def _pad_memset(nc, t, npart, pp, pad, w, ncols):
    """Zero only the padding cells of a padded image tile t [npart, ncols]."""
    head = pad * pp + pad
    nc.gpsimd.memset(t[0:npart, 0:head], 0.0)
    gapw = pp - w
    r0, r1 = pad, pad + w  # gap r covers trail of row r / lead of row r+1
    ngap = r1 - r0 - 1
    gv = _rows(t[0:npart, :], r0 * pp + pad + w, pp, ngap, gapw)
    nc.gpsimd.memset(gv, 0.0)
    tail0 = (r1 - 1) * pp + pad + w
    nc.gpsimd.memset(t[0:npart, tail0:ncols], 0.0)



def _body(nc, tc, x_in, consts, out_ext, hT_dram, quad3,
          h1Tx, h1Tm, quad1x, quad1m, gate):
    V, S, G, PE, SY = nc.vector, nc.scalar, nc.gpsimd, nc.tensor, nc.sync

    with (
        tc.tile_pool(name='persist', bufs=1) as pp,
        tc.tile_pool(name='psum', bufs=2, space='PSUM') as psp,
        tc.tile_pool(name='psumv', bufs=2, space='PSUM') as psv,
    ):
        # ---------------- constants / persistent tiles -------------------
        blobs = {}
        for name, shape, dt in CONST_SPECS:
            t = pp.tile(list(shape), dt, tag='c_' + name, name='c_' + name)
            blobs[name] = t
        C = {}
        for i, (n, p_) in enumerate(_VEC_SPECS):
            C[n] = blobs['cvec'][0:p_, i:i + 1]
        col = 0
        for n, sz in _MAP_SPECS:
            C[n] = blobs['cmap'][:, col:col + sz]
            col += sz
        col = 0
        for n, sz in _W_SPECS:
            C[n] = blobs['cw'][:, col:col + sz]
            col += sz
        idt32 = pp.tile([32, 32], F32, tag='idt32', name='idt32')
        idt128h = pp.tile([128, 128], F16, tag='idt128h', name='idt128h')
        h1pad = pp.tile([128, HT1_COLS], F16, tag='h1pad', name='h1pad')
        coefD = pp.tile([128, 9, NCH, 4, 2], F16, tag='coefD', name='coefD')
        coef1D = pp.tile([128, 2, NCH, 2, 2], F16, tag='coef1D',
                 name='coef1D')
        wrapped3 = pp.tile([128, 9, 256], I16, tag='wrapped3', name='wrapped3')
        wrapped1 = pp.tile([128, 2, 256], I16, tag='wrapped1', name='wrapped1')
        h1Tst = pp.tile([128, 37, 128], F16, tag='h1Tst', name='h1Tst')
        off3T = pp.tile([128, NCH, 18], F32, tag='off3T', name='off3T')
        off1T = pp.tile([128, NCH, 2], F32, tag='off1T', name='off1T')

        # ---------------- phase A: bn3, transposes, conv, coords ---------
        with tc.tile_pool(name='ph1', bufs=1) as ph1, \
             tc.tile_pool(name='oev', bufs=2) as oev:
            x2 = ph1.tile([64, HW], F32, tag='x2', name='x2')
            hpad2 = ph1.tile([128, HT3_COLS], F16, tag='hpad2',
                             name='hpad2')
            # cvec first (gates bn3), then x, then cw/cmap (needed later)
            SY.dma_start(out=blobs['cvec'][:, :], in_=consts['cvec'][:])
            SY.dma_start(out=x2[:, 0:2048], in_=x_in[:][:, 0:2048])
            SY.dma_start(out=x2[:, 2048:HW], in_=x_in[:][:, 2048:HW])
            SY.dma_start(out=blobs['cw'][:, :], in_=consts['cw'][:])
            SY.dma_start(out=blobs['cmap'][:, :], in_=consts['cmap'][:])
            make_identity(nc, idt32[:, :])
            make_identity(nc, idt128h[:, :])
            _pad_memset(nc, hpad2, 128, PP3, PAD3, W, HT3_COLS)
            _pad_memset(nc, h1pad, 128, PP1, PAD1, W, HT1_COLS)
            xv = x2[:, :].rearrange('p (r c) -> p r c', r=H)

            # bn3 in two row-chunks (pipelines with the x DMA halves);
            # partitions 64:128 hold the same rows shifted up one padded
            # row so the conv can pair taps (ky, ky+1) with K=128
            for half in range(2):
                S.activation(_rows(hpad2[0:64, :],
                                   (PAD3 + 32 * half) * PP3 + PAD3,
                                   PP3, 32, W),
                             xv[0:64, 32 * half:32 * half + 32, :], AF.Relu,
                             bias=C['t3'], scale=C['s3'])
            for half in range(2):
                c0 = (PAD3 - 1 + 32 * half) * PP3
                SY.dma_start(out=hpad2[64:128, c0:c0 + 32 * PP3],
                             in_=hpad2[0:64, c0 + PP3:c0 + 33 * PP3])
            # hT transposes: [64, 128] -> [128, 64] per 128-col chunk;
            # store + quad3 build in two stages so the table is ready early
            hTst = ph1.tile([128, 39, 64], F16, tag='hTst', name='hTst')
            for t0 in range(0, 39, 4):
                nt = min(4, 39 - t0)
                pv = psv.tile([128, 512], F16, tag='pv', name='pv')
                for j in range(nt):
                    PE.transpose(pv[:, 64 * j:64 * j + 64],
                                 hpad2[0:64,
                                       128 * (t0 + j):128 * (t0 + j) + 128],
                                 idt128h[0:64, 0:64])
                if (t0 // 4) % 2 == 1:
                    V.tensor_copy(out=hTst[:, t0:t0 + nt, :],
                                  in_=pv[:, 0:64 * nt])
                else:
                    S.activation(hTst[:, t0:t0 + nt, :],
                                 pv[:, 0:64 * nt]
                                 .rearrange('p (a b) -> p a b', b=64),
                                 AF.Identity)
                if t0 == 16:
                    SY.dma_start(out=hT_dram[0:2560, :]
                                 .rearrange('(a p) c -> p a c', p=128),
                                 in_=hTst[:, 0:20, :])
                    _quad_build(nc, hT_dram, quad3, PP3, 64, 0, 17)
            SY.dma_start(out=hT_dram[2560:4992, :]
                         .rearrange('(a p) c -> p a c', p=128),
                         in_=hTst[:, 20:39, :])
            _quad_build(nc, hT_dram, quad3, PP3, 64, 17, NB)

            # conv3x3: 2 blocks x 4 psum chunks x 6 groups
            # (K=128 tap-pairs (ky=0,1) + K=64 singles (ky=2) per kx)
            for blk in range(2):
                pcs = [psp.tile([128, 512], F32, tag=f'pmm{i}', name=f'pc{i}',
                                bufs=1) for i in range(4)]
                for g in range(6):
                    kx, sub = g // 2, g % 2
                    ky = 0 if sub == 0 else 2
                    npt = 128 if sub == 0 else 64
                    for i in range(4):
                        ch = 4 * blk + i
                        rhs = _rows(hpad2[0:npt, :],
                                    (2 + ky) * PP3 + 2 + kx + 8 * PP3 * ch,
                                    PP3, 8, W)
                        PE.matmul(pcs[i][0:18, :],
                                  C['wC'][0:npt, 18 * g:18 * g + 18],
                                  rhs, start=(g == 0), stop=(g == 5))
                for i in range(4):
                    ch = 4 * blk + i
                    o3 = oev.tile([18, 512], F32, tag='o3', name='o3')
                    S.activation(o3[:, :], pcs[i][0:18, :], AF.Identity,
                                 bias=C['boff3'])
                    pt = psp.tile([128, 128], F32, tag='ptr', name='pt',
                                  bufs=1)
                    for t in range(4):
                        PE.transpose(pt[:, 18 * t:18 * t + 18],
                                     o3[:, 128 * t:128 * t + 128],
                                     idt32[0:18, 0:18])
                    V.tensor_copy(out=off3T[:, 4 * ch:4 * ch + 4, :]
                                  .rearrange('p a b -> p (a b)'),
                                  in_=pt[:, 0:72])

            # coordinates / gather rows -> wrap matmuls / coefficients
            sc = [ph1.tile([128, 288], F32, tag=f'sc{i}', name=f'sc{i}')
                  for i in range(4)]
            rowf3 = ph1.tile([128, 2, 9, 16], F32, tag='rowf3', name='rowf3')
            scv = [s[:, :].rearrange('p (a b) -> p a b', b=9) for s in sc]
            wr3v = wrapped3[:, :, :].rearrange('p k (h c r) -> p h k c r',
                                               h=2, r=8)
            cmv = [C[n].rearrange('p (a b) -> p a b', b=9)
                   for n in ('c03', 'cB3')]
            for hf in range(2):
                cs = slice(16 * hf, 16 * hf + 16)
                _coords_rows(nc,
                             [sv[:, cs, :] for sv in scv],
                             off3T[:, cs, 0:18:2], off3T[:, cs, 1:18:2],
                             cmv[0][:, cs, :], cmv[1][:, cs, :],
                             rowf3[:, hf, :, :].transpose([0, 2, 1]))
                for r in range(8):
                    ptag = ['pmm0', 'pmm1', 'pmm2', 'pmm3'][r % 4]
                    pw = psp.tile([128, 512], F32, tag=ptag, name='pw',
                                  bufs=1)
                    PE.matmul(pw[:, 0:144],
                              C['wrapR'][:, 128 * r:128 * r + 128],
                              rowf3[:, hf, :, :], start=True, stop=True)
                    if r % 2 == 0:
                        S.activation(wr3v[:, hf, :, :, r], pw[:, 0:144],
                                     AF.Identity)
                    else:
                        V.tensor_copy(out=wr3v[:, hf, :, :, r],
                                      in_=pw[:, 0:144])
                _coords_coefs(nc,
                              [sv[:, cs, :] for sv in scv],
                              off3T[:, cs, 0:18:2], off3T[:, cs, 1:18:2],
                              coefD[:, :, cs, :, :].transpose([0, 2, 1, 3, 4]))

            # h1 x-part: relu(bn1(x)) into h1pad interior (ACT, off path)
            S.activation(_rows(h1pad[0:64, :], PAD1 * PP1 + PAD1, PP1, H, W),
                         xv[0:64], AF.Relu, bias=C['t1x'], scale=C['s1x'])
            # h1Tst x-half transposes + x-side table build (DMA idle here)
            for t0 in range(0, 37, 4):
                nt = min(4, 37 - t0)
                pv = psv.tile([128, 512], F16, tag='pv', name='pv')
                for j in range(nt):
                    PE.transpose(pv[:, 64 * j:64 * j + 64],
                                 h1pad[0:64, 128 * (t0 + j):
                                       128 * (t0 + j) + 128],
                                 idt128h[0:64, 0:64])
                S.activation(h1Tst[:, t0:t0 + nt, 0:64],
                             pv[:, 0:64 * nt], AF.Identity)

        # ---------------- phase C: gathers + in-place combine ------------
        vhgs = {}
        pend_fadd = []
        with tc.tile_pool(name='gpool', bufs=4) as gp, \
             tc.tile_pool(name='vhp', bufs=3) as vhp:
            for pos, k in enumerate([0, 1, 8, 2, 3, 4, 5, 6, 7]):
                gp_i, sl = k // 2, k % 2
                if gp_i not in vhgs:
                    nt = 2 if gp_i < 4 else 1
                    tag = 'vh' if gp_i < 4 else 'vh4'
                    vhgs[gp_i] = vhp.tile([128, NCH, nt, 64], F16, tag=tag,
                                          name=f'vh{gp_i}')
                vhg = vhgs[gp_i]
                g = gp.tile([128, 8192], F16, tag='g', name='g')
                G.dma_gather(g[:, :].rearrange('p (a c) -> p a c', c=256),
                             quad3[:], wrapped3[:, k, :], 4096, 4096,
                             256, queue_num=0, single_packet=False)
                if pos == 6:
                    # x-side table: built late in phase C where the gather
                    # stream has accumulated DVE-pace slack; the dummy
                    # rewrite gates the store on tap 4's combine so the
                    # scheduler can't hoist these DMAs into the ramp
                    V.scalar_tensor_tensor(out=h1Tst[0:1, 0, 0:16],
                                           in0=vhgs[2][0:1, 0, 0, 0:16],
                                           scalar=0.0,
                                           in1=h1Tst[0:1, 0, 0:16],
                                           op0=ALU.mult, op1=ALU.add)
                    SY.dma_start(out=h1Tx[0:4736, :]
                                 .rearrange('(a p) c -> p a c', p=128),
                                 in_=h1Tst[:, :, 0:64])
                    _xpar_build(nc, h1Tx, quad1x, 0, PP1)

                gq = g[:, :].rearrange('p (a b c d) -> p a b c d',
                                       a=NCH, b=4, c=32)
                cf = coefD[:, k][:, :, :, None, :].broadcast_to(
                    [128, NCH, 4, 32, 2])
                gw = g[:, :].rearrange('p (a b c) -> p a b c', a=NCH, b=4)
                nspl = 2 if pos == 8 else 1
                nh = NCH // nspl
                for sp in range(nspl):
                    chs = slice(nh * sp, nh * sp + nh)
                    V.tensor_tensor(out=gq[:, chs], in0=gq[:, chs],
                                    in1=cf[:, chs], op=ALU.mult)
                    with nc.allow_low_precision('fp16 middle precision'):
                        V.tensor_tensor(out=gw[:, chs, 0:2, :],
                                        in0=gw[:, chs, 0:2, :],
                                        in1=gw[:, chs, 2:4, :], op=ALU.add)
                        V.tensor_tensor(out=vhg[:, chs, sl, :],
                                        in0=gw[:, chs, 0, :],
                                        in1=gw[:, chs, 1, :], op=ALU.add)

            # ------------ phase C2/D: transposes + einsum3 ---------------
            sc1 = [pp.tile([128, 32], F32, tag=f't1s{i}', name=f't1s{i}')
                   for i in range(4)]
            rowf = pp.tile([128, 32], F32, tag='rowf', name='rowf')
            rowt = pp.tile([128, 16, 8], F32, tag='rowt', name='rowt')
            vhat1 = pp.tile([128, NCH, 2, 64], F16, tag='vhat1',
                            name='vhat1')
            wr1v = wrapped1[:, :, :].rearrange('p a (c r) -> p a c r', r=8)

            def _coords1_q(q):
                hs = slice(8 * q, 8 * q + 8)
                _coords_rows(nc, [s[:, hs] for s in sc1],
                             off1T[:, hs, 0], off1T[:, hs, 1],
                             C['c01'][:, hs], C['cB1'][:, hs], rowf[:, hs])
                pw1 = psp.tile([128, 512], F32, tag='ptr', name='pw1',
                               bufs=1)
                for r in range(8):
                    PE.matmul(pw1[:, 8 * r:8 * r + 8],
                              C['wrapR'][:, 128 * r:128 * r + 128],
                              rowf[:, hs], start=True, stop=True)
                pwv = pw1[:, 0:64].rearrange('p (r c) -> p c r', c=8)
                V.tensor_copy(out=wr1v[:, 0, hs, :], in_=pwv)
                # second y-corner row sits one padded row (+PP1) below
                V.tensor_scalar(out=rowt[:, 0:8, :], in0=pwv,
                                scalar1=float(PP1), scalar2=None,
                                op0=ALU.add)
                V.tensor_copy(out=wr1v[:, 1, hs, :], in_=rowt[:, 0:8, :])
                _coords_coefs(nc, [s[:, hs] for s in sc1],
                              off1T[:, hs, 0], off1T[:, hs, 1],
                              coef1D[:, :, hs, :, :]
                              .transpose([0, 2, 1, 3, 4]),
                              split_corners=True)

            def _gather1(quadap, c0, nch):
                """Gather chunks [c0, c0+nch) of the 1x1 deform from the
                x-parity table: one gather per y-corner a, each row holding
                the two x-corners for 64 channels."""
                g1 = gp.tile([128, 8192], F16, tag='g', name='g1')
                for a in range(2):
                    G.dma_gather(g1[:, 4096 * a:4096 * a + 128 * nch]
                                 .rearrange('p (c e) -> p c e', e=128),
                                 quadap,
                                 wrapped1[:, a, 8 * c0:8 * (c0 + nch)],
                                 128 * nch, 128 * nch, 128, queue_num=0,
                                 single_packet=False)
                return g1[:, :].rearrange('p (a q) -> p a q', a=2)[
                    :, :, 0:128 * nch] \
                    .rearrange('p a (c b e) -> p a c b e', b=2, e=64)

            def _combine1(g1v, hx, c0, nch):
                for a in range(2):
                    gq = g1v[:, a].rearrange('p c b (f d) -> p c b f d', d=2)
                    cf = coef1D[:, a, c0:c0 + nch, :, :][
                        :, :, :, None, :].broadcast_to([128, nch, 2, 32, 2])
                    V.tensor_tensor(out=gq, in0=gq, in1=cf, op=ALU.mult)
                with nc.allow_low_precision('fp16 by design'):
                    V.tensor_tensor(out=g1v[:, 0], in0=g1v[:, 0],
                                    in1=g1v[:, 1], op=ALU.add)
                    V.tensor_tensor(
                        out=vhat1[:, c0:c0 + nch, hx, :],
                        in0=g1v[:, 0, :, 0, :], in1=g1v[:, 0, :, 1, :],
                        op=ALU.add)

            with tc.tile_pool(name='vpool', bufs=1) as vp:
                v = vp.tile([128, 5, HW], F16, tag='v', name='v')
                pms = {}

                def _mm3(ch, gp_i):
                    if ch not in pms:
                        pms[ch] = psp.tile([128, 512], F32,
                                           tag=f'pmm{ch % 4}',
                                           name=f'pm{ch}', bufs=1)
                    if gp_i < 4:
                        PE.matmul(pms[ch][64:128, :],
                                  C['wd3T'][:, 64 * gp_i:64 * gp_i + 64],
                                  v[:, gp_i, 512 * ch:512 * ch + 512],
                                  start=(gp_i == 0), stop=False)
                    else:
                        PE.matmul(pms[ch][64:128, :],
                                  C['wd3T'][0:64, 256:320],
                                  v[0:64, 4, 512 * ch:512 * ch + 512],
                                  start=False, stop=True)

                def _vtrans(gp_i, ch4):
                    pv = psv.tile([128, 512], F16, tag='pv', name='pv')
                    for sub in range(4):
                        ch = 4 * ch4 + sub
                        if gp_i < 4:
                            PE.transpose(
                                pv[:, 128 * sub:128 * sub + 128],
                                vhgs[gp_i][:, ch, :, :]
                                .rearrange('p a b -> p (a b)'),
                                idt128h[:, :])
                        else:
                            PE.transpose(
                                pv[0:64, 128 * sub:128 * sub + 128],
                                vhgs[4][:, ch, 0, :], idt128h[:, :])
                    np_ = 128 if gp_i < 4 else 64
                    if gp_i == 3 and ch4 % 2 == 1:
                        # last group lands post-combine; DVE is free then
                        V.tensor_copy(out=v[0:np_, gp_i,
                                            512 * ch4:512 * ch4 + 512],
                                      in_=pv[0:np_, :])
                    else:
                        S.activation(v[0:np_, gp_i,
                                       512 * ch4:512 * ch4 + 512],
                                     pv[0:np_, :], AF.Identity)

                for gp_i in (0, 4, 1, 2, 3):
                    for ch4 in range(8):
                        _vtrans(gp_i, ch4)
                    for ch in range(4):
                        _mm3(ch, gp_i)

                # ---- phase E: evacs + off1 + tables + 1x1 gathers -------
                estate = {'tch': 0, 'pt1': None}
                with tc.tile_pool(name='oev1', bufs=2) as oev1:

                    def _evac_chunk(ch):
                        hv = _rows(h1pad[64:128, :],
                                   (8 * ch + PAD1) * PP1 + PAD1, PP1, 8, W)
                        pmv = pms[ch][64:128, :] \
                            .rearrange('p (r c) -> p r c', r=8)
                        S.activation(hv, pmv, AF.Relu, bias=C['t1m'],
                                     scale=C['s1m'])
                        pc1 = psp.tile([128, 512], F32, tag='pc1',
                                       name='pc1', bufs=1)
                        PE.matmul(pc1[0:2, :], C['woff1T'],
                                  _rows(h1pad[0:128, :],
                                        (8 * ch + PAD1) * PP1 + PAD1,
                                        PP1, 8, W),
                                  start=True, stop=True)
                        o1 = oev1.tile([2, 512], F32, tag='o1', name='o1')
                        if ch % 2 == 0:
                            S.activation(o1[:, :], pc1[0:2, :], AF.Identity,
                                         bias=C['boff1'])
                        else:
                            V.tensor_scalar(out=o1[:, :], in0=pc1[0:2, :],
                                            scalar1=C['boff1'], scalar2=None,
                                            op0=ALU.add)
                        if ch % 2 == 0:
                            estate['pt1'] = psp.tile([128, 128], F32,
                                                     tag='ptr', name='pt1',
                                                     bufs=1)
                        pt1 = estate['pt1']
                        for t in range(4):
                            PE.transpose(pt1[:, 8 * (ch % 2) + 2 * t:
                                             8 * (ch % 2) + 2 * t + 2],
                                         o1[:, 128 * t:128 * t + 128],
                                         idt32[0:2, 0:2])
                        if ch % 2 == 1:
                            V.tensor_copy(out=off1T[:, 8 * (ch // 2):
                                                    8 * (ch // 2) + 8, :]
                                          .rearrange('p a b -> p (a b)'),
                                          in_=pt1[:, 0:16])
                        # h1Tst mid-half transposes ready with this chunk
                        r_hi_ready = 2 + 8 * ch + 8
                        ready = []
                        while estate['tch'] < 37:
                            tch = estate['tch']
                            r_hi = (128 * tch + 127) // PP1
                            if r_hi >= r_hi_ready and ch < 7:
                                break
                            ready.append(tch)
                            estate['tch'] += 1
                        for i0 in range(0, len(ready), 4):
                            grp = ready[i0:i0 + 4]
                            pv = psv.tile([128, 512], F16, tag='pv',
                                          name='pv')
                            for j, tch in enumerate(grp):
                                PE.transpose(pv[:, 64 * j:64 * j + 64],
                                             h1pad[64:128,
                                                   128 * tch:128 * tch + 128],
                                             idt128h[64:128, 64:128])
                            t0g = grp[0]
                            estate['evp'] = estate.get('evp', 0) + 1
                            if estate['evp'] % 2 == 1:
                                S.activation(h1Tst[:, t0g:t0g + len(grp),
                                                   64:128],
                                             pv[:, 0:64 * len(grp)]
                                             .rearrange('p (a b) -> p a b',
                                                        b=64),
                                             AF.Identity)
                            else:
                                V.tensor_copy(out=h1Tst[:, t0g:t0g + len(grp),
                                                        64:128],
                                              in_=pv[:, 0:64 * len(grp)])
                        # staged h1Tm stores + x-parity table builds: stage s
                        # covers padded rows [QB[s], QB[s+1]) and needs h1Tst
                        # cols [QC[s], QC[s+1])
                        QB = [0, 19, 35, PP1]
                        QC = [0, 11, 19, 37]
                        st = {2: 0, 4: 1, 7: 2}.get(ch)
                        if st is not None:
                            SY.dma_start(
                                out=h1Tm[128 * QC[st]:128 * QC[st + 1], :]
                                .rearrange('(a p) c -> p a c', p=128),
                                in_=h1Tst[:, QC[st]:QC[st + 1], 64:128])
                            _xpar_build(nc, h1Tm, quad1m, QB[st], QB[st + 1])

                    _evac_chunk(0)
                    _evac_chunk(1)
                    _coords1_q(0)
                    gx0 = _gather1(quad1x[:], 0, 8)
                    _evac_chunk(2)             # store-a + build-a
                    gm0 = _gather1(quad1m[:][0:PP1 * 19, :], 0, 8)
                    _evac_chunk(3)
                    _coords1_q(1)
                    gx1 = _gather1(quad1x[:], 8, 8)
                    for gp_i in (0, 4, 1, 2, 3):
                        for ch in range(4, 8):
                            _mm3(ch, gp_i)
                    _evac_chunk(4)             # store-b + build-b
                    gm1 = _gather1(quad1m[:][0:PP1 * 35, :], 8, 8)
                    _combine1(gx0, 0, 0, 8)
                    _combine1(gm0, 1, 0, 8)
                    _evac_chunk(5)
                    _coords1_q(2)
                    gx2 = _gather1(quad1x[:], 16, 8)
                    _combine1(gx1, 0, 8, 8)
                    _combine1(gm1, 1, 8, 8)
                    _evac_chunk(6)
                    _evac_chunk(7)             # store-c + build-c
                    gm2 = _gather1(quad1m[:], 16, 8)
                    _coords1_q(3)
                    gx3 = _gather1(quad1x[:], 24, 8)
                    _combine1(gx2, 0, 16, 8)
                    _combine1(gm2, 1, 16, 8)
                    gm3 = _gather1(quad1m[:], 24, 8)
                    _combine1(gx3, 0, 24, 8)
                    _combine1(gm3, 1, 24, 8)

            # ---------------- v1 transposes + einsum1 + upsample ---------
            with tc.tile_pool(name='tailp', bufs=1) as tp:
                yd = tp.tile([32, H, 2 * W], F32, tag='yd', name='yd')
                v1s = tp.tile([128, 8, 512], F16, tag='v1s', name='v1s')
                for ch4 in range(8):
                    pv = psv.tile([128, 512], F16, tag='pv', name='pv')
                    for sub in range(4):
                        PE.transpose(pv[:, 128 * sub:128 * sub + 128],
                                     vhat1[:, 4 * ch4 + sub, :, :]
                                     .rearrange('p a b -> p (a b)'),
                                     idt128h[:, :])
                    if ch4 % 2 == 0:
                        S.activation(v1s[:, ch4, :], pv[:, :], AF.Identity)
                    else:
                        V.tensor_copy(out=v1s[:, ch4, :], in_=pv[:, :])
                    pmy = psp.tile([128, 512], F32, tag=f'pmm{ch4 % 4}',
                                   name='pmy', bufs=1)
                    PE.matmul(pmy[0:32, :], C['wd1T'], v1s[:, ch4, :],
                              start=True, stop=True)
                    pmv = pmy[0:32, :].rearrange('p (r c) -> p r c', r=8)
                    S.activation(yd[:, 8 * ch4:8 * ch4 + 8, 0::2], pmv,
                                 AF.Identity, bias=C['bd1'])
                    V.tensor_scalar(out=yd[:, 8 * ch4:8 * ch4 + 8, 1::2],
                                    in0=pmv, scalar1=C['bd1'], scalar2=None,
                                    op0=ALU.add)
                    if ch4 % 2 == 1:
                        gq = ch4 // 2
                        SY.dma_start(out=out_ext[:, 32 * gq:32 * gq + 32:2,
                                                 :],
                                     in_=yd[:, 16 * gq:16 * gq + 16, :])
                        SY.dma_start(out=out_ext[:,
                                                 32 * gq + 1:32 * gq + 32:2,
                                                 :],
                                     in_=yd[:, 16 * gq:16 * gq + 16, :])



# --------------------------------------------------------------------------
# host entry point
# --------------------------------------------------------------------------

_CACHE = {}


def kernel(**inputs):
    x = np.ascontiguousarray(inputs['x'], np.float32)      # [8, 64, 64, 64]
    B = x.shape[0]
    consts = host_constants(inputs)

    if 'nc' not in _CACHE:
        _CACHE['nc'] = build_nc()
    nc = _CACHE['nc']

    packed = pack_constants(consts)
    in_maps = []
    for b in range(B):
        m = {'x': x[b].reshape(64, HW)}
        for name, shape, dt in CONST_SPECS:
            m['c_' + name] = packed[name]
        in_maps.append(m)

    res = run_bass_kernel_spmd(nc, in_maps, list(range(B)))
    out = np.stack([res.results[b]['out'] for b in range(B)])
    return out.astype(np.float32)



# revision 8
# speedup vs baseline: 1.0592x; 1.0164x over previous
"""Trainium2 Bass kernel for nn_DeformableUpsampleBlock (fixed instance).

Quad-parity gather tables + on-PE einsums, tuned against TimelineSim:
  - conv3x3 as 6 K=128/K=64 groups via a row-shifted copy of bn3(x) in
    partitions 64:127 (one SBUF->SBUF DMA builds the shifted half)
  - gather-row indices simplified to row = C0 + 68*sy + sx*CB (by-outer
    table layout cancels the y-parity term); index wrap/replication done
    with 8 permutation matmuls on PE instead of DMA round-trips
  - 3x3 deform: single-gather quad-parity table (512B descriptors),
    gather stream paced by the DVE bilinear combine (~7.6us/tap)
  - 1x1 deform: x-parity-only tables (2-DMA builds, 8.5KB-contiguous
    runs) with two gathers per quarter (one per y-corner row, +68 row
    offset); mid table built in 3 stages keyed to the einsum evacuation
    ladder; coords/gather/combine/einsum1/output quartered and
    interleaved so the output DMAs overlap the gather tail
"""

import numpy as np

import concourse.bass as bass
import concourse.mybir as mybir
from concourse import bacc
import concourse.tile as tile
from concourse.bass_utils import run_bass_kernel_spmd
from concourse.masks import make_identity

F32 = mybir.dt.float32
F16 = mybir.dt.float16
I16 = mybir.dt.int16
AF = mybir.ActivationFunctionType
ALU = mybir.AluOpType

H = W = 64
HW = H * W              # 4096
NCH = 32                # pixel chunks of 128; pixel p -> [p % 128, p // 128]
PAD3 = 3
PP3 = H + 2 * PAD3      # 70
PAD1 = 2
PP1 = H + 2 * PAD1      # 68
NB = 34                 # quad blocks per side (both tables)
NROW = 4 * NB * NB      # 4624
HT3_COLS = 4992         # 39*128 >= 70*70 (+ quad-build overread)
HT1_COLS = 4864         # 38*128; quad build reads to 4761
EPS = 1e-5


# --------------------------------------------------------------------------
# host-side constants
# --------------------------------------------------------------------------

def _f16(a):
    return np.ascontiguousarray(a).astype(np.float16)


def host_constants(p):
    c = {}
    inv3 = (1.0 / np.sqrt(p['bn3_var'].astype(np.float64) + EPS)).astype(np.float32)
    s3 = (p['bn3_gamma'] * inv3).astype(np.float32)
    t3 = (p['bn3_beta'] - p['bn3_mean'] * s3).astype(np.float32)
    c['s3'] = s3.reshape(64, 1).copy()
    c['t3'] = t3.reshape(64, 1).copy()

    inv1 = (1.0 / np.sqrt(p['bn1_var'].astype(np.float64) + EPS)).astype(np.float32)
    s1 = (p['bn1_gamma'] * inv1).astype(np.float32)
    t1 = (p['bn1_beta'] - p['bn1_mean'] * s1).astype(np.float32)
    c['s1x'] = s1[:64].reshape(64, 1).copy()
    c['t1x'] = t1[:64].reshape(64, 1).copy()
    c['s1m'] = s1[64:].reshape(64, 1).copy()
    c['t1m'] = (t1[64:] + s1[64:] * p['b_d3']).reshape(64, 1).astype(np.float32)

    w3 = p['w_off3'].astype(np.float32)          # [18, 64, 3, 3]
    # 6 conv groups: per kx a K=128 pair (ky=0 in parts 0:64, ky=1 in the
    # row-shifted parts 64:128) plus a K=64 single (ky=2)
    wC = np.zeros((128, 162), np.float32)
    for kx in range(3):
        wC[:64, 36 * kx:36 * kx + 18] = w3[:, :, 0, kx].T
        wC[64:, 36 * kx:36 * kx + 18] = w3[:, :, 1, kx].T
        wC[:64, 36 * kx + 18:36 * kx + 36] = w3[:, :, 2, kx].T
    c['wC'] = _f16(wC)
    c['boff3'] = p['b_off3'].astype(np.float32).reshape(18, 1)
    c['boff1'] = p['b_off1'].astype(np.float32).reshape(2, 1)

    wd3 = p['w_d3'].astype(np.float32).reshape(64, 64, 9)    # [o, c, k]
    wt = np.zeros((128, 320), np.float32)
    for g in range(5):
        for part in range(128):
            kap = 128 * g + part
            if kap < 576:
                wt[part, 64 * g:64 * g + 64] = wd3[:, kap % 64, kap // 64]
    c['wd3T'] = _f16(wt)

    c['woff1T'] = _f16(p['w_off1'].reshape(2, 128).T)
    c['wd1T'] = _f16(p['w_d1'].reshape(32, 128).T)
    c['bd1'] = p['b_d1'].astype(np.float32).reshape(32, 1)

    part = np.arange(128)[:, None]
    chunk = np.arange(NCH)[None, :]
    pix = chunk * 128 + part
    ymap = (pix // W).astype(np.float32)
    xmap = (pix % W).astype(np.float32)
    yb3 = np.zeros((128, NCH, 9), np.float32)
    xb3 = np.zeros((128, NCH, 9), np.float32)
    for k in range(9):
        yb3[:, :, k] = ymap + (k // 3 + PAD3 - 2)
        xb3[:, :, k] = xmap + (k % 3 + PAD3 - 2)
    yb1 = ymap + (PAD1 - 1)
    xb1 = xmap + (PAD1 - 1)

    # quad tables are laid out by-outer: row = 136*by + 34*(2*ay+ax) + bx.
    # With Y0 = yb+sy, X0 = xb+sx, ay = Y0%2, ax = X0%2 the ay terms cancel:
    # row = 68*Y0 + 0.5*X0 + 33.5*ax = C0 + 68*sy + sx*CB (exact in f32)
    def _rowconsts(yb, xb):
        pbx = np.mod(xb, 2.0)
        c0 = 68.0 * yb + 0.5 * xb + 33.5 * pbx
        cb = 34.0 - 67.0 * pbx
        return c0.astype(np.float32), cb.astype(np.float32)

    c03, cB3 = _rowconsts(yb3, xb3)
    c['c03'] = c03.reshape(128, 288)
    c['cB3'] = cB3.reshape(128, 288)
    c01, cB1 = _rowconsts(yb1, xb1)
    c['c01'] = c01
    c['cB1'] = cB1
    # wrap matrices: wrapR[p, 128*r + q] = 1 iff p == 16*r + q%16
    wrapR = np.zeros((128, 1024), np.float32)
    for r in range(8):
        for q in range(128):
            wrapR[16 * r + q % 16, 128 * r + q] = 1.0
    c['wrapR'] = wrapR
    return c


_VEC_SPECS = [   # [P<=128, 1] f32 per-partition vectors -> blob 'cvec'
    ('s3', 64), ('t3', 64), ('s1x', 64), ('t1x', 64), ('s1m', 64),
    ('t1m', 64), ('boff3', 18), ('boff1', 2), ('bd1', 32),
]
_MAP_SPECS = [   # [128, N] f32 coordinate maps -> blob 'cmap'
    ('c03', 288), ('cB3', 288),
    ('c01', 32), ('cB1', 32),
    ('wrapR', 1024),
]
_W_SPECS = [     # [128, N] f16 weights -> blob 'cw'
    ('wC', 162), ('wd3T', 320), ('woff1T', 2), ('wd1T', 32),
]
CONST_SPECS = [
    ('cvec', (128, len(_VEC_SPECS)), F32),
    ('cmap', (128, sum(n for _, n in _MAP_SPECS)), F32),
    ('cw', (128, sum(n for _, n in _W_SPECS)), F16),
]


def pack_constants(c):
    cvec = np.zeros((128, len(_VEC_SPECS)), np.float32)
    for i, (n, p) in enumerate(_VEC_SPECS):
        cvec[:p, i] = c[n].reshape(-1)
    cmap = np.concatenate([c[n].reshape(128, sz) for n, sz in _MAP_SPECS], axis=1)
    cw = np.concatenate([c[n].reshape(128, sz) for n, sz in _W_SPECS],
                        axis=1).astype(np.float16)
    return {'cvec': cvec.astype(np.float32), 'cmap': cmap.astype(np.float32),
            'cw': cw}


# --------------------------------------------------------------------------
# AP helpers
# --------------------------------------------------------------------------

def _rows(ap2d, off, rstride, nr, ncols):
    """[P, nr, ncols] view of a [P, N] AP: rows of length ncols, stride rstride."""
    v = ap2d[:, off:off + nr * rstride].rearrange('p (r q) -> p r q', q=rstride)
    return v[:, :, 0:ncols]


# --------------------------------------------------------------------------
# device program
# --------------------------------------------------------------------------

def build_nc():
    nc = bacc.Bacc()
    x_in = nc.declare_dram_parameter('x', [64, HW], F32, isOutput=False)
    consts = {}
    for name, shape, dt in CONST_SPECS:
        consts[name] = nc.declare_dram_parameter('c_' + name, list(shape), dt,
                                                 isOutput=False)
    out_ext = nc.declare_dram_parameter('out', [32, 2 * H, 2 * W], F32,
                                        isOutput=True)

    hT_dram = nc.dram_tensor('hT_dram', [HT3_COLS, 64], F16)
    quad3 = nc.dram_tensor('quad3', [NROW, 256], F16)
    h1Tx = nc.dram_tensor('h1Tx', [HT1_COLS, 64], F16)
    h1Tm = nc.dram_tensor('h1Tm', [HT1_COLS, 64], F16)
    quad1x = nc.dram_tensor('quad1x', [NROW, 128], F16)
    quad1m = nc.dram_tensor('quad1m', [NROW, 128], F16)
    gate = nc.dram_tensor('gate', [1, 16], F16)

    with tile.TileContext(nc) as tc:
        _body(nc, tc, x_in, consts, out_ext, hT_dram, quad3,
              h1Tx, h1Tm, quad1x, quad1m, gate)
    nc.finalize()
    return nc


def _coords_rows(nc, scratch, dyv, dxv, c0v, cBv, row_out, eng=None):
    """row = C0 + 68*sy + sx*CB; sy/sx persist in scratch for _coords_coefs."""
    sy, sx, ta, tb = scratch
    V = eng or nc.vector
    V.tensor_scalar(out=sy, in0=dyv, scalar1=0.0, scalar2=None, op0=ALU.is_ge)
    V.tensor_scalar(out=sx, in0=dxv, scalar1=0.0, scalar2=None, op0=ALU.is_ge)
    V.scalar_tensor_tensor(out=ta, in0=sy, scalar=68.0, in1=c0v,
                           op0=ALU.mult, op1=ALU.add)
    V.tensor_tensor(out=tb, in0=sx, in1=cBv, op=ALU.mult)
    V.tensor_tensor(out=row_out, in0=ta, in1=tb, op=ALU.add)


def _coords_coefs(nc, scratch, dyv, dxv, coef_out, eng=None,
                  split_corners=False):
    """Corner coefficients from dy/dx and the sy/sx left in scratch."""
    sy, sx, fy, fx = scratch
    V = eng or nc.vector
    # fy = dy + 1 - sy; gy = 1 - fy = sy - dy (reuse sy/sx slots for gy/gx)
    V.scalar_tensor_tensor(out=fy, in0=dyv, scalar=1.0, in1=sy,
                           op0=ALU.add, op1=ALU.subtract)
    V.scalar_tensor_tensor(out=fx, in0=dxv, scalar=1.0, in1=sx,
                           op0=ALU.add, op1=ALU.subtract)
    V.tensor_tensor(out=sy, in0=sy, in1=dyv, op=ALU.subtract)
    V.tensor_tensor(out=sx, in0=sx, in1=dxv, op=ALU.subtract)
    nd = coef_out.ndim - (3 if split_corners else 2)
    for i, (a, b) in enumerate([(sy, sx), (sy, fx), (fy, sx), (fy, fx)]):
        for j in range(2):
            idx = (i // 2, i % 2, j) if split_corners else (i, j)
            V.tensor_tensor(out=coef_out[(slice(None),) * nd + idx],
                            in0=a, in1=b, op=ALU.mult)


def _wrap_idx(nc, rowi16_v, wrapped, eng=None):
    """rowi16_v: [128, nk, nch] i16 (contiguous) -> wrapped [128, nk, 256]:
    wrapped[q, k, chunk*8 + r] = row[16r+q, k, chunk], replicated to the 8
    16-partition groups."""
    eng = eng or nc.sync
    for r in range(8):
        eng.dma_start(out=wrapped[0:16, :, r::8],
                      in_=rowi16_v[16 * r:16 * r + 16, :, :])
    for gsz in (16, 32, 64):
        eng.dma_start(out=wrapped[gsz:2 * gsz, :, :],
                      in_=wrapped[0:gsz, :, :])


def _quad_build(nc, src_dram, dst_dram, pp, chans, b0=0, b1=NB,
                parities=None, eng=None):
    """DRAM->DRAM DMAs (3-dim APs) building the quad-parity block table
    (by-outer layout: row = 4*NB*by + NB*(2*ay+ax) + bx) for block rows
    by in [b0, b1); optionally only some (ay, ax) parities."""
    q = 2 * pp
    nb = b1 - b0
    for ay in range(2):
        for ax in range(2):
            if parities is not None and (ay, ax) not in parities:
                continue
            s = ay * 2 + ax
            for yy in range(2):
                r0 = (ay + yy) * pp + ax + b0 * q
                sv = src_dram[:][r0:r0 + nb * q, :] \
                    .rearrange('(by q) c -> by q c', q=q)[:, 0:2 * NB, :] \
                    .rearrange('by (bx xx) c -> by bx (xx c)', xx=2)
                dv = dst_dram[4 * NB * b0:4 * NB * b1,
                              2 * chans * yy:2 * chans * (yy + 1)] \
                    .rearrange('(by sx) e -> by sx e', sx=4 * NB)[
                        :, NB * s:NB * s + NB, :]
                (eng or nc.sync).dma_start(out=dv, in_=sv)


def _xpar_build(nc, src_dram, dst_dram, p0, p1, eng=None):
    """x-parity table for the 1x1 deform: dst row 68*py + 34*ax + px2 holds
    the 128 f16 of padded positions (68*py + 2*px2 + ax, +1). One DMA per ax
    with 8.5KB-contiguous dst runs; builds py in [p0, p1)."""
    sflat = src_dram[:].rearrange('r c -> (r c)')
    for ax in range(2):
        base = 64 * (PP1 * p0 + ax)
        sv = sflat[base:base + (p1 - p0) * 64 * PP1] \
            .rearrange('(py q) -> py q', q=64 * PP1) \
            .rearrange('py (px2 e) -> py px2 e', e=128)[:, 0:NB, :]
        dv = dst_dram[PP1 * p0:PP1 * p1, :] \
            .rearrange('(py sx) e -> py sx e', sx=PP1)[:, NB * ax:
                                                       NB * ax + NB, :]
        (eng or nc.sync).dma_start(out=dv, in_=sv)


def _pad_memset(nc, t, npart, pp, pad, w, ncols):
    """Zero only the padding cells of a padded image tile t [npart, ncols]."""
    head = pad * pp + pad
    nc.gpsimd.memset(t[0:npart, 0:head], 0.0)
    gapw = pp - w
    r0, r1 = pad, pad + w  # gap r covers trail of row r / lead of row r+1
    ngap = r1 - r0 - 1
    gv = _rows(t[0:npart, :], r0 * pp + pad + w, pp, ngap, gapw)
    nc.gpsimd.memset(gv, 0.0)
    tail0 = (r1 - 1) * pp + pad + w
    nc.gpsimd.memset(t[0:npart, tail0:ncols], 0.0)



def _body(nc, tc, x_in, consts, out_ext, hT_dram, quad3,
          h1Tx, h1Tm, quad1x, quad1m, gate):
    V, S, G, PE, SY = nc.vector, nc.scalar, nc.gpsimd, nc.tensor, nc.sync

    with (
        tc.tile_pool(name='persist', bufs=1) as pp,
        tc.tile_pool(name='psum', bufs=2, space='PSUM') as psp,
        tc.tile_pool(name='psumv', bufs=2, space='PSUM') as psv,
    ):
        # ---------------- constants / persistent tiles -------------------
        blobs = {}
        for name, shape, dt in CONST_SPECS:
            t = pp.tile(list(shape), dt, tag='c_' + name, name='c_' + name)
            blobs[name] = t
        C = {}
        for i, (n, p_) in enumerate(_VEC_SPECS):
            C[n] = blobs['cvec'][0:p_, i:i + 1]
        col = 0
        for n, sz in _MAP_SPECS:
            C[n] = blobs['cmap'][:, col:col + sz]
            col += sz
        col = 0
        for n, sz in _W_SPECS:
            C[n] = blobs['cw'][:, col:col + sz]
            col += sz
        idt32 = pp.tile([32, 32], F32, tag='idt32', name='idt32')
        idt128h = pp.tile([128, 128], F16, tag='idt128h', name='idt128h')
        h1pad = pp.tile([128, HT1_COLS], F16, tag='h1pad', name='h1pad')
        coefD = pp.tile([128, 9, NCH, 4, 2], F16, tag='coefD', name='coefD')
        coef1D = pp.tile([128, 2, NCH, 2, 2], F16, tag='coef1D',
                 name='coef1D')
        wrapped3 = pp.tile([128, 9, 256], I16, tag='wrapped3', name='wrapped3')
        wrapped1 = pp.tile([128, 2, 256], I16, tag='wrapped1', name='wrapped1')
        h1Tst = pp.tile([128, 37, 128], F16, tag='h1Tst', name='h1Tst')
        off3T = pp.tile([128, NCH, 18], F32, tag='off3T', name='off3T')
        off1T = pp.tile([128, NCH, 2], F32, tag='off1T', name='off1T')

        # ---------------- phase A: bn3, transposes, conv, coords ---------
        with tc.tile_pool(name='ph1', bufs=1) as ph1, \
             tc.tile_pool(name='oev', bufs=2) as oev:
            x2 = ph1.tile([64, HW], F32, tag='x2', name='x2')
            hpad2 = ph1.tile([128, HT3_COLS], F16, tag='hpad2',
                             name='hpad2')
            # cvec first (gates bn3), then x, then cw/cmap (needed later)
            SY.dma_start(out=blobs['cvec'][:, :], in_=consts['cvec'][:])
            for q4 in range(4):
                SY.dma_start(out=x2[:, 1024 * q4:1024 * q4 + 1024],
                             in_=x_in[:][:, 1024 * q4:1024 * q4 + 1024])
            SY.dma_start(out=blobs['cw'][:, :], in_=consts['cw'][:])
            make_identity(nc, idt32[:, :])
            make_identity(nc, idt128h[:, :])
            _pad_memset(nc, hpad2, 128, PP3, PAD3, W, HT3_COLS)
            _pad_memset(nc, h1pad, 128, PP1, PAD1, W, HT1_COLS)
            xv = x2[:, :].rearrange('p (r c) -> p r c', r=H)

            # bn3 in two row-chunks (pipelines with the x DMA halves);
            # partitions 64:128 hold the same rows shifted up one padded
            # row so the conv can pair taps (ky, ky+1) with K=128
            for q4 in range(4):
                S.activation(_rows(hpad2[0:64, :],
                                   (PAD3 + 16 * q4) * PP3 + PAD3,
                                   PP3, 16, W),
                             xv[0:64, 16 * q4:16 * q4 + 16, :], AF.Relu,
                             bias=C['t3'], scale=C['s3'])
            for half in range(2):
                c0 = (PAD3 - 1 + 32 * half) * PP3
                SY.dma_start(out=hpad2[64:128, c0:c0 + 32 * PP3],
                             in_=hpad2[0:64, c0 + PP3:c0 + 33 * PP3])
            # hT transposes: [64, 128] -> [128, 64] per 128-col chunk;
            # store + quad3 build in two stages so the table is ready early
            hTst = ph1.tile([128, 39, 64], F16, tag='hTst', name='hTst')
            for t0 in range(0, 39, 4):
                nt = min(4, 39 - t0)
                pv = psv.tile([128, 512], F16, tag='pv', name='pv')
                for j in range(nt):
                    PE.transpose(pv[:, 64 * j:64 * j + 64],
                                 hpad2[0:64,
                                       128 * (t0 + j):128 * (t0 + j) + 128],
                                 idt128h[0:64, 0:64])
                if (t0 // 4) % 2 == 1:
                    V.tensor_copy(out=hTst[:, t0:t0 + nt, :],
                                  in_=pv[:, 0:64 * nt])
                else:
                    S.activation(hTst[:, t0:t0 + nt, :],
                                 pv[:, 0:64 * nt]
                                 .rearrange('p (a b) -> p a b', b=64),
                                 AF.Identity)
                if t0 == 16:
                    SY.dma_start(out=hT_dram[0:2560, :]
                                 .rearrange('(a p) c -> p a c', p=128),
                                 in_=hTst[:, 0:20, :])
                    _quad_build(nc, hT_dram, quad3, PP3, 64, 0, 17)
            SY.dma_start(out=hT_dram[2560:4992, :]
                         .rearrange('(a p) c -> p a c', p=128),
                         in_=hTst[:, 20:39, :])
            _quad_build(nc, hT_dram, quad3, PP3, 64, 17, NB)

            # conv3x3: 2 blocks x 4 psum chunks x 6 groups
            # (K=128 tap-pairs (ky=0,1) + K=64 singles (ky=2) per kx)
            for blk in range(2):
                pcs = [psp.tile([128, 512], F32, tag=f'pmm{i}', name=f'pc{i}',
                                bufs=1) for i in range(4)]
                for g in range(6):
                    kx, sub = g // 2, g % 2
                    ky = 0 if sub == 0 else 2
                    npt = 128 if sub == 0 else 64
                    for i in range(4):
                        ch = 4 * blk + i
                        rhs = _rows(hpad2[0:npt, :],
                                    (2 + ky) * PP3 + 2 + kx + 8 * PP3 * ch,
                                    PP3, 8, W)
                        PE.matmul(pcs[i][0:18, :],
                                  C['wC'][0:npt, 18 * g:18 * g + 18],
                                  rhs, start=(g == 0), stop=(g == 5))
                for i in range(4):
                    ch = 4 * blk + i
                    o3 = oev.tile([18, 512], F32, tag='o3', name='o3')
                    S.activation(o3[:, :], pcs[i][0:18, :], AF.Identity,
                                 bias=C['boff3'])
                    pt = psp.tile([128, 128], F32, tag='ptr', name='pt',
                                  bufs=1)
                    for t in range(4):
                        PE.transpose(pt[:, 18 * t:18 * t + 18],
                                     o3[:, 128 * t:128 * t + 128],
                                     idt32[0:18, 0:18])
                    V.tensor_copy(out=off3T[:, 4 * ch:4 * ch + 4, :]
                                  .rearrange('p a b -> p (a b)'),
                                  in_=pt[:, 0:72])

            # cmap lands here: first needed by the coords row math
            SY.dma_start(out=blobs['cmap'][:, :], in_=consts['cmap'][:])
            # coordinates / gather rows -> wrap matmuls / coefficients
            sc = [ph1.tile([128, 288], F32, tag=f'sc{i}', name=f'sc{i}')
                  for i in range(4)]
            rowf3 = ph1.tile([128, 2, 9, 16], F32, tag='rowf3', name='rowf3')
            scv = [s[:, :].rearrange('p (a b) -> p a b', b=9) for s in sc]
            wr3v = wrapped3[:, :, :].rearrange('p k (h c r) -> p h k c r',
                                               h=2, r=8)
            cmv = [C[n].rearrange('p (a b) -> p a b', b=9)
                   for n in ('c03', 'cB3')]
            for hf in range(2):
                cs = slice(16 * hf, 16 * hf + 16)
                _coords_rows(nc,
                             [sv[:, cs, :] for sv in scv],
                             off3T[:, cs, 0:18:2], off3T[:, cs, 1:18:2],
                             cmv[0][:, cs, :], cmv[1][:, cs, :],
                             rowf3[:, hf, :, :].transpose([0, 2, 1]))
                for r in range(8):
                    ptag = ['pmm0', 'pmm1', 'pmm2', 'pmm3'][r % 4]
                    pw = psp.tile([128, 512], F32, tag=ptag, name='pw',
                                  bufs=1)
                    PE.matmul(pw[:, 0:144],
                              C['wrapR'][:, 128 * r:128 * r + 128],
                              rowf3[:, hf, :, :], start=True, stop=True)
                    if r % 2 == 0:
                        S.activation(wr3v[:, hf, :, :, r], pw[:, 0:144],
                                     AF.Identity)
                    else:
                        V.tensor_copy(out=wr3v[:, hf, :, :, r],
                                      in_=pw[:, 0:144])
                _coords_coefs(nc,
                              [sv[:, cs, :] for sv in scv],
                              off3T[:, cs, 0:18:2], off3T[:, cs, 1:18:2],
                              coefD[:, :, cs, :, :].transpose([0, 2, 1, 3, 4]))

            # h1 x-part: relu(bn1(x)) into h1pad interior (ACT, off path)
            S.activation(_rows(h1pad[0:64, :], PAD1 * PP1 + PAD1, PP1, H, W),
                         xv[0:64], AF.Relu, bias=C['t1x'], scale=C['s1x'])
            # h1Tst x-half transposes + x-side table build (DMA idle here)
            for t0 in range(0, 37, 4):
                nt = min(4, 37 - t0)
                pv = psv.tile([128, 512], F16, tag='pv', name='pv')
                for j in range(nt):
                    PE.transpose(pv[:, 64 * j:64 * j + 64],
                                 h1pad[0:64, 128 * (t0 + j):
                                       128 * (t0 + j) + 128],
                                 idt128h[0:64, 0:64])
                S.activation(h1Tst[:, t0:t0 + nt, 0:64],
                             pv[:, 0:64 * nt], AF.Identity)

        # ---------------- phase C: gathers + in-place combine ------------
        vhgs = {}
        pend_fadd = []
        with tc.tile_pool(name='gpool', bufs=4) as gp, \
             tc.tile_pool(name='vhp', bufs=3) as vhp:
            for pos, k in enumerate([0, 1, 8, 2, 3, 4, 5, 6, 7]):
                gp_i, sl = k // 2, k % 2
                if gp_i not in vhgs:
                    nt = 2 if gp_i < 4 else 1
                    tag = 'vh' if gp_i < 4 else 'vh4'
                    vhgs[gp_i] = vhp.tile([128, NCH, nt, 64], F16, tag=tag,
                                          name=f'vh{gp_i}')
                vhg = vhgs[gp_i]
                g = gp.tile([128, 8192], F16, tag='g', name='g')
                G.dma_gather(g[:, :].rearrange('p (a c) -> p a c', c=256),
                             quad3[:], wrapped3[:, k, :], 4096, 4096,
                             256, queue_num=0, single_packet=False)
                if pos == 6:
                    # x-side table: built late in phase C where the gather
                    # stream has accumulated DVE-pace slack; the dummy
                    # rewrite gates the store on tap 4's combine so the
                    # scheduler can't hoist these DMAs into the ramp
                    V.scalar_tensor_tensor(out=h1Tst[0:1, 0, 0:16],
                                           in0=vhgs[2][0:1, 0, 0, 0:16],
                                           scalar=0.0,
                                           in1=h1Tst[0:1, 0, 0:16],
                                           op0=ALU.mult, op1=ALU.add)
                    SY.dma_start(out=h1Tx[0:4736, :]
                                 .rearrange('(a p) c -> p a c', p=128),
                                 in_=h1Tst[:, :, 0:64])
                    _xpar_build(nc, h1Tx, quad1x, 0, PP1)

                gq = g[:, :].rearrange('p (a b c d) -> p a b c d',
                                       a=NCH, b=4, c=32)
                cf = coefD[:, k][:, :, :, None, :].broadcast_to(
                    [128, NCH, 4, 32, 2])
                gw = g[:, :].rearrange('p (a b c) -> p a b c', a=NCH, b=4)
                nspl = 2 if pos == 8 else 1
                nh = NCH // nspl
                for sp in range(nspl):
                    chs = slice(nh * sp, nh * sp + nh)
                    V.tensor_tensor(out=gq[:, chs], in0=gq[:, chs],
                                    in1=cf[:, chs], op=ALU.mult)
                    with nc.allow_low_precision('fp16 middle precision'):
                        V.tensor_tensor(out=gw[:, chs, 0:2, :],
                                        in0=gw[:, chs, 0:2, :],
                                        in1=gw[:, chs, 2:4, :], op=ALU.add)
                        V.tensor_tensor(out=vhg[:, chs, sl, :],
                                        in0=gw[:, chs, 0, :],
                                        in1=gw[:, chs, 1, :], op=ALU.add)

            # ------------ phase C2/D: transposes + einsum3 ---------------
            sc1 = [pp.tile([128, 32], F32, tag=f't1s{i}', name=f't1s{i}')
                   for i in range(4)]
            rowf = pp.tile([128, 32], F32, tag='rowf', name='rowf')
            rowt = pp.tile([128, 16, 8], F32, tag='rowt', name='rowt')
            vhat1 = pp.tile([128, NCH, 2, 64], F16, tag='vhat1',
                            name='vhat1')
            wr1v = wrapped1[:, :, :].rearrange('p a (c r) -> p a c r', r=8)

            def _coords1_q(q):
                hs = slice(8 * q, 8 * q + 8)
                _coords_rows(nc, [s[:, hs] for s in sc1],
                             off1T[:, hs, 0], off1T[:, hs, 1],
                             C['c01'][:, hs], C['cB1'][:, hs], rowf[:, hs])
                pw1 = psp.tile([128, 512], F32, tag='ptr', name='pw1',
                               bufs=1)
                for r in range(8):
                    PE.matmul(pw1[:, 8 * r:8 * r + 8],
                              C['wrapR'][:, 128 * r:128 * r + 128],
                              rowf[:, hs], start=True, stop=True)
                pwv = pw1[:, 0:64].rearrange('p (r c) -> p c r', c=8)
                V.tensor_copy(out=wr1v[:, 0, hs, :], in_=pwv)
                # second y-corner row sits one padded row (+PP1) below
                V.tensor_scalar(out=rowt[:, 0:8, :], in0=pwv,
                                scalar1=float(PP1), scalar2=None,
                                op0=ALU.add)
                V.tensor_copy(out=wr1v[:, 1, hs, :], in_=rowt[:, 0:8, :])
                _coords_coefs(nc, [s[:, hs] for s in sc1],
                              off1T[:, hs, 0], off1T[:, hs, 1],
                              coef1D[:, :, hs, :, :]
                              .transpose([0, 2, 1, 3, 4]),
                              split_corners=True)

            def _gather1(quadap, c0, nch):
                """Gather chunks [c0, c0+nch) of the 1x1 deform from the
                x-parity table: one gather per y-corner a, each row holding
                the two x-corners for 64 channels."""
                g1 = gp.tile([128, 8192], F16, tag='g', name='g1')
                for a in range(2):
                    G.dma_gather(g1[:, 4096 * a:4096 * a + 128 * nch]
                                 .rearrange('p (c e) -> p c e', e=128),
                                 quadap,
                                 wrapped1[:, a, 8 * c0:8 * (c0 + nch)],
                                 128 * nch, 128 * nch, 128, queue_num=0,
                                 single_packet=False)
                return g1[:, :].rearrange('p (a q) -> p a q', a=2)[
                    :, :, 0:128 * nch] \
                    .rearrange('p a (c b e) -> p a c b e', b=2, e=64)

            def _combine1(g1v, hx, c0, nch):
                for a in range(2):
                    gq = g1v[:, a].rearrange('p c b (f d) -> p c b f d', d=2)
                    cf = coef1D[:, a, c0:c0 + nch, :, :][
                        :, :, :, None, :].broadcast_to([128, nch, 2, 32, 2])
                    V.tensor_tensor(out=gq, in0=gq, in1=cf, op=ALU.mult)
                with nc.allow_low_precision('fp16 by design'):
                    V.tensor_tensor(out=g1v[:, 0], in0=g1v[:, 0],
                                    in1=g1v[:, 1], op=ALU.add)
                    V.tensor_tensor(
                        out=vhat1[:, c0:c0 + nch, hx, :],
                        in0=g1v[:, 0, :, 0, :], in1=g1v[:, 0, :, 1, :],
                        op=ALU.add)

            with tc.tile_pool(name='vpool', bufs=1) as vp:
                v = vp.tile([128, 5, HW], F16, tag='v', name='v')
                pms = {}

                def _mm3(ch, gp_i):
                    if ch not in pms:
                        pms[ch] = psp.tile([128, 512], F32,
                                           tag=f'pmm{ch % 4}',
                                           name=f'pm{ch}', bufs=1)
                    if gp_i < 4:
                        PE.matmul(pms[ch][64:128, :],
                                  C['wd3T'][:, 64 * gp_i:64 * gp_i + 64],
                                  v[:, gp_i, 512 * ch:512 * ch + 512],
                                  start=(gp_i == 0), stop=False)
                    else:
                        PE.matmul(pms[ch][64:128, :],
                                  C['wd3T'][0:64, 256:320],
                                  v[0:64, 4, 512 * ch:512 * ch + 512],
                                  start=False, stop=True)

                def _vtrans(gp_i, ch4):
                    pv = psv.tile([128, 512], F16, tag='pv', name='pv')
                    for sub in range(4):
                        ch = 4 * ch4 + sub
                        if gp_i < 4:
                            PE.transpose(
                                pv[:, 128 * sub:128 * sub + 128],
                                vhgs[gp_i][:, ch, :, :]
                                .rearrange('p a b -> p (a b)'),
                                idt128h[:, :])
                        else:
                            PE.transpose(
                                pv[0:64, 128 * sub:128 * sub + 128],
                                vhgs[4][:, ch, 0, :], idt128h[:, :])
                    np_ = 128 if gp_i < 4 else 64
                    if gp_i == 3 and ch4 % 2 == 1:
                        # last group lands post-combine; DVE is free then
                        V.tensor_copy(out=v[0:np_, gp_i,
                                            512 * ch4:512 * ch4 + 512],
                                      in_=pv[0:np_, :])
                    else:
                        S.activation(v[0:np_, gp_i,
                                       512 * ch4:512 * ch4 + 512],
                                     pv[0:np_, :], AF.Identity)

                for gp_i in (0, 4, 1, 2, 3):
                    for ch4 in range(8):
                        _vtrans(gp_i, ch4)
                    for ch in range(4):
                        _mm3(ch, gp_i)

                # ---- phase E: evacs + off1 + tables + 1x1 gathers -------
                estate = {'tch': 0, 'pt1': None}
                with tc.tile_pool(name='oev1', bufs=2) as oev1:

                    def _evac_chunk(ch):
                        hv = _rows(h1pad[64:128, :],
                                   (8 * ch + PAD1) * PP1 + PAD1, PP1, 8, W)
                        pmv = pms[ch][64:128, :] \
                            .rearrange('p (r c) -> p r c', r=8)
                        S.activation(hv, pmv, AF.Relu, bias=C['t1m'],
                                     scale=C['s1m'])
                        pc1 = psp.tile([128, 512], F32, tag='pc1',
                                       name='pc1', bufs=1)
                        PE.matmul(pc1[0:2, :], C['woff1T'],
                                  _rows(h1pad[0:128, :],
                                        (8 * ch + PAD1) * PP1 + PAD1,
                                        PP1, 8, W),
                                  start=True, stop=True)
                        o1 = oev1.tile([2, 512], F32, tag='o1', name='o1')
                        if ch % 2 == 0:
                            S.activation(o1[:, :], pc1[0:2, :], AF.Identity,
                                         bias=C['boff1'])
                        else:
                            V.tensor_scalar(out=o1[:, :], in0=pc1[0:2, :],
                                            scalar1=C['boff1'], scalar2=None,
                                            op0=ALU.add)
                        if ch % 2 == 0:
                            estate['pt1'] = psp.tile([128, 128], F32,
                                                     tag='ptr', name='pt1',
                                                     bufs=1)
                        pt1 = estate['pt1']
                        for t in range(4):
                            PE.transpose(pt1[:, 8 * (ch % 2) + 2 * t:
                                             8 * (ch % 2) + 2 * t + 2],
                                         o1[:, 128 * t:128 * t + 128],
                                         idt32[0:2, 0:2])
                        if ch % 2 == 1:
                            V.tensor_copy(out=off1T[:, 8 * (ch // 2):
                                                    8 * (ch // 2) + 8, :]
                                          .rearrange('p a b -> p (a b)'),
                                          in_=pt1[:, 0:16])
                        # h1Tst mid-half transposes ready with this chunk
                        r_hi_ready = 2 + 8 * ch + 8
                        ready = []
                        while estate['tch'] < 37:
                            tch = estate['tch']
                            r_hi = (128 * tch + 127) // PP1
                            if r_hi >= r_hi_ready and ch < 7:
                                break
                            ready.append(tch)
                            estate['tch'] += 1
                        for i0 in range(0, len(ready), 4):
                            grp = ready[i0:i0 + 4]
                            pv = psv.tile([128, 512], F16, tag='pv',
                                          name='pv')
                            for j, tch in enumerate(grp):
                                PE.transpose(pv[:, 64 * j:64 * j + 64],
                                             h1pad[64:128,
                                                   128 * tch:128 * tch + 128],
                                             idt128h[64:128, 64:128])
                            t0g = grp[0]
                            estate['evp'] = estate.get('evp', 0) + 1
                            if estate['evp'] % 2 == 1:
                                S.activation(h1Tst[:, t0g:t0g + len(grp),
                                                   64:128],
                                             pv[:, 0:64 * len(grp)]
                                             .rearrange('p (a b) -> p a b',
                                                        b=64),
                                             AF.Identity)
                            else:
                                V.tensor_copy(out=h1Tst[:, t0g:t0g + len(grp),
                                                        64:128],
                                              in_=pv[:, 0:64 * len(grp)])
                        # staged h1Tm stores + x-parity table builds: stage s
                        # covers padded rows [QB[s], QB[s+1]) and needs h1Tst
                        # cols [QC[s], QC[s+1])
                        QB = [0, 19, 35, PP1]
                        QC = [0, 11, 19, 37]
                        st = {2: 0, 4: 1, 7: 2}.get(ch)
                        if st is not None:
                            SY.dma_start(
                                out=h1Tm[128 * QC[st]:128 * QC[st + 1], :]
                                .rearrange('(a p) c -> p a c', p=128),
                                in_=h1Tst[:, QC[st]:QC[st + 1], 64:128])
                            _xpar_build(nc, h1Tm, quad1m, QB[st], QB[st + 1])

                    _evac_chunk(0)
                    _evac_chunk(1)
                    _coords1_q(0)
                    gx0 = _gather1(quad1x[:], 0, 8)
                    _evac_chunk(2)             # store-a + build-a
                    gm0 = _gather1(quad1m[:][0:PP1 * 19, :], 0, 8)
                    _evac_chunk(3)
                    _coords1_q(1)
                    gx1 = _gather1(quad1x[:], 8, 8)
                    for gp_i in (0, 4, 1, 2, 3):
                        for ch in range(4, 8):
                            _mm3(ch, gp_i)
                    _evac_chunk(4)             # store-b + build-b
                    gm1 = _gather1(quad1m[:][0:PP1 * 35, :], 8, 8)
                    _combine1(gx0, 0, 0, 8)
                    _combine1(gm0, 1, 0, 8)
                    _evac_chunk(5)
                    _coords1_q(2)
                    gx2 = _gather1(quad1x[:], 16, 8)
                    _combine1(gx1, 0, 8, 8)
                    _combine1(gm1, 1, 8, 8)
                    _evac_chunk(6)
                    _evac_chunk(7)             # store-c + build-c
                    gm2 = _gather1(quad1m[:], 16, 8)
                    _coords1_q(3)
                    gx3 = _gather1(quad1x[:], 24, 8)
                    _combine1(gx2, 0, 16, 8)
                    _combine1(gm2, 1, 16, 8)
                    gm3 = _gather1(quad1m[:], 24, 8)
                    _combine1(gx3, 0, 24, 8)
                    _combine1(gm3, 1, 24, 8)

            # ---------------- v1 transposes + einsum1 + upsample ---------
            with tc.tile_pool(name='tailp', bufs=1) as tp:
                yd = tp.tile([32, H, 2 * W], F32, tag='yd', name='yd')
                v1s = tp.tile([128, 8, 512], F16, tag='v1s', name='v1s')
                for ch4 in range(8):
                    pv = psv.tile([128, 512], F16, tag='pv', name='pv')
                    for sub in range(4):
                        PE.transpose(pv[:, 128 * sub:128 * sub + 128],
                                     vhat1[:, 4 * ch4 + sub, :, :]
                                     .rearrange('p a b -> p (a b)'),
                                     idt128h[:, :])
                    if ch4 % 2 == 0:
                        S.activation(v1s[:, ch4, :], pv[:, :], AF.Identity)
                    else:
                        V.tensor_copy(out=v1s[:, ch4, :], in_=pv[:, :])
                    pmy = psp.tile([128, 512], F32, tag=f'pmm{ch4 % 4}',
                                   name='pmy', bufs=1)
                    PE.matmul(pmy[0:32, :], C['wd1T'], v1s[:, ch4, :],
                              start=True, stop=True)
                    pmv = pmy[0:32, :].rearrange('p (r c) -> p r c', r=8)
                    S.activation(yd[:, 8 * ch4:8 * ch4 + 8, 0::2], pmv,
                                 AF.Identity, bias=C['bd1'])
                    V.tensor_scalar(out=yd[:, 8 * ch4:8 * ch4 + 8, 1::2],
                                    in0=pmv, scalar1=C['bd1'], scalar2=None,
                                    op0=ALU.add)
                    if ch4 % 2 == 1:
                        gq = ch4 // 2
                        SY.dma_start(out=out_ext[:, 32 * gq:32 * gq + 32:2,
                                                 :],
                                     in_=yd[:, 16 * gq:16 * gq + 16, :])
                        SY.dma_start(out=out_ext[:,
                                                 32 * gq + 1:32 * gq + 32:2,
                                                 :],
                                     in_=yd[:, 16 * gq:16 * gq + 16, :])



# --------------------------------------------------------------------------
# host entry point
# --------------------------------------------------------------------------

_CACHE = {}


def kernel(**inputs):
    x = np.ascontiguousarray(inputs['x'], np.float32)      # [8, 64, 64, 64]
    B = x.shape[0]
    consts = host_constants(inputs)

    if 'nc' not in _CACHE:
        _CACHE['nc'] = build_nc()
    nc = _CACHE['nc']

    packed = pack_constants(consts)
    in_maps = []
    for b in range(B):
        m = {'x': x[b].reshape(64, HW)}
        for name, shape, dt in CONST_SPECS:
            m['c_' + name] = packed[name]
        in_maps.append(m)

    res = run_bass_kernel_spmd(nc, in_maps, list(range(B)))
    out = np.stack([res.results[b]['out'] for b in range(B)])
    return out.astype(np.float32)

